# revision 47
# baseline (speedup 1.0000x reference)
"""Trainium (trn2) Bass kernel for a 2-layer GAT over N=100k nodes / E=1.7M edges.

Strategy (degree-sorted edge grids + identity-stationary PE accumulation)
-------------------------------------------------------------------------
Nodes are sorted by in-degree on the host and packed into windows of 128
similar-degree destination nodes; windows are dealt round-robin across the 8
NeuronCores.  Each window's edges form a dense grid [128 nodes x D slots]
(D = max in-window degree, padded slots carry -inf logits so exp()==0), so
slot j of all 128 nodes is a 128-edge tile whose destination map is the
IDENTITY: the tensor engine accumulates the per-slot message tiles straight
into the window's PSUM bank with a never-changing fp8 identity stationary.
Degree sorting keeps grid padding at ~1.3%, and the one-hot selection stream
of the classic dst-sorted formulation (128 B/edge of pure index overhead)
disappears entirely.

Each GAT layer runs as TWO SPMD kernels with host-side index gathers (pure
permutations / casts - no host FLOPs) between them:

* node kernel (P0/P2): h = x @ W plus folded attention logits computed once
  per node (dense matmuls).  The full per-core input/output panels live in
  SBUF, loaded/stored with a handful of fat DMAs (per-chunk 1 KB/partition
  DMAs were latency-bound at ~140 GB/s); every DMA rides the SP queue since
  a queued DMA holds its issuing engine's sequencer for the whole transfer.
  P0's 16 logit rows stack two chunks per PSUM bank at partitions 0/32
  (tile_position) so one PSUM->SBUF copy drains two matmuls; P2 computes the
  inter-layer ELU as exp (one fat ACT op per quarter-panel, emitted a
  quarter ahead) + two 2x DVE ops, with PSUM copies balanced across ACT/DVE.
* edge kernel (E1/E2): streams h[src] grids (256/128 B per edge slot) and
  al_src logit grids (16/2 B); al_dst is a tiny per-window constant for E1
  and a host-replicated per-slot stream for E2 (one group-wide DVE add
  instead of 21 window-sized ones).  Windows are processed in groups
  (sum of D <= 96/192) software-pipelined three deep: group g's DMA +
  logits + leaky-relu + exp land while g-1 runs its DVE multiply + PE
  accumulation and g-2 runs its epilogue, so no engine ever stalls on
  another's latency.  ACT writes exp(z-4) into the message tile's trailing
  8 columns ((c,h)-interleaved broadcast for layer 1's 8 heads, an 8x
  replica for layer 2's single head so the DVE multiply keeps its
  packed-innermost 2x mode).  Epilogues drain each window's PSUM with a
  single f16 ACT copy, then one reciprocal + one scale per group.

Measured per-core DMA floor is ~343 GB/s on one queue / ~355 on two (HBM
fair share); the edge kernels stream ~62/~32 MB per core per inference and
run within ~15% of that floor.

Environment workarounds: this container's walrus build allows only ONE
semaphore wait per instruction (split onto nop carriers post-scheduling), and
the GPSIMD ucode libraries are absent (so no dma_gather/indirect-DMA fast
paths - hence the host-gather design).
"""
import numpy as np

import concourse.bass as bass
import concourse.mybir as mybir
import concourse.tile as tile
from concourse.bass_utils import run_bass_kernel_spmd

P = 128
F16 = mybir.dt.float16
F32 = mybir.dt.float32
F8 = mybir.dt.float8e4
AF = mybir.ActivationFunctionType
OP = mybir.AluOpType
NEG_SLOPE = 0.2
EXP_BIAS = -4.0     # exp(z + EXP_BIAS): constant shift cancels in softmax
NEG_INF = -60000.0  # pad-slot logit: exp(lrelu(.)+bias) underflows to 0
N_CORES = 8
EPS = 1e-30
CH = 448            # node-kernel matmul chunk (PSUM: 448*4B <= 2KB bank)
GCAP1, NWG1 = 96, 12     # E1 group capacity (sum of D's / max windows)
GCAP2, NWG2 = 192, 21    # E2 group capacity (smaller tiles -> fatter groups)

# ------------------------------------------------------------------ patches

_wsplit_counter = [0]


def _split_excess_waits(nc, max_waits=1):
    """This walrus build rejects >1 sem-wait per instruction ("Too many sync
    wait commands"). Move overflow waits onto same-engine nop carriers."""
    n_split = 0
    for f in nc.m.functions:
        for blk in f.blocks:
            changed = False
            out = []
            for inst in blk.instructions:
                si = inst.sync_info
                if si is not None and len(si.on_wait) > max_waits:
                    waits = list(si.on_wait)
                    keep = waits[len(waits) - max_waits:]
                    overflow = waits[: len(waits) - max_waits]
                    for i in range(0, len(overflow), max_waits):
                        _wsplit_counter[0] += 1
                        nop = mybir.InstNoOp(
                            name=f"I-wsplit-{_wsplit_counter[0]}", ins=[], outs=[])
                        nop.engine = inst.engine
                        nop.sync_info = mybir.SyncInfo(
                            on_wait=overflow[i: i + max_waits], on_update=[])
                        out.append(nop)
                    inst.sync_info = mybir.SyncInfo(
                        on_wait=keep, on_update=list(si.on_update))
                    changed = True
                    n_split += 1
                out.append(inst)
            if changed:
                blk.instructions = out
    return n_split


def _finalize_kernel(nc):
    import bass_rust as _bass_rust
    from concourse.library_config import all_libraries, standard
    from concourse.library_overlay import lower_extended_insts

    inst_type_to_lib_mask = {}
    for lib in all_libraries:
        for inst_type in lib.instructions:
            inst_type_to_lib_mask[inst_type] = inst_type_to_lib_mask.get(
                inst_type, 0) | (1 << lib.index)
    _bass_rust.insert_library_loads(
        nc, inst_type_to_lib_mask, len(all_libraries), standard.index)
    lower_extended_insts(nc)
    _split_excess_waits(nc)


# ------------------------------------------------------------------ host prep

class _Graph:
    """Degree-sorted grid preprocessing: sort nodes by in-degree, pack 128
    similar-degree nodes per window, deal windows round-robin across cores
    (slot i of every core shares one padded depth D_i so all cores run one
    identical SPMD program), and scatter each node's edges into its grid row.
    """

    def __init__(self, edge_index, n_nodes, n_cores):
        self.N = n_nodes
        self.C = n_cores
        src = np.asarray(edge_index[0], dtype=np.int64)
        dst = np.asarray(edge_index[1], dtype=np.int64)
        E = src.shape[0]

        deg = np.bincount(dst, minlength=n_nodes)
        order = np.argsort(deg, kind="stable")

        n_win_total = (n_nodes + P - 1) // P
        self.wpc = (n_win_total + n_cores - 1) // n_cores
        n_win = self.wpc * n_cores
        self.n_pad = n_win * P
        self.shard_nodes = self.wpc * P
        n_dummy = self.n_pad - n_nodes

        snode = np.full(self.n_pad, -1, dtype=np.int64)
        snode[n_dummy:] = order                      # ascending degree
        # rows_nodes[k][i, e] = natural node id at (core k, slot i, row e)
        self.rows_nodes = np.ascontiguousarray(
            snode.reshape(self.wpc, n_cores, P).transpose(1, 0, 2))

        wdeg = np.where(snode >= 0, deg[np.clip(snode, 0, None)], 0)
        wmax = wdeg.reshape(self.wpc, n_cores, P).max(axis=2)   # [wpc, cores]
        self.D = np.maximum(wmax.max(axis=1), 1).astype(np.int64)  # [wpc]
        self.off = np.concatenate([[0], np.cumsum(self.D)])
        self.TOT = int(self.D.sum())

        # position of each node in the sorted layout
        posq = np.empty(n_nodes, dtype=np.int64)
        posq[order] = np.arange(n_nodes) + n_dummy

        # scatter edges (dst-sorted, ranked within dst run) into grids
        perm = np.argsort(dst, kind="stable")
        src_s = src[perm]
        dst_s = dst[perm]
        bounds = np.searchsorted(dst_s, np.arange(n_nodes + 1))
        j_e = np.arange(E) - bounds[dst_s]           # rank within dst run
        q_e = posq[dst_s]
        g_e = q_e // P
        row_e = q_e % P
        core_e = g_e % n_cores
        slot_e = g_e // n_cores
        flat_e = self.off[slot_e] + j_e              # grid slot within [TOT]
        self.gidx = np.zeros((n_cores, self.TOT, P), dtype=np.int32)
        self.gidx[core_e, flat_e, row_e] = (src_s + 1).astype(np.int32)

        self.groups1 = self.make_groups(GCAP1, NWG1)
        self.groups2 = self.make_groups(GCAP2, NWG2)
        self.D_key = tuple(int(d) for d in self.D)

    def make_groups(self, gcap, nwg):
        """Window groups: sum(D) <= gcap, <= nwg windows per group."""
        groups = []
        i = 0
        while i < self.wpc:
            i0, sd, nw = i, 0, 0
            while (i < self.wpc and nw < nwg
                   and (nw == 0 or sd + int(self.D[i]) <= gcap)):
                sd += int(self.D[i])
                i += 1
                nw += 1
            groups.append((i0, nw, int(self.off[i0]), sd))
        return groups

    def stream_h(self, table, core):
        """[128, TOT*C] f16 grid gather: table rows by gidx (0 = zero pad)."""
        C = table.shape[1]
        tp = np.zeros((self.N + 1, C), dtype=np.float16)
        tp[1:] = table
        arr = tp[self.gidx[core]]                    # [TOT, P, C]
        return np.ascontiguousarray(arr.transpose(1, 0, 2)).reshape(
            P, self.TOT * C)

    def stream_als(self, table, core):
        """[128, TOT*H] f16: al_src grid; pad slots -> NEG_INF so exp()==0.
        Dummy rows get one j=0 slot with logit 0 so their softmax denominator
        stays finite (their h rows are zero, so the output row is 0)."""
        H = table.shape[1]
        tp = np.full((self.N + 1, H), NEG_INF, dtype=np.float16)
        tp[1:] = table
        arr = tp[self.gidx[core]]                    # [TOT, P, H]
        i_d, e_d = np.nonzero(self.rows_nodes[core] < 0)
        arr[self.off[i_d], e_d, :] = 0.0
        return np.ascontiguousarray(arr.transpose(1, 0, 2)).reshape(
            P, self.TOT * H)

    def stream_ald(self, table, core):
        """[128, wpc*H] f16: al_dst per (window, row). Dummy rows -> 0."""
        H = table.shape[1]
        tp = np.zeros((self.N + 1, H), dtype=np.float16)
        tp[1:] = table
        arr = tp[self.rows_nodes[core] + 1]          # [wpc, P, H]
        return np.ascontiguousarray(arr.transpose(1, 0, 2)).reshape(
            P, self.wpc * H)

    def stream_ald_exp(self, table, core):
        """[128, TOT*H] f16: al_dst replicated across each window's slots
        (slot grids are per-window blocks of D_i slots)."""
        H = table.shape[1]
        tp = np.zeros((self.N + 1, H), dtype=np.float16)
        tp[1:] = table
        arr = tp[self.rows_nodes[core] + 1]          # [wpc, P, H]
        rep = np.repeat(arr, self.D, axis=0)         # [TOT, P, H]
        return np.ascontiguousarray(rep.transpose(1, 0, 2)).reshape(
            P, self.TOT * H)

    def ident8(self):
        import ml_dtypes
        return np.eye(P, dtype=np.float32).astype(ml_dtypes.float8_e4m3)


# ------------------------------------------------------------------ builders

def _build_node(SH, c_in, m_h, m_al, elu, bias_in, bench_loop=1):
    """Per-node transform: hT = (elu?(xT+b)) @ w, alT = same @ wal.
    When m_h+m_al <= 128 the two matmuls merge into one.  The whole per-core
    panel is SBUF-resident: quarters stream in with fat DMAs, chunked matmuls
    write a staged output panel, and a few fat DMAs store it."""
    merged = (m_h + m_al) <= P
    M = m_h + m_al if merged else m_h
    QN = 4
    QS = SH // QN
    NQUAD = SH // (2 * CH)        # 2 al-chunks stack into one PSUM bank
    assert SH % QN == 0 and QS % CH == 0 and SH % (2 * CH) == 0
    nc = bass.Bass()
    xT = nc.dram_tensor("xT", [c_in, SH], F16, kind="ExternalInput")
    w = nc.dram_tensor("w", [c_in, M], F16, kind="ExternalInput")
    if not merged:
        assert m_al <= 32
        wal = nc.dram_tensor("wal", [c_in, 32], F16, kind="ExternalInput")
    if bias_in:
        bvec = nc.dram_tensor("bvec", [c_in, 1], F32, kind="ExternalInput")
    hT = nc.dram_tensor("hT", [M, SH], F16, kind="ExternalOutput")
    if not merged:
        # partition-stacked al panel: row 32k+r, col cq*CH+x holds
        # al[r] of chunk 2*cq+k (host unscrambles)
        alT = nc.dram_tensor("alT", [64, NQUAD * CH], F16,
                             kind="ExternalOutput")

    with tile.TileContext(nc) as tc:
        with (
            tc.tile_pool(name="const", bufs=1) as constp,
            tc.tile_pool(name="xin", bufs=2) as xinp,
            tc.tile_pool(name="hout", bufs=2) as houtp,
            tc.tile_pool(name="work", bufs=4) as workp,
            tc.tile_pool(name="psH", bufs=5, space="PSUM") as psH,
            tc.tile_pool(name="psA", bufs=3, space="PSUM") as psA,
        ):
            w_sb = constp.tile([c_in, M], F16)
            nc.sync.dma_start(out=w_sb[:], in_=w[:])
            if not merged:
                # wal host-padded to 32 cols (zeros) so every partition of
                # the stacked al PSUM region is written (no uninit reads)
                wal_sb = constp.tile([c_in, 32], F16)
                nc.sync.dma_start(out=wal_sb[:], in_=wal[:])
            if bias_in:
                b_sb = constp.tile([c_in, 1], F32)
                nc.sync.dma_start(out=b_sb[:], in_=bvec[:])

            def body(_iv=None):
                # every DMA rides SP: a queued DMA holds its issuing engine's
                # sequencer for the whole transfer, so ACT/DVE must stay clean
                xq = [xinp.tile([c_in, QS], F16, tag=f"x{q}", name=f"xq{q}")
                      for q in range(QN)]
                for q in range(QN):
                    nc.sync.dma_start(out=xq[q][:],
                                      in_=xT[:, q * QS:(q + 1) * QS])
                hq = [houtp.tile([M, QS], F16, tag=f"h{q}", name=f"hq{q}")
                      for q in range(QN)]
                if not merged:
                    alout = houtp.tile([64, NQUAD * CH], F16, tag="alo")
                quad = {}

                def qfront(q):
                    """Quarter-granular ELU stage A: one fat ACT exp."""
                    if not elu:
                        return None
                    rhs = xq[q][:]
                    if bias_in:
                        nc.vector.tensor_scalar(
                            rhs, rhs, b_sb[:, 0:1], None, OP.add)
                    et = workp.tile([c_in, QS], F16, tag="et")
                    nc.scalar.activation(et[:], rhs, AF.Exp)
                    return et

                def qback(q, et):
                    if elu:
                        # elu(x) = (min(exp(x),1) - 1) + max(x,0), all 2x DVE
                        mn = workp.tile([c_in, QS], F16, tag="mn")
                        nc.vector.tensor_scalar(
                            mn[:], et[:], 1.0, -1.0, OP.min, OP.add)
                        mx = workp.tile([c_in, QS], F16, tag="mx")
                        nc.vector.tensor_scalar(
                            mx[:], xq[q][:], 0.0, None, OP.max)
                        xe = workp.tile([c_in, QS], F16, tag="xe")
                        nc.vector.tensor_tensor(
                            out=xe[:], in0=mn[:], in1=mx[:], op=OP.add)
                        src = xe
                    else:
                        src = xq[q]
                    for j in range(QS // CH):
                        ci = q * (QS // CH) + j
                        qo = j * CH
                        rhs = src[:, qo:qo + CH]
                        ph = psH.tile([M, CH], F32, tag="ph")
                        nc.tensor.matmul(ph[:], w_sb[:], rhs,
                                         start=True, stop=True)
                        dve_copy = (ci % 7 < 3) if elu else (ci % 2 == 1)
                        if dve_copy:
                            nc.vector.tensor_copy(hq[q][:, qo:qo + CH],
                                                  ph[:])
                        else:
                            nc.scalar.activation(hq[q][:, qo:qo + CH],
                                                 ph[:], AF.Copy)
                        if not merged:
                            # stack 2 chunks' al outputs on partitions
                            # 0/32 of one PSUM bank -> 1 copy per pair
                            k = ci % 2
                            if k == 0:
                                quad["pa"] = psA.tile([64, CH], F32,
                                                      tag="paq", name="paq")
                            pa = quad["pa"]
                            nc.tensor.matmul(pa[32 * k:32 * k + 32, :],
                                             wal_sb[:], rhs,
                                             start=True, stop=True)
                            if k == 1:
                                cq = ci // 2
                                if cq % 2 == 0:
                                    nc.vector.tensor_copy(
                                        alout[:, cq * CH:(cq + 1) * CH],
                                        pa[:])
                                else:
                                    nc.scalar.activation(
                                        alout[:, cq * CH:(cq + 1) * CH],
                                        pa[:], AF.Copy)
                    nc.sync.dma_start(out=hT[:, q * QS:(q + 1) * QS],
                                      in_=hq[q][:])

                prev = None
                for q in range(QN):
                    et = qfront(q)
                    if prev is not None:
                        qback(*prev)
                    prev = (q, et)
                qback(*prev)
                if not merged:
                    nc.sync.dma_start(out=alT[:], in_=alout[:])

            if bench_loop > 1:
                with tc.For_i(0, bench_loop, 1) as _iv:
                    body(_iv)
            else:
                body()
    _finalize_kernel(nc)
    return nc


def _build_edge_g(D_list, groups, TOT, Cc, H, bias_out=False, elu_out=False,
                  ald_exp=False, bench_loop=1):
    """Edge aggregation over degree-sorted grids.  Per group of windows:
    one h[src] grid DMA, one DVE logit add per window, one ACT leaky-relu,
    one ACT exp into the message tile's trailing EB columns, one DVE
    multiply, then D accumulating identity matmuls per window.  Epilogues
    run one group late so no engine stalls on PSUM completion."""
    EB = 8
    SLOT = Cc + EB
    G = Cc // EB
    NW = len(D_list)
    GS = max(sd for _, _, _, sd in groups)
    NWmax = max(nw for _, nw, _, _ in groups)

    nc = bass.Bass()
    hsrc = nc.dram_tensor("hsrc", [P, TOT * Cc], F16, kind="ExternalInput")
    als = nc.dram_tensor("als", [P, TOT * H], F16, kind="ExternalInput")
    ald = nc.dram_tensor("ald", [P, (TOT if ald_exp else NW) * H], F16,
                         kind="ExternalInput")
    ident = nc.dram_tensor("ident", [P, P], F8, kind="ExternalInput")
    if bias_out:
        brep = nc.dram_tensor("brep", [P, Cc], F32, kind="ExternalInput")
    out = nc.dram_tensor("out", [NW * P, Cc], F16, kind="ExternalOutput")

    with tile.TileContext(nc) as tc:
        with (
            tc.tile_pool(name="const", bufs=1) as constp,
            tc.tile_pool(name="aldp", bufs=2) as aldp,
            tc.tile_pool(name="alg", bufs=3) as algp,
            tc.tile_pool(name="hs", bufs=3) as hsp,
            tc.tile_pool(name="za", bufs=3) as zap,
            tc.tile_pool(name="msg", bufs=3) as msgp,
            tc.tile_pool(name="epi", bufs=3) as epip,
            tc.tile_pool(name="og", bufs=2) as ogp,
            tc.tile_pool(name="psW", bufs=8, space="PSUM") as pswp,
        ):
            BSLOT = 512 // SLOT      # windows per PSUM bank
            ident_sb = constp.tile([P, P], F8)
            nc.scalar.dma_start(out=ident_sb[:], in_=ident[:])
            ebias_sb = constp.tile([P, 1], F32)
            nc.vector.memset(ebias_sb[:], EXP_BIAS)
            if bias_out:
                brep_sb = constp.tile([P, Cc], F32)
                nc.scalar.dma_start(out=brep_sb[:], in_=brep[:])

            pend = []

            def front(grp, ald_sb):
                """DMA + logit add + leaky-relu + exp for one group."""
                i0, nw, off0, sd = grp
                hs = hsp.tile([P, GS * Cc], F16, tag="hs")
                nc.sync.dma_start(out=hs[:, :sd * Cc],
                                  in_=hsrc[:, off0 * Cc:(off0 + sd) * Cc])
                alg = algp.tile([P, GS * H], F16, tag="alg")
                nc.sync.dma_start(out=alg[:, :sd * H],
                                  in_=als[:, off0 * H:(off0 + sd) * H])
                za = zap.tile([P, GS * H], F16, tag="za")
                if ald_exp:
                    # host replicated al_dst per slot: one add per group
                    adx = algp.tile([P, GS * H], F16, tag="adx")
                    nc.sync.dma_start(out=adx[:, :sd * H],
                                      in_=ald[:, off0 * H:(off0 + sd) * H])
                    nc.vector.tensor_tensor(out=za[:, :sd * H],
                                            in0=alg[:, :sd * H],
                                            in1=adx[:, :sd * H], op=OP.add)
                doff = 0
                for wl in range(nw) if not ald_exp else ():
                    D = int(D_list[i0 + wl])
                    o0 = doff * H
                    if H > 1:
                        av = alg[:, o0:o0 + D * H].rearrange(
                            "p (d h) -> p d h", d=D)
                        zv = za[:, o0:o0 + D * H].rearrange(
                            "p (d h) -> p d h", d=D)
                        ad = ald_sb[:, (i0 + wl) * H:(i0 + wl + 1) * H]
                        ab = bass.AP(ad.tensor, ad.offset,
                                     [ad.ap[0], [0, D], [1, H]])
                    else:
                        av = alg[:, o0:o0 + D]
                        zv = za[:, o0:o0 + D]
                        ad = ald_sb[:, i0 + wl:i0 + wl + 1]
                        ab = bass.AP(ad.tensor, ad.offset,
                                     [ad.ap[0], [0, D]])
                    nc.vector.tensor_tensor(out=zv, in0=av, in1=ab, op=OP.add)
                    doff += D
                nc.scalar.activation(za[:, :sd * H], za[:, :sd * H],
                                     AF.Prelu, alpha=NEG_SLOPE)
                msg = msgp.tile([P, GS * SLOT], F16, tag="msg")
                m3 = msg[:, :sd * SLOT].rearrange("p (d s) -> p d s", s=SLOT)
                eb_out = m3[:, :, Cc:Cc + EB]
                if H > 1:
                    e_in = za[:, :sd * H].rearrange("p (d h) -> p d h", d=sd)
                else:
                    z0 = za[:, :sd]
                    e_in = bass.AP(z0.tensor, z0.offset,
                                   [z0.ap[0], [1, sd], [0, EB]])
                nc.scalar.activation(eb_out, e_in, AF.Exp, bias=ebias_sb[:])
                return hs, msg

            def back(grp, st):
                """DVE message multiply + PE identity accumulation."""
                i0, nw, off0, sd = grp
                hs, msg = st
                m3 = msg[:, :sd * SLOT].rearrange("p (d s) -> p d s", s=SLOT)
                eb_out = m3[:, :, Cc:Cc + EB]
                mo = m3[:, :, 0:Cc].rearrange("p d (g h) -> p d g h", h=EB)
                hi = hs[:, :sd * Cc].rearrange(
                    "p (d g h) -> p d g h", d=sd, h=EB)
                ei = bass.AP(eb_out.tensor, eb_out.offset,
                             [eb_out.ap[0], eb_out.ap[1], [0, G], [1, EB]])
                nc.vector.tensor_tensor(out=mo, in0=hi, in1=ei, op=OP.mult)
                doff = 0
                bank = None
                for wl in range(nw):
                    D = int(D_list[i0 + wl])
                    if wl % BSLOT == 0:
                        bank = pswp.tile([P, 512], F32, tag="psw",
                                         name="pswbank")
                    sl = (wl % BSLOT) * SLOT
                    psw = bank[:, sl:sl + SLOT]
                    for j in range(D):
                        mv = msg[:, (doff + j) * SLOT:(doff + j + 1) * SLOT]
                        nc.tensor.matmul(psw, ident_sb[:], mv,
                                         start=(j == 0), stop=(j == D - 1))
                    pend.append(psw)
                    doff += D

            def epilogue(grp):
                """One f16 PSUM copy per window, then a single reciprocal +
                scale + output DMA for the whole group."""
                i0, nw, off0, sd = grp
                op_t = epip.tile([P, NWmax * SLOT], F16, tag="o1p")
                for wl in range(nw):
                    psw = pend.pop(0)
                    nc.scalar.activation(op_t[:, wl * SLOT:(wl + 1) * SLOT],
                                         psw, AF.Copy)
                opv = op_t[:, :nw * SLOT]
                rec = epip.tile([P, NWmax * EB], F16, tag="rec")
                rv = rec[:, :nw * EB].rearrange("p (w h) -> p w h", w=nw)
                dap = bass.AP(opv.tensor, opv.offset + Cc,
                              [opv.ap[0], [SLOT, nw], [1, EB]])
                with nc.allow_low_precision(
                        reason="softmax denominators are O(1)"):
                    nc.vector.reciprocal(rv, dap)
                og = ogp.tile([P, NWmax * Cc], F16, tag="og")
                o_in = bass.AP(opv.tensor, opv.offset,
                               [opv.ap[0], [SLOT, nw], [EB, G], [1, EB]])
                r0 = rec[:]
                r_b = bass.AP(r0.tensor, r0.offset,
                              [r0.ap[0], [EB, nw], [0, G], [1, EB]])
                oo = og[:, :nw * Cc].rearrange(
                    "p (w g h) -> p w g h", w=nw, h=EB)
                nc.vector.tensor_tensor(out=oo, in0=o_in, in1=r_b,
                                        op=OP.mult)
                if bias_out:     # layer bias: before the inter-layer elu
                    ov2 = og[:, :nw * Cc].rearrange("p (w c) -> p w c", w=nw)
                    b0 = brep_sb[:]
                    b_b = bass.AP(b0.tensor, b0.offset,
                                  [b0.ap[0], [0, nw], [1, Cc]])
                    nc.vector.tensor_tensor(out=ov2, in0=ov2, in1=b_b,
                                            op=OP.add)
                if elu_out:
                    # elu(x) = max(x,0) + (min(exp(x),1) - 1), in place on og
                    ogv = og[:, :nw * Cc]
                    et = epip.tile([P, NWmax * Cc], F16, tag="et")
                    etv = et[:, :nw * Cc]
                    nc.scalar.activation(etv, ogv, AF.Exp)
                    nc.vector.tensor_scalar(etv, etv, 1.0, -1.0,
                                            OP.min, OP.add)
                    nc.vector.scalar_tensor_tensor(ogv, ogv, 0.0, etv,
                                                   OP.max, OP.add)
                dr = out[i0 * P:(i0 + nw) * P, :].rearrange(
                    "(w e) c -> e w c", e=P)
                nc.scalar.dma_start(
                    out=dr,
                    in_=og[:, :nw * Cc].rearrange("p (w c) -> p w c", w=nw))

            def body(_iv=None):
                if not ald_exp:
                    ald_sb = aldp.tile([P, NW * H], F16, tag="ald")
                    nc.scalar.dma_start(out=ald_sb[:], in_=ald[:])
                else:
                    ald_sb = None
                pend.clear()
                sts = [None] * len(groups)
                for gi, grp in enumerate(groups):
                    sts[gi] = front(grp, ald_sb)
                    if gi >= 1:
                        back(groups[gi - 1], sts[gi - 1])
                        sts[gi - 1] = None
                    if gi >= 2:
                        epilogue(groups[gi - 2])
                ng = len(groups)
                back(groups[ng - 1], sts[ng - 1])
                if ng >= 2:
                    epilogue(groups[ng - 2])
                epilogue(groups[ng - 1])

            if bench_loop > 1:
                with tc.For_i(0, bench_loop, 1) as _iv:
                    body(_iv)
            else:
                body()
    _finalize_kernel(nc)
    return nc


# ------------------------------------------------------------------ runner

def _fold_att(W, a):
    heads, hid = a.shape
    return np.einsum("ihc,hc->ih", W.reshape(W.shape[0], heads, hid), a)


class _GatRunner:
    def __init__(self, n_cores=N_CORES):
        self.C = n_cores
        self._graph = None
        self._graph_key = None
        self._kernels = {}
        self.last_maps = {}

    def graph(self, edge_index, n_nodes):
        key = hash(np.asarray(edge_index).tobytes())
        if key != self._graph_key:
            self._graph = _Graph(edge_index, n_nodes, self.C)
            self._graph_key = key
            self._kernels.clear()
        return self._graph

    def kernel(self, name, bench_loop=1, **kw):
        key = (name, bench_loop, tuple(sorted(kw.items())))
        if key not in self._kernels:
            g = self._graph
            if name.startswith("P"):
                self._kernels[key] = _build_node(
                    g.shard_nodes, bench_loop=bench_loop, **kw)
            elif name == "E1":
                self._kernels[key] = _build_edge_g(
                    g.D, g.groups1, g.TOT, 128, 8,
                    bench_loop=bench_loop, **kw)
            else:
                self._kernels[key] = _build_edge_g(
                    g.D, g.groups2, g.TOT, 64, 1, ald_exp=True,
                    bench_loop=bench_loop, **kw)
        return self._kernels[key]

    def _run(self, name, nc, maps):
        self.last_maps[name] = maps
        res = run_bass_kernel_spmd(nc, maps, core_ids=list(range(self.C)))
        return res.results

    def run(self, x, edge_index, W1, a_src1, a_dst1, b1, W2, a_src2, a_dst2,
            b2):
        C = self.C
        N, IN_C = x.shape
        HEADS, HID = a_src1.shape
        HC = HEADS * HID
        OUT_C = W2.shape[1]
        g = self.graph(edge_index, N)
        SH = g.shard_nodes
        # (c,h)-interleaved channel order for the layer-1 hidden features:
        # col c*H+h of h1 holds math channel h*HID+c. Folded into W1's
        # columns (P0) and W2's rows (P2) on the host - pure permutation.
        perm = np.array([(j % HEADS) * HID + j // HEADS
                         for j in range(HC)], dtype=np.int64)

        # ---- P0: per-node h1 / logits --------------------------------
        xT_pad = np.zeros((IN_C, g.n_pad), dtype=np.float16)
        xT_pad[:, :N] = np.asarray(x, np.float32).T
        w1 = np.asarray(W1, np.float32)
        m_al = 2 * HEADS
        wal1 = np.zeros((IN_C, 32), dtype=np.float32)
        wal1[:, :m_al] = np.concatenate(
            [_fold_att(w1, np.asarray(a_src1, np.float32)),
             _fold_att(w1, np.asarray(a_dst1, np.float32))], axis=1)
        mapsP0 = [{"xT": np.ascontiguousarray(xT_pad[:, k * SH:(k + 1) * SH]),
                   "w": np.ascontiguousarray(w1[:, perm]).astype(np.float16),
                   "wal": wal1.astype(np.float16)} for k in range(C)]
        ncP0 = self.kernel("P0", c_in=IN_C, m_h=HC, m_al=m_al,
                           elu=False, bias_in=False)
        resP0 = self._run("P0", ncP0, mapsP0)
        h1 = np.ascontiguousarray(
            np.concatenate([r["hT"] for r in resP0], axis=1).T)[:N]
        # unscramble the partition-stacked al panel: row 32k+r, col cq*CH+x
        # holds al[r] of chunk 4*cq+k
        nq = SH // (2 * CH)
        al1 = np.concatenate(
            [r["alT"].reshape(2, 32, nq, CH)[:, :m_al]
             .transpose(1, 2, 0, 3).reshape(m_al, SH)
             for r in resP0], axis=1)                    # [16, Np]
        als1 = np.ascontiguousarray(al1[:HEADS, :N].T)
        ald1 = np.ascontiguousarray(al1[HEADS:, :N].T)

        # ---- E1: layer-1 edge aggregation + bias + ELU ---------------
        id8 = g.ident8()
        b1nz = bool(np.any(np.asarray(b1)))
        mapsE1 = []
        for k in range(C):
            m = {"hsrc": g.stream_h(h1, k),
                 "als": g.stream_als(als1, k),
                 "ald": g.stream_ald(ald1, k),
                 "ident": id8}
            if b1nz:
                m["brep"] = np.tile(
                    np.asarray(b1, np.float32)[perm], (P, 1))
            mapsE1.append(m)
        ncE1 = self.kernel("E1", bias_out=b1nz)
        resE1 = self._run("E1", ncE1, mapsE1)
        out1 = np.concatenate([r["out"] for r in resE1], axis=0)
        # rows of out1 are (core, slot, row) -> natural node rowmap
        rowmap = g.rows_nodes.reshape(-1)            # [C*wpc*P]

        # ---- P2: ELU + per-node h2 / logits --------------------------
        o1T = np.ascontiguousarray(out1.T)           # [HC, C*SH] f16
        w2 = np.asarray(W2, np.float32)
        wal2 = np.concatenate(
            [_fold_att(w2, np.asarray(a_src2, np.float32)),
             _fold_att(w2, np.asarray(a_dst2, np.float32))], axis=1)
        w2all = np.concatenate([w2[perm], wal2[perm]], axis=1)  # [HC, 66]
        mapsP2 = [
            {"xT": np.ascontiguousarray(o1T[:, k * SH:(k + 1) * SH]),
             "w": w2all.astype(np.float16)} for k in range(C)]
        # out1 already carries b1 (E1 bias_out); P2 applies the ELU
        ncP2 = self.kernel("P2", c_in=HC, m_h=OUT_C, m_al=2, elu=True,
                           bias_in=False)
        resP2 = self._run("P2", ncP2, mapsP2)
        h2al = np.concatenate([r["hT"] for r in resP2], axis=1)  # [66, Np]
        valid = rowmap >= 0
        vrows = rowmap[valid]
        h2 = np.zeros((N, OUT_C), dtype=np.float16)
        h2[vrows] = h2al[:OUT_C].T[valid]
        als2 = np.zeros((N, 1), dtype=np.float16)
        als2[vrows, 0] = h2al[OUT_C][valid]
        ald2 = np.zeros((N, 1), dtype=np.float16)
        ald2[vrows, 0] = h2al[OUT_C + 1][valid]

        # ---- E2: layer-2 edge aggregation ----------------------------
        b2nz = bool(np.any(np.asarray(b2)))
        mapsE2 = []
        for k in range(C):
            m = {"hsrc": g.stream_h(h2, k),
                 "als": g.stream_als(als2, k),
                 "ald": g.stream_ald_exp(ald2, k),
                 "ident": id8}
            if b2nz:
                m["brep"] = np.tile(np.asarray(b2, np.float32), (P, 1))
            mapsE2.append(m)
        ncE2 = self.kernel("E2", bias_out=b2nz)
        resE2 = self._run("E2", ncE2, mapsE2)
        out2 = np.concatenate([r["out"] for r in resE2], axis=0)
        out_full = np.zeros((N, OUT_C), dtype=np.float32)
        out_full[vrows] = out2[valid]
        return out_full


_RUNNER = _GatRunner()


def kernel(x, edge_index, W1, a_src1, a_dst1, b1, W2, a_src2, a_dst2, b2):
    """Full-input / full-output entry point. Returns [N, OUT_C] float32."""
    args = [np.asarray(v) for v in
            (x, edge_index, W1, a_src1, a_dst1, b1, W2, a_src2, a_dst2, b2)]
    return _RUNNER.run(*args).astype(np.float32)


# revision 48
# speedup vs baseline: 1.0433x; 1.0433x over previous
"""Trainium (trn2) Bass kernel for a 2-layer GAT over N=100k nodes / E=1.7M edges.

Strategy (degree-sorted edge grids + identity-stationary PE accumulation)
-------------------------------------------------------------------------
Nodes are sorted by in-degree on the host and packed into windows of 128
similar-degree destination nodes; windows are dealt round-robin across the 8
NeuronCores.  Each window's edges form a dense grid [128 nodes x D slots]
(D = max in-window degree, padded slots carry -inf logits so exp()==0), so
slot j of all 128 nodes is a 128-edge tile whose destination map is the
IDENTITY: the tensor engine accumulates the per-slot message tiles straight
into the window's PSUM bank with a never-changing fp8 identity stationary.
Degree sorting keeps grid padding at ~1.3%, and the one-hot selection stream
of the classic dst-sorted formulation (128 B/edge of pure index overhead)
disappears entirely.

Each GAT layer runs as TWO SPMD kernels with host-side index gathers (pure
permutations / casts - no host FLOPs) between them:

* node kernel (P0/P2): h = x @ W plus folded attention logits computed once
  per node (dense matmuls).  The full per-core input/output panels live in
  SBUF, loaded/stored with a handful of fat DMAs (per-chunk 1 KB/partition
  DMAs were latency-bound at ~140 GB/s); every DMA rides the SP queue since
  a queued DMA holds its issuing engine's sequencer for the whole transfer.
  P0's 16 logit rows stack two chunks per PSUM bank at partitions 0/32
  (tile_position) so one PSUM->SBUF copy drains two matmuls; P2 computes the
  inter-layer ELU as exp (one fat ACT op per quarter-panel, emitted a
  quarter ahead) + two 2x DVE ops, with PSUM copies balanced across ACT/DVE.
* edge kernel (E1/E2): streams h[src] grids (256/128 B per edge slot) and
  al_src logit grids (16/2 B); al_dst is a tiny per-window constant for E1
  and a host-replicated per-slot stream for E2 (one group-wide DVE add
  instead of 21 window-sized ones).  Windows are processed in groups
  (sum of D <= 96/192) software-pipelined three deep: group g's DMA +
  logits + leaky-relu + exp land while g-1 runs its DVE multiply + PE
  accumulation and g-2 runs its epilogue, so no engine ever stalls on
  another's latency.  ACT writes exp(z-4) into the message tile's trailing
  8 columns ((c,h)-interleaved broadcast for layer 1's 8 heads, an 8x
  replica for layer 2's single head so the DVE multiply keeps its
  packed-innermost 2x mode).  Epilogues drain each window's PSUM with a
  single f16 ACT copy, then one reciprocal + one scale per group.

Measured per-core DMA floor is ~343 GB/s on one queue / ~355 on two (HBM
fair share); the edge kernels stream ~62/~32 MB per core per inference and
run within ~15% of that floor.

Environment workarounds: this container's walrus build allows only ONE
semaphore wait per instruction (split onto nop carriers post-scheduling), and
the GPSIMD ucode libraries are absent (so no dma_gather/indirect-DMA fast
paths - hence the host-gather design).
"""
import numpy as np

import concourse.bass as bass
import concourse.mybir as mybir
import concourse.tile as tile
from concourse.bass_utils import run_bass_kernel_spmd

P = 128
F16 = mybir.dt.float16
F32 = mybir.dt.float32
F8 = mybir.dt.float8e4
AF = mybir.ActivationFunctionType
OP = mybir.AluOpType
NEG_SLOPE = 0.2
EXP_BIAS = -4.0     # exp(z + EXP_BIAS): constant shift cancels in softmax
NEG_INF = -60000.0  # pad-slot logit: exp(lrelu(.)+bias) underflows to 0
N_CORES = 8
EPS = 1e-30
CH = 448            # node-kernel matmul chunk (PSUM: 448*4B <= 2KB bank)
GCAP1, NWG1 = 96, 12     # E1 group capacity (sum of D's / max windows)
GCAP2, NWG2 = 192, 21    # E2 group capacity (smaller tiles -> fatter groups)

# ------------------------------------------------------------------ patches

_wsplit_counter = [0]


def _split_excess_waits(nc, max_waits=1):
    """This walrus build rejects >1 sem-wait per instruction ("Too many sync
    wait commands"). Move overflow waits onto same-engine nop carriers."""
    n_split = 0
    for f in nc.m.functions:
        for blk in f.blocks:
            changed = False
            out = []
            for inst in blk.instructions:
                si = inst.sync_info
                if si is not None and len(si.on_wait) > max_waits:
                    waits = list(si.on_wait)
                    keep = waits[len(waits) - max_waits:]
                    overflow = waits[: len(waits) - max_waits]
                    for i in range(0, len(overflow), max_waits):
                        _wsplit_counter[0] += 1
                        nop = mybir.InstNoOp(
                            name=f"I-wsplit-{_wsplit_counter[0]}", ins=[], outs=[])
                        nop.engine = inst.engine
                        nop.sync_info = mybir.SyncInfo(
                            on_wait=overflow[i: i + max_waits], on_update=[])
                        out.append(nop)
                    inst.sync_info = mybir.SyncInfo(
                        on_wait=keep, on_update=list(si.on_update))
                    changed = True
                    n_split += 1
                out.append(inst)
            if changed:
                blk.instructions = out
    return n_split


def _finalize_kernel(nc):
    import bass_rust as _bass_rust
    from concourse.library_config import all_libraries, standard
    from concourse.library_overlay import lower_extended_insts

    inst_type_to_lib_mask = {}
    for lib in all_libraries:
        for inst_type in lib.instructions:
            inst_type_to_lib_mask[inst_type] = inst_type_to_lib_mask.get(
                inst_type, 0) | (1 << lib.index)
    _bass_rust.insert_library_loads(
        nc, inst_type_to_lib_mask, len(all_libraries), standard.index)
    lower_extended_insts(nc)
    _split_excess_waits(nc)


# ------------------------------------------------------------------ host prep

class _Graph:
    """Degree-sorted grid preprocessing: sort nodes by in-degree, pack 128
    similar-degree nodes per window, deal windows round-robin across cores
    (slot i of every core shares one padded depth D_i so all cores run one
    identical SPMD program), and scatter each node's edges into its grid row.
    """

    def __init__(self, edge_index, n_nodes, n_cores):
        self.N = n_nodes
        self.C = n_cores
        src = np.asarray(edge_index[0], dtype=np.int64)
        dst = np.asarray(edge_index[1], dtype=np.int64)
        E = src.shape[0]

        deg = np.bincount(dst, minlength=n_nodes)
        order = np.argsort(deg, kind="stable")

        n_win_total = (n_nodes + P - 1) // P
        self.wpc = (n_win_total + n_cores - 1) // n_cores
        n_win = self.wpc * n_cores
        self.n_pad = n_win * P
        self.shard_nodes = self.wpc * P
        n_dummy = self.n_pad - n_nodes

        snode = np.full(self.n_pad, -1, dtype=np.int64)
        snode[n_dummy:] = order                      # ascending degree
        # rows_nodes[k][i, e] = natural node id at (core k, slot i, row e)
        self.rows_nodes = np.ascontiguousarray(
            snode.reshape(self.wpc, n_cores, P).transpose(1, 0, 2))

        wdeg = np.where(snode >= 0, deg[np.clip(snode, 0, None)], 0)
        wmax = wdeg.reshape(self.wpc, n_cores, P).max(axis=2)   # [wpc, cores]
        self.D = np.maximum(wmax.max(axis=1), 1).astype(np.int64)  # [wpc]
        self.off = np.concatenate([[0], np.cumsum(self.D)])
        self.TOT = int(self.D.sum())

        # position of each node in the sorted layout
        posq = np.empty(n_nodes, dtype=np.int64)
        posq[order] = np.arange(n_nodes) + n_dummy

        # scatter edges (dst-sorted, ranked within dst run) into grids
        perm = np.argsort(dst, kind="stable")
        src_s = src[perm]
        dst_s = dst[perm]
        bounds = np.searchsorted(dst_s, np.arange(n_nodes + 1))
        j_e = np.arange(E) - bounds[dst_s]           # rank within dst run
        q_e = posq[dst_s]
        g_e = q_e // P
        row_e = q_e % P
        core_e = g_e % n_cores
        slot_e = g_e // n_cores
        flat_e = self.off[slot_e] + j_e              # grid slot within [TOT]
        self.gidx = np.zeros((n_cores, self.TOT, P), dtype=np.int32)
        self.gidx[core_e, flat_e, row_e] = (src_s + 1).astype(np.int32)

        self.groups1 = self.make_groups(GCAP1, NWG1)
        self.groups2 = self.make_groups(GCAP2, NWG2)
        self.D_key = tuple(int(d) for d in self.D)

    def make_groups(self, gcap, nwg):
        """Window groups: sum(D) <= gcap, <= nwg windows per group."""
        groups = []
        i = 0
        while i < self.wpc:
            i0, sd, nw = i, 0, 0
            while (i < self.wpc and nw < nwg
                   and (nw == 0 or sd + int(self.D[i]) <= gcap)):
                sd += int(self.D[i])
                i += 1
                nw += 1
            groups.append((i0, nw, int(self.off[i0]), sd))
        return groups

    def stream_h(self, table, core):
        """[128, TOT*C] f16 grid gather: table rows by gidx (0 = zero pad)."""
        C = table.shape[1]
        tp = np.zeros((self.N + 1, C), dtype=np.float16)
        tp[1:] = table
        arr = tp[self.gidx[core]]                    # [TOT, P, C]
        return np.ascontiguousarray(arr.transpose(1, 0, 2)).reshape(
            P, self.TOT * C)

    def stream_als(self, table, core):
        """[128, TOT*H] f16: al_src grid; pad slots -> NEG_INF so exp()==0.
        Dummy rows get one j=0 slot with logit 0 so their softmax denominator
        stays finite (their h rows are zero, so the output row is 0)."""
        H = table.shape[1]
        tp = np.full((self.N + 1, H), NEG_INF, dtype=np.float16)
        tp[1:] = table
        arr = tp[self.gidx[core]]                    # [TOT, P, H]
        i_d, e_d = np.nonzero(self.rows_nodes[core] < 0)
        arr[self.off[i_d], e_d, :] = 0.0
        return np.ascontiguousarray(arr.transpose(1, 0, 2)).reshape(
            P, self.TOT * H)

    def stream_ald(self, table, core):
        """[128, wpc*H] f16: al_dst per (window, row). Dummy rows -> 0."""
        H = table.shape[1]
        tp = np.zeros((self.N + 1, H), dtype=np.float16)
        tp[1:] = table
        arr = tp[self.rows_nodes[core] + 1]          # [wpc, P, H]
        return np.ascontiguousarray(arr.transpose(1, 0, 2)).reshape(
            P, self.wpc * H)

    def stream_ald_exp(self, table, core):
        """[128, TOT*H] f16: al_dst replicated across each window's slots
        (slot grids are per-window blocks of D_i slots)."""
        H = table.shape[1]
        tp = np.zeros((self.N + 1, H), dtype=np.float16)
        tp[1:] = table
        arr = tp[self.rows_nodes[core] + 1]          # [wpc, P, H]
        rep = np.repeat(arr, self.D, axis=0)         # [TOT, P, H]
        return np.ascontiguousarray(rep.transpose(1, 0, 2)).reshape(
            P, self.TOT * H)

    def ident8(self):
        import ml_dtypes
        return np.eye(P, dtype=np.float32).astype(ml_dtypes.float8_e4m3)


# ------------------------------------------------------------------ builders

def _build_node(SH, c_in, m_h, m_al, elu, bias_in, bench_loop=1):
    """Per-node transform: hT = (elu?(xT+b)) @ w, alT = same @ wal.
    When m_h+m_al <= 128 the two matmuls merge into one.  The whole per-core
    panel is SBUF-resident: quarters stream in with fat DMAs, chunked matmuls
    write a staged output panel, and a few fat DMAs store it."""
    merged = (m_h + m_al) <= P
    M = m_h + m_al if merged else m_h
    QN = 4
    QS = SH // QN
    NQUAD = SH // (2 * CH)        # 2 al-chunks stack into one PSUM bank
    assert SH % QN == 0 and QS % CH == 0 and SH % (2 * CH) == 0
    nc = bass.Bass()
    xT = nc.dram_tensor("xT", [c_in, SH], F16, kind="ExternalInput")
    w = nc.dram_tensor("w", [c_in, M], F16, kind="ExternalInput")
    if not merged:
        assert m_al <= 32
        wal = nc.dram_tensor("wal", [c_in, 32], F16, kind="ExternalInput")
    if bias_in:
        bvec = nc.dram_tensor("bvec", [c_in, 1], F32, kind="ExternalInput")
    hT = nc.dram_tensor("hT", [M, SH], F16, kind="ExternalOutput")
    if not merged:
        # partition-stacked al panel: row 32k+r, col cq*CH+x holds
        # al[r] of chunk 2*cq+k (host unscrambles)
        alT = nc.dram_tensor("alT", [64, NQUAD * CH], F16,
                             kind="ExternalOutput")

    with tile.TileContext(nc) as tc:
        with (
            tc.tile_pool(name="const", bufs=1) as constp,
            tc.tile_pool(name="xin", bufs=2) as xinp,
            tc.tile_pool(name="hout", bufs=2) as houtp,
            tc.tile_pool(name="work", bufs=4) as workp,
            tc.tile_pool(name="psH", bufs=5, space="PSUM") as psH,
            tc.tile_pool(name="psA", bufs=3, space="PSUM") as psA,
        ):
            w_sb = constp.tile([c_in, M], F16)
            nc.sync.dma_start(out=w_sb[:], in_=w[:])
            if not merged:
                # wal host-padded to 32 cols (zeros) so every partition of
                # the stacked al PSUM region is written (no uninit reads)
                wal_sb = constp.tile([c_in, 32], F16)
                nc.sync.dma_start(out=wal_sb[:], in_=wal[:])
            if bias_in:
                b_sb = constp.tile([c_in, 1], F32)
                nc.sync.dma_start(out=b_sb[:], in_=bvec[:])

            def body(_iv=None):
                # every DMA rides SP: a queued DMA holds its issuing engine's
                # sequencer for the whole transfer, so ACT/DVE must stay clean
                xq = [xinp.tile([c_in, QS], F16, tag=f"x{q}", name=f"xq{q}")
                      for q in range(QN)]
                for q in range(QN):
                    nc.sync.dma_start(out=xq[q][:],
                                      in_=xT[:, q * QS:(q + 1) * QS])
                hq = [houtp.tile([M, QS], F16, tag=f"h{q}", name=f"hq{q}")
                      for q in range(QN)]
                if not merged:
                    alout = houtp.tile([64, NQUAD * CH], F16, tag="alo")
                quad = {}

                def qfront(q):
                    """Quarter-granular ELU stage A: one fat ACT exp."""
                    if not elu:
                        return None
                    rhs = xq[q][:]
                    if bias_in:
                        nc.vector.tensor_scalar(
                            rhs, rhs, b_sb[:, 0:1], None, OP.add)
                    et = workp.tile([c_in, QS], F16, tag="et")
                    nc.scalar.activation(et[:], rhs, AF.Exp)
                    return et

                def qback(q, et):
                    if elu:
                        # elu(x) = (min(exp(x),1) - 1) + max(x,0), all 2x DVE
                        mn = workp.tile([c_in, QS], F16, tag="mn")
                        nc.vector.tensor_scalar(
                            mn[:], et[:], 1.0, -1.0, OP.min, OP.add)
                        mx = workp.tile([c_in, QS], F16, tag="mx")
                        nc.vector.tensor_scalar(
                            mx[:], xq[q][:], 0.0, None, OP.max)
                        xe = workp.tile([c_in, QS], F16, tag="xe")
                        nc.vector.tensor_tensor(
                            out=xe[:], in0=mn[:], in1=mx[:], op=OP.add)
                        src = xe
                    else:
                        src = xq[q]
                    for j in range(QS // CH):
                        ci = q * (QS // CH) + j
                        qo = j * CH
                        rhs = src[:, qo:qo + CH]
                        ph = psH.tile([M, CH], F32, tag="ph")
                        nc.tensor.matmul(ph[:], w_sb[:], rhs,
                                         start=True, stop=True)
                        dve_copy = (ci % 7 < 3) if elu else (ci % 2 == 1)
                        if dve_copy:
                            nc.vector.tensor_copy(hq[q][:, qo:qo + CH],
                                                  ph[:])
                        else:
                            nc.scalar.activation(hq[q][:, qo:qo + CH],
                                                 ph[:], AF.Copy)
                        if not merged:
                            # stack 2 chunks' al outputs on partitions
                            # 0/32 of one PSUM bank -> 1 copy per pair
                            k = ci % 2
                            if k == 0:
                                quad["pa"] = psA.tile([64, CH], F32,
                                                      tag="paq", name="paq")
                            pa = quad["pa"]
                            nc.tensor.matmul(pa[32 * k:32 * k + 32, :],
                                             wal_sb[:], rhs,
                                             start=True, stop=True)
                            if k == 1:
                                cq = ci // 2
                                if cq % 2 == 0:
                                    nc.vector.tensor_copy(
                                        alout[:, cq * CH:(cq + 1) * CH],
                                        pa[:])
                                else:
                                    nc.scalar.activation(
                                        alout[:, cq * CH:(cq + 1) * CH],
                                        pa[:], AF.Copy)
                    nc.sync.dma_start(out=hT[:, q * QS:(q + 1) * QS],
                                      in_=hq[q][:])

                prev = None
                for q in range(QN):
                    et = qfront(q)
                    if prev is not None:
                        qback(*prev)
                    prev = (q, et)
                qback(*prev)
                if not merged:
                    nc.sync.dma_start(out=alT[:], in_=alout[:])

            if bench_loop > 1:
                with tc.For_i(0, bench_loop, 1) as _iv:
                    body(_iv)
            else:
                body()
    _finalize_kernel(nc)
    return nc


def _build_edge_g(D_list, groups, TOT, Cc, H, bias_out=False, elu_out=False,
                  ald_exp=False, bench_loop=1):
    """Edge aggregation over degree-sorted grids.  Per group of windows:
    one h[src] grid DMA, one DVE logit add per window, one ACT leaky-relu,
    one ACT exp into the message tile's trailing EB columns, one DVE
    multiply, then D accumulating identity matmuls per window.  Epilogues
    run one group late so no engine stalls on PSUM completion."""
    EB = 8
    SLOT = Cc + EB
    G = Cc // EB
    NW = len(D_list)
    GS = max(sd for _, _, _, sd in groups)
    NWmax = max(nw for _, nw, _, _ in groups)

    nc = bass.Bass()
    hsrc = nc.dram_tensor("hsrc", [P, TOT * Cc], F16, kind="ExternalInput")
    als = nc.dram_tensor("als", [P, TOT * H], F16, kind="ExternalInput")
    ald = nc.dram_tensor("ald", [P, (TOT if ald_exp else NW) * H], F16,
                         kind="ExternalInput")
    ident = nc.dram_tensor("ident", [P, P], F8, kind="ExternalInput")
    if bias_out:
        brep = nc.dram_tensor("brep", [P, Cc], F32, kind="ExternalInput")
    # partition-major output: per-partition contiguous runs (the [NW*P, Cc]
    # layout had 128-256 B dram runs, under the 512 B fast-DMA threshold)
    out = nc.dram_tensor("out", [P, NW * Cc], F16, kind="ExternalOutput")

    with tile.TileContext(nc) as tc:
        with (
            tc.tile_pool(name="const", bufs=1) as constp,
            tc.tile_pool(name="aldp", bufs=2) as aldp,
            tc.tile_pool(name="alg", bufs=3) as algp,
            tc.tile_pool(name="hs", bufs=3) as hsp,
            tc.tile_pool(name="za", bufs=3) as zap,
            tc.tile_pool(name="msg", bufs=3) as msgp,
            tc.tile_pool(name="epi", bufs=3) as epip,
            tc.tile_pool(name="og", bufs=2) as ogp,
            tc.tile_pool(name="psW", bufs=8, space="PSUM") as pswp,
        ):
            BSLOT = 512 // SLOT      # windows per PSUM bank
            ident_sb = constp.tile([P, P], F8)
            nc.scalar.dma_start(out=ident_sb[:], in_=ident[:])
            ebias_sb = constp.tile([P, 1], F32)
            nc.vector.memset(ebias_sb[:], EXP_BIAS)
            if bias_out:
                brep_sb = constp.tile([P, Cc], F32)
                nc.scalar.dma_start(out=brep_sb[:], in_=brep[:])

            pend = []

            def front(grp, ald_sb):
                """DMA + logit add + leaky-relu + exp for one group."""
                i0, nw, off0, sd = grp
                hs = hsp.tile([P, GS * Cc], F16, tag="hs")
                nc.sync.dma_start(out=hs[:, :sd * Cc],
                                  in_=hsrc[:, off0 * Cc:(off0 + sd) * Cc])
                alg = algp.tile([P, GS * H], F16, tag="alg")
                nc.sync.dma_start(out=alg[:, :sd * H],
                                  in_=als[:, off0 * H:(off0 + sd) * H])
                za = zap.tile([P, GS * H], F16, tag="za")
                if ald_exp:
                    # host replicated al_dst per slot: one add per group
                    adx = algp.tile([P, GS * H], F16, tag="adx")
                    nc.sync.dma_start(out=adx[:, :sd * H],
                                      in_=ald[:, off0 * H:(off0 + sd) * H])
                    nc.vector.tensor_tensor(out=za[:, :sd * H],
                                            in0=alg[:, :sd * H],
                                            in1=adx[:, :sd * H], op=OP.add)
                doff = 0
                for wl in range(nw) if not ald_exp else ():
                    D = int(D_list[i0 + wl])
                    o0 = doff * H
                    if H > 1:
                        av = alg[:, o0:o0 + D * H].rearrange(
                            "p (d h) -> p d h", d=D)
                        zv = za[:, o0:o0 + D * H].rearrange(
                            "p (d h) -> p d h", d=D)
                        ad = ald_sb[:, (i0 + wl) * H:(i0 + wl + 1) * H]
                        ab = bass.AP(ad.tensor, ad.offset,
                                     [ad.ap[0], [0, D], [1, H]])
                    else:
                        av = alg[:, o0:o0 + D]
                        zv = za[:, o0:o0 + D]
                        ad = ald_sb[:, i0 + wl:i0 + wl + 1]
                        ab = bass.AP(ad.tensor, ad.offset,
                                     [ad.ap[0], [0, D]])
                    nc.vector.tensor_tensor(out=zv, in0=av, in1=ab, op=OP.add)
                    doff += D
                nc.scalar.activation(za[:, :sd * H], za[:, :sd * H],
                                     AF.Prelu, alpha=NEG_SLOPE)
                msg = msgp.tile([P, GS * SLOT], F16, tag="msg")
                m3 = msg[:, :sd * SLOT].rearrange("p (d s) -> p d s", s=SLOT)
                eb_out = m3[:, :, Cc:Cc + EB]
                if H > 1:
                    e_in = za[:, :sd * H].rearrange("p (d h) -> p d h", d=sd)
                else:
                    z0 = za[:, :sd]
                    e_in = bass.AP(z0.tensor, z0.offset,
                                   [z0.ap[0], [1, sd], [0, EB]])
                nc.scalar.activation(eb_out, e_in, AF.Exp, bias=ebias_sb[:])
                return hs, msg

            def back(grp, st):
                """DVE message multiply + PE identity accumulation."""
                i0, nw, off0, sd = grp
                hs, msg = st
                m3 = msg[:, :sd * SLOT].rearrange("p (d s) -> p d s", s=SLOT)
                eb_out = m3[:, :, Cc:Cc + EB]
                mo = m3[:, :, 0:Cc].rearrange("p d (g h) -> p d g h", h=EB)
                hi = hs[:, :sd * Cc].rearrange(
                    "p (d g h) -> p d g h", d=sd, h=EB)
                ei = bass.AP(eb_out.tensor, eb_out.offset,
                             [eb_out.ap[0], eb_out.ap[1], [0, G], [1, EB]])
                nc.vector.tensor_tensor(out=mo, in0=hi, in1=ei, op=OP.mult)
                doff = 0
                bank = None
                for wl in range(nw):
                    D = int(D_list[i0 + wl])
                    if wl % BSLOT == 0:
                        bank = pswp.tile([P, 512], F32, tag="psw",
                                         name="pswbank")
                    sl = (wl % BSLOT) * SLOT
                    psw = bank[:, sl:sl + SLOT]
                    for j in range(D):
                        mv = msg[:, (doff + j) * SLOT:(doff + j + 1) * SLOT]
                        nc.tensor.matmul(psw, ident_sb[:], mv,
                                         start=(j == 0), stop=(j == D - 1))
                    pend.append(psw)
                    doff += D

            def epilogue(grp):
                """One f16 PSUM copy per window, then a single reciprocal +
                scale + output DMA for the whole group."""
                i0, nw, off0, sd = grp
                op_t = epip.tile([P, NWmax * SLOT], F16, tag="o1p")
                for wl in range(nw):
                    psw = pend.pop(0)
                    nc.scalar.activation(op_t[:, wl * SLOT:(wl + 1) * SLOT],
                                         psw, AF.Copy)
                opv = op_t[:, :nw * SLOT]
                rec = epip.tile([P, NWmax * EB], F16, tag="rec")
                rv = rec[:, :nw * EB].rearrange("p (w h) -> p w h", w=nw)
                dap = bass.AP(opv.tensor, opv.offset + Cc,
                              [opv.ap[0], [SLOT, nw], [1, EB]])
                with nc.allow_low_precision(
                        reason="softmax denominators are O(1)"):
                    nc.vector.reciprocal(rv, dap)
                og = ogp.tile([P, NWmax * Cc], F16, tag="og")
                o_in = bass.AP(opv.tensor, opv.offset,
                               [opv.ap[0], [SLOT, nw], [EB, G], [1, EB]])
                r0 = rec[:]
                r_b = bass.AP(r0.tensor, r0.offset,
                              [r0.ap[0], [EB, nw], [0, G], [1, EB]])
                oo = og[:, :nw * Cc].rearrange(
                    "p (w g h) -> p w g h", w=nw, h=EB)
                nc.vector.tensor_tensor(out=oo, in0=o_in, in1=r_b,
                                        op=OP.mult)
                if bias_out:     # layer bias: before the inter-layer elu
                    ov2 = og[:, :nw * Cc].rearrange("p (w c) -> p w c", w=nw)
                    b0 = brep_sb[:]
                    b_b = bass.AP(b0.tensor, b0.offset,
                                  [b0.ap[0], [0, nw], [1, Cc]])
                    nc.vector.tensor_tensor(out=ov2, in0=ov2, in1=b_b,
                                            op=OP.add)
                if elu_out:
                    # elu(x) = max(x,0) + (min(exp(x),1) - 1), in place on og
                    ogv = og[:, :nw * Cc]
                    et = epip.tile([P, NWmax * Cc], F16, tag="et")
                    etv = et[:, :nw * Cc]
                    nc.scalar.activation(etv, ogv, AF.Exp)
                    nc.vector.tensor_scalar(etv, etv, 1.0, -1.0,
                                            OP.min, OP.add)
                    nc.vector.scalar_tensor_tensor(ogv, ogv, 0.0, etv,
                                                   OP.max, OP.add)
                nc.scalar.dma_start(out=out[:, i0 * Cc:(i0 + nw) * Cc],
                                    in_=og[:, :nw * Cc])

            def body(_iv=None):
                if not ald_exp:
                    ald_sb = aldp.tile([P, NW * H], F16, tag="ald")
                    nc.scalar.dma_start(out=ald_sb[:], in_=ald[:])
                else:
                    ald_sb = None
                pend.clear()
                sts = [None] * len(groups)
                for gi, grp in enumerate(groups):
                    sts[gi] = front(grp, ald_sb)
                    if gi >= 1:
                        back(groups[gi - 1], sts[gi - 1])
                        sts[gi - 1] = None
                    if gi >= 2:
                        epilogue(groups[gi - 2])
                ng = len(groups)
                back(groups[ng - 1], sts[ng - 1])
                if ng >= 2:
                    epilogue(groups[ng - 2])
                epilogue(groups[ng - 1])

            if bench_loop > 1:
                with tc.For_i(0, bench_loop, 1) as _iv:
                    body(_iv)
            else:
                body()
    _finalize_kernel(nc)
    return nc


# ------------------------------------------------------------------ runner

def _fold_att(W, a):
    heads, hid = a.shape
    return np.einsum("ihc,hc->ih", W.reshape(W.shape[0], heads, hid), a)


class _GatRunner:
    def __init__(self, n_cores=N_CORES):
        self.C = n_cores
        self._graph = None
        self._graph_key = None
        self._kernels = {}
        self.last_maps = {}

    def graph(self, edge_index, n_nodes):
        key = hash(np.asarray(edge_index).tobytes())
        if key != self._graph_key:
            self._graph = _Graph(edge_index, n_nodes, self.C)
            self._graph_key = key
            self._kernels.clear()
        return self._graph

    def kernel(self, name, bench_loop=1, **kw):
        key = (name, bench_loop, tuple(sorted(kw.items())))
        if key not in self._kernels:
            g = self._graph
            if name.startswith("P"):
                self._kernels[key] = _build_node(
                    g.shard_nodes, bench_loop=bench_loop, **kw)
            elif name == "E1":
                self._kernels[key] = _build_edge_g(
                    g.D, g.groups1, g.TOT, 128, 8,
                    bench_loop=bench_loop, **kw)
            else:
                self._kernels[key] = _build_edge_g(
                    g.D, g.groups2, g.TOT, 64, 1, ald_exp=True,
                    bench_loop=bench_loop, **kw)
        return self._kernels[key]

    def _run(self, name, nc, maps):
        self.last_maps[name] = maps
        res = run_bass_kernel_spmd(nc, maps, core_ids=list(range(self.C)))
        return res.results

    def run(self, x, edge_index, W1, a_src1, a_dst1, b1, W2, a_src2, a_dst2,
            b2):
        C = self.C
        N, IN_C = x.shape
        HEADS, HID = a_src1.shape
        HC = HEADS * HID
        OUT_C = W2.shape[1]
        g = self.graph(edge_index, N)
        SH = g.shard_nodes
        # (c,h)-interleaved channel order for the layer-1 hidden features:
        # col c*H+h of h1 holds math channel h*HID+c. Folded into W1's
        # columns (P0) and W2's rows (P2) on the host - pure permutation.
        perm = np.array([(j % HEADS) * HID + j // HEADS
                         for j in range(HC)], dtype=np.int64)

        # ---- P0: per-node h1 / logits --------------------------------
        xT_pad = np.zeros((IN_C, g.n_pad), dtype=np.float16)
        xT_pad[:, :N] = np.asarray(x, np.float32).T
        w1 = np.asarray(W1, np.float32)
        m_al = 2 * HEADS
        wal1 = np.zeros((IN_C, 32), dtype=np.float32)
        wal1[:, :m_al] = np.concatenate(
            [_fold_att(w1, np.asarray(a_src1, np.float32)),
             _fold_att(w1, np.asarray(a_dst1, np.float32))], axis=1)
        mapsP0 = [{"xT": np.ascontiguousarray(xT_pad[:, k * SH:(k + 1) * SH]),
                   "w": np.ascontiguousarray(w1[:, perm]).astype(np.float16),
                   "wal": wal1.astype(np.float16)} for k in range(C)]
        ncP0 = self.kernel("P0", c_in=IN_C, m_h=HC, m_al=m_al,
                           elu=False, bias_in=False)
        resP0 = self._run("P0", ncP0, mapsP0)
        h1 = np.ascontiguousarray(
            np.concatenate([r["hT"] for r in resP0], axis=1).T)[:N]
        # unscramble the partition-stacked al panel: row 32k+r, col cq*CH+x
        # holds al[r] of chunk 4*cq+k
        nq = SH // (2 * CH)
        al1 = np.concatenate(
            [r["alT"].reshape(2, 32, nq, CH)[:, :m_al]
             .transpose(1, 2, 0, 3).reshape(m_al, SH)
             for r in resP0], axis=1)                    # [16, Np]
        als1 = np.ascontiguousarray(al1[:HEADS, :N].T)
        ald1 = np.ascontiguousarray(al1[HEADS:, :N].T)

        # ---- E1: layer-1 edge aggregation + bias + ELU ---------------
        id8 = g.ident8()
        b1nz = bool(np.any(np.asarray(b1)))
        mapsE1 = []
        for k in range(C):
            m = {"hsrc": g.stream_h(h1, k),
                 "als": g.stream_als(als1, k),
                 "ald": g.stream_ald(ald1, k),
                 "ident": id8}
            if b1nz:
                m["brep"] = np.tile(
                    np.asarray(b1, np.float32)[perm], (P, 1))
            mapsE1.append(m)
        ncE1 = self.kernel("E1", bias_out=b1nz)
        resE1 = self._run("E1", ncE1, mapsE1)
        out1 = np.concatenate(
            [r["out"].reshape(P, g.wpc, HC).transpose(1, 0, 2)
             .reshape(g.wpc * P, HC) for r in resE1], axis=0)
        # rows of out1 are (core, slot, row) -> natural node rowmap
        rowmap = g.rows_nodes.reshape(-1)            # [C*wpc*P]

        # ---- P2: ELU + per-node h2 / logits --------------------------
        o1T = np.ascontiguousarray(out1.T)           # [HC, C*SH] f16
        w2 = np.asarray(W2, np.float32)
        wal2 = np.concatenate(
            [_fold_att(w2, np.asarray(a_src2, np.float32)),
             _fold_att(w2, np.asarray(a_dst2, np.float32))], axis=1)
        w2all = np.concatenate([w2[perm], wal2[perm]], axis=1)  # [HC, 66]
        mapsP2 = [
            {"xT": np.ascontiguousarray(o1T[:, k * SH:(k + 1) * SH]),
             "w": w2all.astype(np.float16)} for k in range(C)]
        # out1 already carries b1 (E1 bias_out); P2 applies the ELU
        ncP2 = self.kernel("P2", c_in=HC, m_h=OUT_C, m_al=2, elu=True,
                           bias_in=False)
        resP2 = self._run("P2", ncP2, mapsP2)
        h2al = np.concatenate([r["hT"] for r in resP2], axis=1)  # [66, Np]
        valid = rowmap >= 0
        vrows = rowmap[valid]
        h2 = np.zeros((N, OUT_C), dtype=np.float16)
        h2[vrows] = h2al[:OUT_C].T[valid]
        als2 = np.zeros((N, 1), dtype=np.float16)
        als2[vrows, 0] = h2al[OUT_C][valid]
        ald2 = np.zeros((N, 1), dtype=np.float16)
        ald2[vrows, 0] = h2al[OUT_C + 1][valid]

        # ---- E2: layer-2 edge aggregation ----------------------------
        b2nz = bool(np.any(np.asarray(b2)))
        mapsE2 = []
        for k in range(C):
            m = {"hsrc": g.stream_h(h2, k),
                 "als": g.stream_als(als2, k),
                 "ald": g.stream_ald_exp(ald2, k),
                 "ident": id8}
            if b2nz:
                m["brep"] = np.tile(np.asarray(b2, np.float32), (P, 1))
            mapsE2.append(m)
        ncE2 = self.kernel("E2", bias_out=b2nz)
        resE2 = self._run("E2", ncE2, mapsE2)
        out2 = np.concatenate(
            [r["out"].reshape(P, g.wpc, OUT_C).transpose(1, 0, 2)
             .reshape(g.wpc * P, OUT_C) for r in resE2], axis=0)
        out_full = np.zeros((N, OUT_C), dtype=np.float32)
        out_full[vrows] = out2[valid]
        return out_full


_RUNNER = _GatRunner()


def kernel(x, edge_index, W1, a_src1, a_dst1, b1, W2, a_src2, a_dst2, b2):
    """Full-input / full-output entry point. Returns [N, OUT_C] float32."""
    args = [np.asarray(v) for v in
            (x, edge_index, W1, a_src1, a_dst1, b1, W2, a_src2, a_dst2, b2)]
    return _RUNNER.run(*args).astype(np.float32)


# revision 49
# speedup vs baseline: 1.0449x; 1.0016x over previous
"""Trainium (trn2) Bass kernel for a 2-layer GAT over N=100k nodes / E=1.7M edges.

Strategy (degree-sorted edge grids + identity-stationary PE accumulation)
-------------------------------------------------------------------------
Nodes are sorted by in-degree on the host and packed into windows of 128
similar-degree destination nodes; windows are dealt round-robin across the 8
NeuronCores.  Each window's edges form a dense grid [128 nodes x D slots]
(D = max in-window degree, padded slots carry -inf logits so exp()==0), so
slot j of all 128 nodes is a 128-edge tile whose destination map is the
IDENTITY: the tensor engine accumulates the per-slot message tiles straight
into the window's PSUM bank with a never-changing fp8 identity stationary.
Degree sorting keeps grid padding at ~1.3%, and the one-hot selection stream
of the classic dst-sorted formulation (128 B/edge of pure index overhead)
disappears entirely.

Each GAT layer runs as TWO SPMD kernels with host-side index gathers (pure
permutations / casts - no host FLOPs) between them:

* node kernel (P0/P2): h = x @ W plus folded attention logits computed once
  per node (dense matmuls).  The full per-core input/output panels live in
  SBUF, loaded/stored with a handful of fat DMAs (per-chunk 1 KB/partition
  DMAs were latency-bound at ~140 GB/s); every DMA rides the SP queue since
  a queued DMA holds its issuing engine's sequencer for the whole transfer.
  P0's 16 logit rows stack two chunks per PSUM bank at partitions 0/32
  (tile_position) so one PSUM->SBUF copy drains two matmuls; P2 computes the
  inter-layer ELU as exp (one fat ACT op per quarter-panel, emitted a
  quarter ahead) + two 2x DVE ops, with PSUM copies balanced across ACT/DVE.
* edge kernel (E1/E2): streams h[src] grids (256/128 B per edge slot) and
  al_src logit grids (16/2 B); al_dst is a tiny per-window constant for E1
  and a host-replicated per-slot stream for E2 (one group-wide DVE add
  instead of 21 window-sized ones).  Windows are processed in groups
  (sum of D <= 96/192) software-pipelined three deep: group g's DMA +
  logits + leaky-relu + exp land while g-1 runs its DVE multiply + PE
  accumulation and g-2 runs its epilogue, so no engine ever stalls on
  another's latency.  ACT writes exp(z-4) into the message tile's trailing
  8 columns ((c,h)-interleaved broadcast for layer 1's 8 heads, an 8x
  replica for layer 2's single head so the DVE multiply keeps its
  packed-innermost 2x mode).  Epilogues drain each window's PSUM with a
  single f16 ACT copy, then one reciprocal + one scale per group.

Measured per-core DMA floor is ~343 GB/s on one queue / ~355 on two (HBM
fair share); the edge kernels stream ~62/~32 MB per core per inference and
run within ~15% of that floor.

Environment workarounds: this container's walrus build allows only ONE
semaphore wait per instruction (split onto nop carriers post-scheduling), and
the GPSIMD ucode libraries are absent (so no dma_gather/indirect-DMA fast
paths - hence the host-gather design).
"""
import numpy as np

import concourse.bass as bass
import concourse.mybir as mybir
import concourse.tile as tile
from concourse.bass_utils import run_bass_kernel_spmd

P = 128
F16 = mybir.dt.float16
F32 = mybir.dt.float32
F8 = mybir.dt.float8e4
AF = mybir.ActivationFunctionType
OP = mybir.AluOpType
NEG_SLOPE = 0.2
EXP_BIAS = -4.0     # exp(z + EXP_BIAS): constant shift cancels in softmax
NEG_INF = -60000.0  # pad-slot logit: exp(lrelu(.)+bias) underflows to 0
N_CORES = 8
EPS = 1e-30
CH = 448            # node-kernel matmul chunk (PSUM: 448*4B <= 2KB bank)
GCAP1, NWG1 = 96, 12     # E1 group capacity (sum of D's / max windows)
GCAP2, NWG2 = 192, 21    # E2 group capacity (smaller tiles -> fatter groups)

# ------------------------------------------------------------------ patches

_wsplit_counter = [0]


def _split_excess_waits(nc, max_waits=1):
    """This walrus build rejects >1 sem-wait per instruction ("Too many sync
    wait commands"). Move overflow waits onto same-engine nop carriers."""
    n_split = 0
    for f in nc.m.functions:
        for blk in f.blocks:
            changed = False
            out = []
            for inst in blk.instructions:
                si = inst.sync_info
                if si is not None and len(si.on_wait) > max_waits:
                    waits = list(si.on_wait)
                    keep = waits[len(waits) - max_waits:]
                    overflow = waits[: len(waits) - max_waits]
                    for i in range(0, len(overflow), max_waits):
                        _wsplit_counter[0] += 1
                        nop = mybir.InstNoOp(
                            name=f"I-wsplit-{_wsplit_counter[0]}", ins=[], outs=[])
                        nop.engine = inst.engine
                        nop.sync_info = mybir.SyncInfo(
                            on_wait=overflow[i: i + max_waits], on_update=[])
                        out.append(nop)
                    inst.sync_info = mybir.SyncInfo(
                        on_wait=keep, on_update=list(si.on_update))
                    changed = True
                    n_split += 1
                out.append(inst)
            if changed:
                blk.instructions = out
    return n_split


def _finalize_kernel(nc):
    import bass_rust as _bass_rust
    from concourse.library_config import all_libraries, standard
    from concourse.library_overlay import lower_extended_insts

    inst_type_to_lib_mask = {}
    for lib in all_libraries:
        for inst_type in lib.instructions:
            inst_type_to_lib_mask[inst_type] = inst_type_to_lib_mask.get(
                inst_type, 0) | (1 << lib.index)
    _bass_rust.insert_library_loads(
        nc, inst_type_to_lib_mask, len(all_libraries), standard.index)
    lower_extended_insts(nc)
    _split_excess_waits(nc)


# ------------------------------------------------------------------ host prep

class _Graph:
    """Degree-sorted grid preprocessing: sort nodes by in-degree, pack 128
    similar-degree nodes per window, deal windows round-robin across cores
    (slot i of every core shares one padded depth D_i so all cores run one
    identical SPMD program), and scatter each node's edges into its grid row.
    """

    def __init__(self, edge_index, n_nodes, n_cores):
        self.N = n_nodes
        self.C = n_cores
        src = np.asarray(edge_index[0], dtype=np.int64)
        dst = np.asarray(edge_index[1], dtype=np.int64)
        E = src.shape[0]

        deg = np.bincount(dst, minlength=n_nodes)
        order = np.argsort(deg, kind="stable")

        n_win_total = (n_nodes + P - 1) // P
        self.wpc = (n_win_total + n_cores - 1) // n_cores
        n_win = self.wpc * n_cores
        self.n_pad = n_win * P
        self.shard_nodes = self.wpc * P
        n_dummy = self.n_pad - n_nodes

        snode = np.full(self.n_pad, -1, dtype=np.int64)
        snode[n_dummy:] = order                      # ascending degree
        # rows_nodes[k][i, e] = natural node id at (core k, slot i, row e)
        self.rows_nodes = np.ascontiguousarray(
            snode.reshape(self.wpc, n_cores, P).transpose(1, 0, 2))

        wdeg = np.where(snode >= 0, deg[np.clip(snode, 0, None)], 0)
        wmax = wdeg.reshape(self.wpc, n_cores, P).max(axis=2)   # [wpc, cores]
        self.D = np.maximum(wmax.max(axis=1), 1).astype(np.int64)  # [wpc]
        self.off = np.concatenate([[0], np.cumsum(self.D)])
        self.TOT = int(self.D.sum())

        # position of each node in the sorted layout
        posq = np.empty(n_nodes, dtype=np.int64)
        posq[order] = np.arange(n_nodes) + n_dummy

        # scatter edges (dst-sorted, ranked within dst run) into grids
        perm = np.argsort(dst, kind="stable")
        src_s = src[perm]
        dst_s = dst[perm]
        bounds = np.searchsorted(dst_s, np.arange(n_nodes + 1))
        j_e = np.arange(E) - bounds[dst_s]           # rank within dst run
        q_e = posq[dst_s]
        g_e = q_e // P
        row_e = q_e % P
        core_e = g_e % n_cores
        slot_e = g_e // n_cores
        flat_e = self.off[slot_e] + j_e              # grid slot within [TOT]
        self.gidx = np.zeros((n_cores, self.TOT, P), dtype=np.int32)
        self.gidx[core_e, flat_e, row_e] = (src_s + 1).astype(np.int32)

        self.groups1 = self.make_groups(GCAP1, NWG1)
        self.groups2 = self.make_groups(GCAP2, NWG2)
        self.D_key = tuple(int(d) for d in self.D)

    def make_groups(self, gcap, nwg):
        """Window groups: sum(D) <= gcap, <= nwg windows per group."""
        groups = []
        i = 0
        while i < self.wpc:
            i0, sd, nw = i, 0, 0
            while (i < self.wpc and nw < nwg
                   and (nw == 0 or sd + int(self.D[i]) <= gcap)):
                sd += int(self.D[i])
                i += 1
                nw += 1
            groups.append((i0, nw, int(self.off[i0]), sd))
        return groups

    def stream_h(self, table, core):
        """[128, TOT*C] f16 grid gather: table rows by gidx (0 = zero pad)."""
        C = table.shape[1]
        tp = np.zeros((self.N + 1, C), dtype=np.float16)
        tp[1:] = table
        arr = tp[self.gidx[core]]                    # [TOT, P, C]
        return np.ascontiguousarray(arr.transpose(1, 0, 2)).reshape(
            P, self.TOT * C)

    def stream_als(self, table, core):
        """[128, TOT*H] f16: al_src grid; pad slots -> NEG_INF so exp()==0.
        Dummy rows get one j=0 slot with logit 0 so their softmax denominator
        stays finite (their h rows are zero, so the output row is 0)."""
        H = table.shape[1]
        tp = np.full((self.N + 1, H), NEG_INF, dtype=np.float16)
        tp[1:] = table
        arr = tp[self.gidx[core]]                    # [TOT, P, H]
        i_d, e_d = np.nonzero(self.rows_nodes[core] < 0)
        arr[self.off[i_d], e_d, :] = 0.0
        return np.ascontiguousarray(arr.transpose(1, 0, 2)).reshape(
            P, self.TOT * H)

    def stream_ald(self, table, core):
        """[128, wpc*H] f16: al_dst per (window, row). Dummy rows -> 0."""
        H = table.shape[1]
        tp = np.zeros((self.N + 1, H), dtype=np.float16)
        tp[1:] = table
        arr = tp[self.rows_nodes[core] + 1]          # [wpc, P, H]
        return np.ascontiguousarray(arr.transpose(1, 0, 2)).reshape(
            P, self.wpc * H)

    def stream_ald_exp(self, table, core):
        """[128, TOT*H] f16: al_dst replicated across each window's slots
        (slot grids are per-window blocks of D_i slots)."""
        H = table.shape[1]
        tp = np.zeros((self.N + 1, H), dtype=np.float16)
        tp[1:] = table
        arr = tp[self.rows_nodes[core] + 1]          # [wpc, P, H]
        rep = np.repeat(arr, self.D, axis=0)         # [TOT, P, H]
        return np.ascontiguousarray(rep.transpose(1, 0, 2)).reshape(
            P, self.TOT * H)

    def ident8(self):
        import ml_dtypes
        return np.eye(P, dtype=np.float32).astype(ml_dtypes.float8_e4m3)


# ------------------------------------------------------------------ builders

def _build_node(SH, c_in, m_h, m_al, elu, bias_in, bench_loop=1):
    """Per-node transform: hT = (elu?(xT+b)) @ w, alT = same @ wal.
    When m_h+m_al <= 128 the two matmuls merge into one.  The whole per-core
    panel is SBUF-resident: quarters stream in with fat DMAs, chunked matmuls
    write a staged output panel, and a few fat DMAs store it."""
    merged = (m_h + m_al) <= P
    M = m_h + m_al if merged else m_h
    QN = 4
    QS = SH // QN
    NQUAD = SH // (2 * CH)        # 2 al-chunks stack into one PSUM bank
    assert SH % QN == 0 and QS % CH == 0 and SH % (2 * CH) == 0
    nc = bass.Bass()
    xT = nc.dram_tensor("xT", [c_in, SH], F16, kind="ExternalInput")
    w = nc.dram_tensor("w", [c_in, M], F16, kind="ExternalInput")
    if not merged:
        assert m_al <= 32
        wal = nc.dram_tensor("wal", [c_in, 32], F16, kind="ExternalInput")
    if bias_in:
        bvec = nc.dram_tensor("bvec", [c_in, 1], F32, kind="ExternalInput")
    hT = nc.dram_tensor("hT", [M, SH], F16, kind="ExternalOutput")
    if not merged:
        # partition-stacked al panel: row 32k+r, col cq*CH+x holds
        # al[r] of chunk 2*cq+k (host unscrambles)
        alT = nc.dram_tensor("alT", [64, NQUAD * CH], F16,
                             kind="ExternalOutput")

    with tile.TileContext(nc) as tc:
        with (
            tc.tile_pool(name="const", bufs=1) as constp,
            tc.tile_pool(name="xin", bufs=2) as xinp,
            tc.tile_pool(name="hout", bufs=2) as houtp,
            tc.tile_pool(name="work", bufs=4) as workp,
            tc.tile_pool(name="psH", bufs=5, space="PSUM") as psH,
            tc.tile_pool(name="psA", bufs=3, space="PSUM") as psA,
        ):
            w_sb = constp.tile([c_in, M], F16)
            nc.sync.dma_start(out=w_sb[:], in_=w[:])
            if not merged:
                # wal host-padded to 32 cols (zeros) so every partition of
                # the stacked al PSUM region is written (no uninit reads)
                wal_sb = constp.tile([c_in, 32], F16)
                nc.sync.dma_start(out=wal_sb[:], in_=wal[:])
            if bias_in:
                b_sb = constp.tile([c_in, 1], F32)
                nc.sync.dma_start(out=b_sb[:], in_=bvec[:])

            def body(_iv=None):
                # every DMA rides SP: a queued DMA holds its issuing engine's
                # sequencer for the whole transfer, so ACT/DVE must stay clean
                xq = [xinp.tile([c_in, QS], F16, tag=f"x{q}", name=f"xq{q}")
                      for q in range(QN)]
                for q in range(QN):
                    nc.sync.dma_start(out=xq[q][:],
                                      in_=xT[:, q * QS:(q + 1) * QS])
                hq = [houtp.tile([M, QS], F16, tag=f"h{q}", name=f"hq{q}")
                      for q in range(QN)]
                if not merged:
                    alout = houtp.tile([64, NQUAD * CH], F16, tag="alo")
                quad = {}

                def qfront(q):
                    """Quarter-granular ELU stage A: one fat ACT exp."""
                    if not elu:
                        return None
                    rhs = xq[q][:]
                    if bias_in:
                        nc.vector.tensor_scalar(
                            rhs, rhs, b_sb[:, 0:1], None, OP.add)
                    et = workp.tile([c_in, QS], F16, tag="et")
                    nc.scalar.activation(et[:], rhs, AF.Exp)
                    return et

                def qback(q, et):
                    if elu:
                        # elu(x) = (min(exp(x),1) - 1) + max(x,0), all 2x DVE
                        mn = workp.tile([c_in, QS], F16, tag="mn")
                        nc.vector.tensor_scalar(
                            mn[:], et[:], 1.0, -1.0, OP.min, OP.add)
                        mx = workp.tile([c_in, QS], F16, tag="mx")
                        nc.vector.tensor_scalar(
                            mx[:], xq[q][:], 0.0, None, OP.max)
                        xe = workp.tile([c_in, QS], F16, tag="xe")
                        nc.vector.tensor_tensor(
                            out=xe[:], in0=mn[:], in1=mx[:], op=OP.add)
                        src = xe
                    else:
                        src = xq[q]
                    for j in range(QS // CH):
                        ci = q * (QS // CH) + j
                        qo = j * CH
                        rhs = src[:, qo:qo + CH]
                        ph = psH.tile([M, CH], F32, tag="ph")
                        nc.tensor.matmul(ph[:], w_sb[:], rhs,
                                         start=True, stop=True)
                        dve_copy = (ci % 7 < 3) if elu else (ci % 2 == 1)
                        if dve_copy:
                            nc.vector.tensor_copy(hq[q][:, qo:qo + CH],
                                                  ph[:])
                        else:
                            nc.scalar.activation(hq[q][:, qo:qo + CH],
                                                 ph[:], AF.Copy)
                        if not merged:
                            # stack 2 chunks' al outputs on partitions
                            # 0/32 of one PSUM bank -> 1 copy per pair
                            k = ci % 2
                            if k == 0:
                                quad["pa"] = psA.tile([64, CH], F32,
                                                      tag="paq", name="paq")
                            pa = quad["pa"]
                            nc.tensor.matmul(pa[32 * k:32 * k + 32, :],
                                             wal_sb[:], rhs,
                                             start=True, stop=True)
                            if k == 1:
                                cq = ci // 2
                                if cq % 2 == 0:
                                    nc.vector.tensor_copy(
                                        alout[:, cq * CH:(cq + 1) * CH],
                                        pa[:])
                                else:
                                    nc.scalar.activation(
                                        alout[:, cq * CH:(cq + 1) * CH],
                                        pa[:], AF.Copy)
                    nc.sync.dma_start(out=hT[:, q * QS:(q + 1) * QS],
                                      in_=hq[q][:])

                prev = None
                for q in range(QN):
                    et = qfront(q)
                    if prev is not None:
                        qback(*prev)
                    prev = (q, et)
                qback(*prev)
                if not merged:
                    nc.sync.dma_start(out=alT[:], in_=alout[:])

            if bench_loop > 1:
                with tc.For_i(0, bench_loop, 1) as _iv:
                    body(_iv)
            else:
                body()
    _finalize_kernel(nc)
    return nc


def _build_edge_g(D_list, groups, TOT, Cc, H, bias_out=False, elu_out=False,
                  ald_exp=False, bench_loop=1):
    """Edge aggregation over degree-sorted grids.  Per group of windows:
    one h[src] grid DMA, one DVE logit add per window, one ACT leaky-relu,
    one ACT exp into the message tile's trailing EB columns, one DVE
    multiply, then D accumulating identity matmuls per window.  Epilogues
    run one group late so no engine stalls on PSUM completion."""
    EB = 8
    SLOT = Cc + EB
    G = Cc // EB
    NW = len(D_list)
    GS = max(sd for _, _, _, sd in groups)
    NWmax = max(nw for _, nw, _, _ in groups)

    nc = bass.Bass()
    hsrc = nc.dram_tensor("hsrc", [P, TOT * Cc], F16, kind="ExternalInput")
    # ald_exp: als carries [al_src | al_dst] interleaved per slot (doubles
    # the per-partition dram run length past the 512 B fast-DMA threshold)
    als = nc.dram_tensor("als", [P, TOT * H * (2 if ald_exp else 1)], F16,
                         kind="ExternalInput")
    if not ald_exp:
        ald = nc.dram_tensor("ald", [P, NW * H], F16, kind="ExternalInput")
    ident = nc.dram_tensor("ident", [P, P], F8, kind="ExternalInput")
    if bias_out:
        brep = nc.dram_tensor("brep", [P, Cc], F32, kind="ExternalInput")
    # partition-major output: per-partition contiguous runs (the [NW*P, Cc]
    # layout had 128-256 B dram runs, under the 512 B fast-DMA threshold)
    out = nc.dram_tensor("out", [P, NW * Cc], F16, kind="ExternalOutput")

    with tile.TileContext(nc) as tc:
        with (
            tc.tile_pool(name="const", bufs=1) as constp,
            tc.tile_pool(name="aldp", bufs=2) as aldp,
            tc.tile_pool(name="alg", bufs=3) as algp,
            tc.tile_pool(name="hs", bufs=3) as hsp,
            tc.tile_pool(name="za", bufs=3) as zap,
            tc.tile_pool(name="msg", bufs=3) as msgp,
            tc.tile_pool(name="epi", bufs=3) as epip,
            tc.tile_pool(name="og", bufs=2) as ogp,
            tc.tile_pool(name="psW", bufs=8, space="PSUM") as pswp,
        ):
            BSLOT = 512 // SLOT      # windows per PSUM bank
            ident_sb = constp.tile([P, P], F8)
            nc.scalar.dma_start(out=ident_sb[:], in_=ident[:])
            ebias_sb = constp.tile([P, 1], F32)
            nc.vector.memset(ebias_sb[:], EXP_BIAS)
            if bias_out:
                brep_sb = constp.tile([P, Cc], F32)
                nc.scalar.dma_start(out=brep_sb[:], in_=brep[:])

            pend = []

            def front(grp, ald_sb):
                """DMA + logit add + leaky-relu + exp for one group."""
                i0, nw, off0, sd = grp
                hs = hsp.tile([P, GS * Cc], F16, tag="hs")
                nc.sync.dma_start(out=hs[:, :sd * Cc],
                                  in_=hsrc[:, off0 * Cc:(off0 + sd) * Cc])
                AW = H * (2 if ald_exp else 1)
                alg = algp.tile([P, GS * AW], F16, tag="alg")
                nc.sync.dma_start(out=alg[:, :sd * AW],
                                  in_=als[:, off0 * AW:(off0 + sd) * AW])
                za = zap.tile([P, GS * H], F16, tag="za")
                if ald_exp:
                    # interleaved [al_src | al_dst] slots: one add per group
                    a0 = alg[:]
                    av = bass.AP(a0.tensor, a0.offset, [a0.ap[0], [2, sd]])
                    bv = bass.AP(a0.tensor, a0.offset + 1,
                                 [a0.ap[0], [2, sd]])
                    nc.vector.tensor_tensor(out=za[:, :sd],
                                            in0=av, in1=bv, op=OP.add)
                doff = 0
                for wl in range(nw) if not ald_exp else ():
                    D = int(D_list[i0 + wl])
                    o0 = doff * H
                    if H > 1:
                        av = alg[:, o0:o0 + D * H].rearrange(
                            "p (d h) -> p d h", d=D)
                        zv = za[:, o0:o0 + D * H].rearrange(
                            "p (d h) -> p d h", d=D)
                        ad = ald_sb[:, (i0 + wl) * H:(i0 + wl + 1) * H]
                        ab = bass.AP(ad.tensor, ad.offset,
                                     [ad.ap[0], [0, D], [1, H]])
                    else:
                        av = alg[:, o0:o0 + D]
                        zv = za[:, o0:o0 + D]
                        ad = ald_sb[:, i0 + wl:i0 + wl + 1]
                        ab = bass.AP(ad.tensor, ad.offset,
                                     [ad.ap[0], [0, D]])
                    nc.vector.tensor_tensor(out=zv, in0=av, in1=ab, op=OP.add)
                    doff += D
                nc.scalar.activation(za[:, :sd * H], za[:, :sd * H],
                                     AF.Prelu, alpha=NEG_SLOPE)
                msg = msgp.tile([P, GS * SLOT], F16, tag="msg")
                m3 = msg[:, :sd * SLOT].rearrange("p (d s) -> p d s", s=SLOT)
                eb_out = m3[:, :, Cc:Cc + EB]
                if H > 1:
                    e_in = za[:, :sd * H].rearrange("p (d h) -> p d h", d=sd)
                else:
                    z0 = za[:, :sd]
                    e_in = bass.AP(z0.tensor, z0.offset,
                                   [z0.ap[0], [1, sd], [0, EB]])
                nc.scalar.activation(eb_out, e_in, AF.Exp, bias=ebias_sb[:])
                return hs, msg

            def back(grp, st):
                """DVE message multiply + PE identity accumulation."""
                i0, nw, off0, sd = grp
                hs, msg = st
                m3 = msg[:, :sd * SLOT].rearrange("p (d s) -> p d s", s=SLOT)
                eb_out = m3[:, :, Cc:Cc + EB]
                mo = m3[:, :, 0:Cc].rearrange("p d (g h) -> p d g h", h=EB)
                hi = hs[:, :sd * Cc].rearrange(
                    "p (d g h) -> p d g h", d=sd, h=EB)
                ei = bass.AP(eb_out.tensor, eb_out.offset,
                             [eb_out.ap[0], eb_out.ap[1], [0, G], [1, EB]])
                nc.vector.tensor_tensor(out=mo, in0=hi, in1=ei, op=OP.mult)
                doff = 0
                bank = None
                for wl in range(nw):
                    D = int(D_list[i0 + wl])
                    if wl % BSLOT == 0:
                        bank = pswp.tile([P, 512], F32, tag="psw",
                                         name="pswbank")
                    sl = (wl % BSLOT) * SLOT
                    psw = bank[:, sl:sl + SLOT]
                    for j in range(D):
                        mv = msg[:, (doff + j) * SLOT:(doff + j + 1) * SLOT]
                        nc.tensor.matmul(psw, ident_sb[:], mv,
                                         start=(j == 0), stop=(j == D - 1))
                    pend.append(psw)
                    doff += D

            def epilogue(grp):
                """One f16 PSUM copy per window, then a single reciprocal +
                scale + output DMA for the whole group."""
                i0, nw, off0, sd = grp
                op_t = epip.tile([P, NWmax * SLOT], F16, tag="o1p")
                for wl in range(nw):
                    psw = pend.pop(0)
                    nc.scalar.activation(op_t[:, wl * SLOT:(wl + 1) * SLOT],
                                         psw, AF.Copy)
                opv = op_t[:, :nw * SLOT]
                rec = epip.tile([P, NWmax * EB], F16, tag="rec")
                rv = rec[:, :nw * EB].rearrange("p (w h) -> p w h", w=nw)
                dap = bass.AP(opv.tensor, opv.offset + Cc,
                              [opv.ap[0], [SLOT, nw], [1, EB]])
                with nc.allow_low_precision(
                        reason="softmax denominators are O(1)"):
                    nc.vector.reciprocal(rv, dap)
                og = ogp.tile([P, NWmax * Cc], F16, tag="og")
                o_in = bass.AP(opv.tensor, opv.offset,
                               [opv.ap[0], [SLOT, nw], [EB, G], [1, EB]])
                r0 = rec[:]
                r_b = bass.AP(r0.tensor, r0.offset,
                              [r0.ap[0], [EB, nw], [0, G], [1, EB]])
                oo = og[:, :nw * Cc].rearrange(
                    "p (w g h) -> p w g h", w=nw, h=EB)
                nc.vector.tensor_tensor(out=oo, in0=o_in, in1=r_b,
                                        op=OP.mult)
                if bias_out:     # layer bias: before the inter-layer elu
                    ov2 = og[:, :nw * Cc].rearrange("p (w c) -> p w c", w=nw)
                    b0 = brep_sb[:]
                    b_b = bass.AP(b0.tensor, b0.offset,
                                  [b0.ap[0], [0, nw], [1, Cc]])
                    nc.vector.tensor_tensor(out=ov2, in0=ov2, in1=b_b,
                                            op=OP.add)
                if elu_out:
                    # elu(x) = max(x,0) + (min(exp(x),1) - 1), in place on og
                    ogv = og[:, :nw * Cc]
                    et = epip.tile([P, NWmax * Cc], F16, tag="et")
                    etv = et[:, :nw * Cc]
                    nc.scalar.activation(etv, ogv, AF.Exp)
                    nc.vector.tensor_scalar(etv, etv, 1.0, -1.0,
                                            OP.min, OP.add)
                    nc.vector.scalar_tensor_tensor(ogv, ogv, 0.0, etv,
                                                   OP.max, OP.add)
                nc.scalar.dma_start(out=out[:, i0 * Cc:(i0 + nw) * Cc],
                                    in_=og[:, :nw * Cc])

            def body(_iv=None):
                if not ald_exp:
                    ald_sb = aldp.tile([P, NW * H], F16, tag="ald")
                    nc.scalar.dma_start(out=ald_sb[:], in_=ald[:])
                else:
                    ald_sb = None
                pend.clear()
                sts = [None] * len(groups)
                for gi, grp in enumerate(groups):
                    sts[gi] = front(grp, ald_sb)
                    if gi >= 1:
                        back(groups[gi - 1], sts[gi - 1])
                        sts[gi - 1] = None
                    if gi >= 2:
                        epilogue(groups[gi - 2])
                ng = len(groups)
                back(groups[ng - 1], sts[ng - 1])
                if ng >= 2:
                    epilogue(groups[ng - 2])
                epilogue(groups[ng - 1])

            if bench_loop > 1:
                with tc.For_i(0, bench_loop, 1) as _iv:
                    body(_iv)
            else:
                body()
    _finalize_kernel(nc)
    return nc


# ------------------------------------------------------------------ runner

def _fold_att(W, a):
    heads, hid = a.shape
    return np.einsum("ihc,hc->ih", W.reshape(W.shape[0], heads, hid), a)


class _GatRunner:
    def __init__(self, n_cores=N_CORES):
        self.C = n_cores
        self._graph = None
        self._graph_key = None
        self._kernels = {}
        self.last_maps = {}

    def graph(self, edge_index, n_nodes):
        key = hash(np.asarray(edge_index).tobytes())
        if key != self._graph_key:
            self._graph = _Graph(edge_index, n_nodes, self.C)
            self._graph_key = key
            self._kernels.clear()
        return self._graph

    def kernel(self, name, bench_loop=1, **kw):
        key = (name, bench_loop, tuple(sorted(kw.items())))
        if key not in self._kernels:
            g = self._graph
            if name.startswith("P"):
                self._kernels[key] = _build_node(
                    g.shard_nodes, bench_loop=bench_loop, **kw)
            elif name == "E1":
                self._kernels[key] = _build_edge_g(
                    g.D, g.groups1, g.TOT, 128, 8,
                    bench_loop=bench_loop, **kw)
            else:
                self._kernels[key] = _build_edge_g(
                    g.D, g.groups2, g.TOT, 64, 1, ald_exp=True,
                    bench_loop=bench_loop, **kw)
        return self._kernels[key]

    def _run(self, name, nc, maps):
        self.last_maps[name] = maps
        res = run_bass_kernel_spmd(nc, maps, core_ids=list(range(self.C)))
        return res.results

    def run(self, x, edge_index, W1, a_src1, a_dst1, b1, W2, a_src2, a_dst2,
            b2):
        C = self.C
        N, IN_C = x.shape
        HEADS, HID = a_src1.shape
        HC = HEADS * HID
        OUT_C = W2.shape[1]
        g = self.graph(edge_index, N)
        SH = g.shard_nodes
        # (c,h)-interleaved channel order for the layer-1 hidden features:
        # col c*H+h of h1 holds math channel h*HID+c. Folded into W1's
        # columns (P0) and W2's rows (P2) on the host - pure permutation.
        perm = np.array([(j % HEADS) * HID + j // HEADS
                         for j in range(HC)], dtype=np.int64)

        # ---- P0: per-node h1 / logits --------------------------------
        xT_pad = np.zeros((IN_C, g.n_pad), dtype=np.float16)
        xT_pad[:, :N] = np.asarray(x, np.float32).T
        w1 = np.asarray(W1, np.float32)
        m_al = 2 * HEADS
        wal1 = np.zeros((IN_C, 32), dtype=np.float32)
        wal1[:, :m_al] = np.concatenate(
            [_fold_att(w1, np.asarray(a_src1, np.float32)),
             _fold_att(w1, np.asarray(a_dst1, np.float32))], axis=1)
        mapsP0 = [{"xT": np.ascontiguousarray(xT_pad[:, k * SH:(k + 1) * SH]),
                   "w": np.ascontiguousarray(w1[:, perm]).astype(np.float16),
                   "wal": wal1.astype(np.float16)} for k in range(C)]
        ncP0 = self.kernel("P0", c_in=IN_C, m_h=HC, m_al=m_al,
                           elu=False, bias_in=False)
        resP0 = self._run("P0", ncP0, mapsP0)
        h1 = np.ascontiguousarray(
            np.concatenate([r["hT"] for r in resP0], axis=1).T)[:N]
        # unscramble the partition-stacked al panel: row 32k+r, col cq*CH+x
        # holds al[r] of chunk 4*cq+k
        nq = SH // (2 * CH)
        al1 = np.concatenate(
            [r["alT"].reshape(2, 32, nq, CH)[:, :m_al]
             .transpose(1, 2, 0, 3).reshape(m_al, SH)
             for r in resP0], axis=1)                    # [16, Np]
        als1 = np.ascontiguousarray(al1[:HEADS, :N].T)
        ald1 = np.ascontiguousarray(al1[HEADS:, :N].T)

        # ---- E1: layer-1 edge aggregation + bias + ELU ---------------
        id8 = g.ident8()
        b1nz = bool(np.any(np.asarray(b1)))
        mapsE1 = []
        for k in range(C):
            m = {"hsrc": g.stream_h(h1, k),
                 "als": g.stream_als(als1, k),
                 "ald": g.stream_ald(ald1, k),
                 "ident": id8}
            if b1nz:
                m["brep"] = np.tile(
                    np.asarray(b1, np.float32)[perm], (P, 1))
            mapsE1.append(m)
        ncE1 = self.kernel("E1", bias_out=b1nz)
        resE1 = self._run("E1", ncE1, mapsE1)
        out1 = np.concatenate(
            [r["out"].reshape(P, g.wpc, HC).transpose(1, 0, 2)
             .reshape(g.wpc * P, HC) for r in resE1], axis=0)
        # rows of out1 are (core, slot, row) -> natural node rowmap
        rowmap = g.rows_nodes.reshape(-1)            # [C*wpc*P]

        # ---- P2: ELU + per-node h2 / logits --------------------------
        o1T = np.ascontiguousarray(out1.T)           # [HC, C*SH] f16
        w2 = np.asarray(W2, np.float32)
        wal2 = np.concatenate(
            [_fold_att(w2, np.asarray(a_src2, np.float32)),
             _fold_att(w2, np.asarray(a_dst2, np.float32))], axis=1)
        w2all = np.concatenate([w2[perm], wal2[perm]], axis=1)  # [HC, 66]
        mapsP2 = [
            {"xT": np.ascontiguousarray(o1T[:, k * SH:(k + 1) * SH]),
             "w": w2all.astype(np.float16)} for k in range(C)]
        # out1 already carries b1 (E1 bias_out); P2 applies the ELU
        ncP2 = self.kernel("P2", c_in=HC, m_h=OUT_C, m_al=2, elu=True,
                           bias_in=False)
        resP2 = self._run("P2", ncP2, mapsP2)
        h2al = np.concatenate([r["hT"] for r in resP2], axis=1)  # [66, Np]
        valid = rowmap >= 0
        vrows = rowmap[valid]
        h2 = np.zeros((N, OUT_C), dtype=np.float16)
        h2[vrows] = h2al[:OUT_C].T[valid]
        als2 = np.zeros((N, 1), dtype=np.float16)
        als2[vrows, 0] = h2al[OUT_C][valid]
        ald2 = np.zeros((N, 1), dtype=np.float16)
        ald2[vrows, 0] = h2al[OUT_C + 1][valid]

        # ---- E2: layer-2 edge aggregation ----------------------------
        b2nz = bool(np.any(np.asarray(b2)))
        mapsE2 = []
        for k in range(C):
            a_s = g.stream_als(als2, k).reshape(P, g.TOT)
            a_d = g.stream_ald_exp(ald2, k).reshape(P, g.TOT)
            m = {"hsrc": g.stream_h(h2, k),
                 "als": np.ascontiguousarray(
                     np.stack([a_s, a_d], axis=2)).reshape(P, g.TOT * 2),
                 "ident": id8}
            if b2nz:
                m["brep"] = np.tile(np.asarray(b2, np.float32), (P, 1))
            mapsE2.append(m)
        ncE2 = self.kernel("E2", bias_out=b2nz)
        resE2 = self._run("E2", ncE2, mapsE2)
        out2 = np.concatenate(
            [r["out"].reshape(P, g.wpc, OUT_C).transpose(1, 0, 2)
             .reshape(g.wpc * P, OUT_C) for r in resE2], axis=0)
        out_full = np.zeros((N, OUT_C), dtype=np.float32)
        out_full[vrows] = out2[valid]
        return out_full


_RUNNER = _GatRunner()


def kernel(x, edge_index, W1, a_src1, a_dst1, b1, W2, a_src2, a_dst2, b2):
    """Full-input / full-output entry point. Returns [N, OUT_C] float32."""
    args = [np.asarray(v) for v in
            (x, edge_index, W1, a_src1, a_dst1, b1, W2, a_src2, a_dst2, b2)]
    return _RUNNER.run(*args).astype(np.float32)


# revision 54
# speedup vs baseline: 1.0694x; 1.0234x over previous
"""Trainium (trn2) Bass kernel for a 2-layer GAT over N=100k nodes / E=1.7M edges.

Strategy (degree-sorted edge grids + identity-stationary PE accumulation)
-------------------------------------------------------------------------
Nodes are sorted by in-degree on the host and packed into windows of 128
similar-degree destination nodes; windows are dealt round-robin across the 8
NeuronCores.  Each window's edges form a dense grid [128 nodes x D slots]
(D = max in-window degree, padded slots carry -inf logits so exp()==0), so
slot j of all 128 nodes is a 128-edge tile whose destination map is the
IDENTITY: the tensor engine accumulates the per-slot message tiles straight
into the window's PSUM bank with a never-changing fp8 identity stationary.
Degree sorting keeps grid padding at ~1.3%, and the one-hot selection stream
of the classic dst-sorted formulation (128 B/edge of pure index overhead)
disappears entirely.

Each GAT layer runs as TWO SPMD kernels with host-side index gathers (pure
permutations / casts - no host FLOPs) between them:

* node kernel (P0/P2): h = x @ W plus folded attention logits computed once
  per node (dense matmuls).  The full per-core input/output panels live in
  SBUF, loaded/stored with a handful of fat DMAs (per-chunk 1 KB/partition
  DMAs were latency-bound at ~140 GB/s); every DMA rides the SP queue since
  a queued DMA holds its issuing engine's sequencer for the whole transfer.
  P0's 16 logit rows stack two chunks per PSUM bank at partitions 0/32
  (tile_position) so one PSUM->SBUF copy drains two matmuls; P2 computes the
  inter-layer ELU as exp (one fat ACT op per quarter-panel, emitted a
  quarter ahead) + two 2x DVE ops, with PSUM copies balanced across ACT/DVE.
* edge kernel (E1/E2): streams h[src] grids (256/128 B per edge slot) and
  al_src logit grids (16/2 B); al_dst is a tiny per-window constant for E1
  and a host-replicated per-slot stream for E2 (one group-wide DVE add
  instead of 21 window-sized ones).  Windows are processed in groups
  (sum of D <= 96/192) software-pipelined three deep: group g's DMA +
  logits + leaky-relu + exp land while g-1 runs its DVE multiply + PE
  accumulation and g-2 runs its epilogue, so no engine ever stalls on
  another's latency.  ACT writes exp(z-4) into the message tile's trailing
  8 columns ((c,h)-interleaved broadcast for layer 1's 8 heads, an 8x
  replica for layer 2's single head so the DVE multiply keeps its
  packed-innermost 2x mode).  Epilogues drain each window's PSUM with a
  single f16 ACT copy, then one reciprocal + one scale per group, into a
  partition-major [128, NW*C] output panel (the row-major layout's 128-256 B
  dram runs fell under the 512 B threshold where DMA cost doubles; the host
  unscrambles for free).

Measured per-core DMA floor is ~343 GB/s on one queue / ~355 on two (HBM
fair share); the edge kernels stream ~62/~32 MB per core per inference and
run within ~15% of that floor.

Environment workarounds: this container's walrus build allows only ONE
semaphore wait per instruction (split onto nop carriers post-scheduling), and
the GPSIMD ucode libraries are absent (so no dma_gather/indirect-DMA fast
paths - hence the host-gather design).
"""
import numpy as np

import concourse.bass as bass
import concourse.mybir as mybir
import concourse.tile as tile
from concourse.bass_utils import run_bass_kernel_spmd

P = 128
F16 = mybir.dt.float16
F32 = mybir.dt.float32
F8 = mybir.dt.float8e4
AF = mybir.ActivationFunctionType
OP = mybir.AluOpType
NEG_SLOPE = 0.2
EXP_BIAS = -4.0     # exp(z + EXP_BIAS): constant shift cancels in softmax
NEG_INF = -60000.0  # pad-slot logit: exp(lrelu(.)+bias) underflows to 0
N_CORES = 8
EPS = 1e-30
CH = 448            # node-kernel matmul chunk (PSUM: 448*4B <= 2KB bank)
GCAP1, NWG1 = 96, 12     # E1 group capacity (sum of D's / max windows)
GCAP2, NWG2 = 192, 21    # E2 group capacity (smaller tiles -> fatter groups)

# ------------------------------------------------------------------ patches

_wsplit_counter = [0]


def _split_excess_waits(nc, max_waits=1):
    """This walrus build rejects >1 sem-wait per instruction ("Too many sync
    wait commands"). Move overflow waits onto same-engine nop carriers."""
    n_split = 0
    for f in nc.m.functions:
        for blk in f.blocks:
            changed = False
            out = []
            for inst in blk.instructions:
                si = inst.sync_info
                if si is not None and len(si.on_wait) > max_waits:
                    waits = list(si.on_wait)
                    keep = waits[len(waits) - max_waits:]
                    overflow = waits[: len(waits) - max_waits]
                    for i in range(0, len(overflow), max_waits):
                        _wsplit_counter[0] += 1
                        nop = mybir.InstNoOp(
                            name=f"I-wsplit-{_wsplit_counter[0]}", ins=[], outs=[])
                        nop.engine = inst.engine
                        nop.sync_info = mybir.SyncInfo(
                            on_wait=overflow[i: i + max_waits], on_update=[])
                        out.append(nop)
                    inst.sync_info = mybir.SyncInfo(
                        on_wait=keep, on_update=list(si.on_update))
                    changed = True
                    n_split += 1
                out.append(inst)
            if changed:
                blk.instructions = out
    return n_split


def _finalize_kernel(nc):
    import bass_rust as _bass_rust
    from concourse.library_config import all_libraries, standard
    from concourse.library_overlay import lower_extended_insts

    inst_type_to_lib_mask = {}
    for lib in all_libraries:
        for inst_type in lib.instructions:
            inst_type_to_lib_mask[inst_type] = inst_type_to_lib_mask.get(
                inst_type, 0) | (1 << lib.index)
    _bass_rust.insert_library_loads(
        nc, inst_type_to_lib_mask, len(all_libraries), standard.index)
    lower_extended_insts(nc)
    _split_excess_waits(nc)


# ------------------------------------------------------------------ host prep

class _Graph:
    """Degree-sorted grid preprocessing: sort nodes by in-degree, pack 128
    similar-degree nodes per window, deal windows round-robin across cores
    (slot i of every core shares one padded depth D_i so all cores run one
    identical SPMD program), and scatter each node's edges into its grid row.
    """

    def __init__(self, edge_index, n_nodes, n_cores):
        self.N = n_nodes
        self.C = n_cores
        src = np.asarray(edge_index[0], dtype=np.int64)
        dst = np.asarray(edge_index[1], dtype=np.int64)
        E = src.shape[0]

        deg = np.bincount(dst, minlength=n_nodes)
        order = np.argsort(deg, kind="stable")

        n_win_total = (n_nodes + P - 1) // P
        self.wpc = (n_win_total + n_cores - 1) // n_cores
        n_win = self.wpc * n_cores
        self.n_pad = n_win * P
        self.shard_nodes = self.wpc * P
        n_dummy = self.n_pad - n_nodes

        snode = np.full(self.n_pad, -1, dtype=np.int64)
        snode[n_dummy:] = order                      # ascending degree
        # rows_nodes[k][i, e] = natural node id at (core k, slot i, row e)
        self.rows_nodes = np.ascontiguousarray(
            snode.reshape(self.wpc, n_cores, P).transpose(1, 0, 2))

        wdeg = np.where(snode >= 0, deg[np.clip(snode, 0, None)], 0)
        wmax = wdeg.reshape(self.wpc, n_cores, P).max(axis=2)   # [wpc, cores]
        self.D = np.maximum(wmax.max(axis=1), 1).astype(np.int64)  # [wpc]
        self.off = np.concatenate([[0], np.cumsum(self.D)])
        self.TOT = int(self.D.sum())

        # position of each node in the sorted layout
        posq = np.empty(n_nodes, dtype=np.int64)
        posq[order] = np.arange(n_nodes) + n_dummy

        # scatter edges (dst-sorted, ranked within dst run) into grids
        perm = np.argsort(dst, kind="stable")
        src_s = src[perm]
        dst_s = dst[perm]
        bounds = np.searchsorted(dst_s, np.arange(n_nodes + 1))
        j_e = np.arange(E) - bounds[dst_s]           # rank within dst run
        q_e = posq[dst_s]
        g_e = q_e // P
        row_e = q_e % P
        core_e = g_e % n_cores
        slot_e = g_e // n_cores
        flat_e = self.off[slot_e] + j_e              # grid slot within [TOT]
        self.gidx = np.zeros((n_cores, self.TOT, P), dtype=np.int32)
        self.gidx[core_e, flat_e, row_e] = (src_s + 1).astype(np.int32)

        self.groups1 = self.make_groups(GCAP1, NWG1)
        self.groups2 = self.make_groups(GCAP2, NWG2)
        self.D_key = tuple(int(d) for d in self.D)

    def make_groups(self, gcap, nwg):
        """Window groups: sum(D) <= gcap, <= nwg windows per group."""
        groups = []
        i = 0
        while i < self.wpc:
            i0, sd, nw = i, 0, 0
            while (i < self.wpc and nw < nwg
                   and (nw == 0 or sd + int(self.D[i]) <= gcap)):
                sd += int(self.D[i])
                i += 1
                nw += 1
            groups.append((i0, nw, int(self.off[i0]), sd))
        return groups

    def stream_h(self, table, core):
        """[128, TOT*C] f16 grid gather: table rows by gidx (0 = zero pad)."""
        C = table.shape[1]
        tp = np.zeros((self.N + 1, C), dtype=np.float16)
        tp[1:] = table
        arr = tp[self.gidx[core]]                    # [TOT, P, C]
        return np.ascontiguousarray(arr.transpose(1, 0, 2)).reshape(
            P, self.TOT * C)

    def stream_als(self, table, core):
        """[128, TOT*H] f16: al_src grid; pad slots -> NEG_INF so exp()==0.
        Dummy rows get one j=0 slot with logit 0 so their softmax denominator
        stays finite (their h rows are zero, so the output row is 0)."""
        H = table.shape[1]
        tp = np.full((self.N + 1, H), NEG_INF, dtype=np.float16)
        tp[1:] = table
        arr = tp[self.gidx[core]]                    # [TOT, P, H]
        i_d, e_d = np.nonzero(self.rows_nodes[core] < 0)
        arr[self.off[i_d], e_d, :] = 0.0
        return np.ascontiguousarray(arr.transpose(1, 0, 2)).reshape(
            P, self.TOT * H)

    def stream_ald(self, table, core):
        """[128, wpc*H] f16: al_dst per (window, row). Dummy rows -> 0."""
        H = table.shape[1]
        tp = np.zeros((self.N + 1, H), dtype=np.float16)
        tp[1:] = table
        arr = tp[self.rows_nodes[core] + 1]          # [wpc, P, H]
        return np.ascontiguousarray(arr.transpose(1, 0, 2)).reshape(
            P, self.wpc * H)

    def stream_ald_exp(self, table, core):
        """[128, TOT*H] f16: al_dst replicated across each window's slots
        (slot grids are per-window blocks of D_i slots)."""
        H = table.shape[1]
        tp = np.zeros((self.N + 1, H), dtype=np.float16)
        tp[1:] = table
        arr = tp[self.rows_nodes[core] + 1]          # [wpc, P, H]
        rep = np.repeat(arr, self.D, axis=0)         # [TOT, P, H]
        return np.ascontiguousarray(rep.transpose(1, 0, 2)).reshape(
            P, self.TOT * H)

    def ident8(self):
        import ml_dtypes
        return np.eye(P, dtype=np.float32).astype(ml_dtypes.float8_e4m3)


# ------------------------------------------------------------------ builders

def _build_node(SH, c_in, m_h, m_al, elu, bias_in, bench_loop=1):
    """Per-node transform: hT = (elu?(xT+b)) @ w, alT = same @ wal.
    When m_h+m_al <= 128 the two matmuls merge into one.  The whole per-core
    panel is SBUF-resident: quarters stream in with fat DMAs, chunked matmuls
    write a staged output panel, and a few fat DMAs store it."""
    merged = (m_h + m_al) <= P
    M = m_h + m_al if merged else m_h
    QN = 4
    QS = SH // QN
    NQUAD = SH // (2 * CH)        # 2 al-chunks stack into one PSUM bank
    assert SH % QN == 0 and QS % CH == 0 and SH % (2 * CH) == 0
    nc = bass.Bass()
    xT = nc.dram_tensor("xT", [c_in, SH], F16, kind="ExternalInput")
    w = nc.dram_tensor("w", [c_in, M], F16, kind="ExternalInput")
    if not merged:
        assert m_al <= 32
        wal = nc.dram_tensor("wal", [c_in, 32], F16, kind="ExternalInput")
    if bias_in:
        bvec = nc.dram_tensor("bvec", [c_in, 1], F32, kind="ExternalInput")
    hT = nc.dram_tensor("hT", [M, SH], F16, kind="ExternalOutput")
    if not merged:
        # partition-stacked al panel: row 32k+r, col cq*CH+x holds
        # al[r] of chunk 2*cq+k (host unscrambles)
        alT = nc.dram_tensor("alT", [64, NQUAD * CH], F16,
                             kind="ExternalOutput")

    with tile.TileContext(nc) as tc:
        with (
            tc.tile_pool(name="const", bufs=1) as constp,
            tc.tile_pool(name="xin", bufs=2) as xinp,
            tc.tile_pool(name="hout", bufs=2) as houtp,
            tc.tile_pool(name="work", bufs=4) as workp,
            tc.tile_pool(name="psH", bufs=5, space="PSUM") as psH,
            tc.tile_pool(name="psA", bufs=3, space="PSUM") as psA,
        ):
            w_sb = constp.tile([c_in, M], F16)
            nc.sync.dma_start(out=w_sb[:], in_=w[:])
            if not merged:
                # wal host-padded to 32 cols (zeros) so every partition of
                # the stacked al PSUM region is written (no uninit reads)
                wal_sb = constp.tile([c_in, 32], F16)
                nc.sync.dma_start(out=wal_sb[:], in_=wal[:])
            if bias_in:
                b_sb = constp.tile([c_in, 1], F32)
                nc.sync.dma_start(out=b_sb[:], in_=bvec[:])

            def body(_iv=None):
                # every DMA rides SP: a queued DMA holds its issuing engine's
                # sequencer for the whole transfer, so ACT/DVE must stay clean
                xq = [xinp.tile([c_in, QS], F16, tag=f"x{q}", name=f"xq{q}")
                      for q in range(QN)]
                for q in range(QN):
                    nc.sync.dma_start(out=xq[q][:],
                                      in_=xT[:, q * QS:(q + 1) * QS])
                hq = [houtp.tile([M, QS], F16, tag=f"h{q}", name=f"hq{q}")
                      for q in range(QN)]
                if not merged:
                    alout = houtp.tile([64, NQUAD * CH], F16, tag="alo")
                quad = {}

                def qfront(q):
                    """Quarter-granular ELU stage A: one fat ACT exp."""
                    if not elu:
                        return None
                    rhs = xq[q][:]
                    if bias_in:
                        nc.vector.tensor_scalar(
                            rhs, rhs, b_sb[:, 0:1], None, OP.add)
                    et = workp.tile([c_in, QS], F16, tag="et")
                    nc.scalar.activation(et[:], rhs, AF.Exp)
                    return et

                def qback(q, et):
                    if elu:
                        # elu(x) = (min(exp(x),1) - 1) + max(x,0), all 2x DVE
                        mn = workp.tile([c_in, QS], F16, tag="mn")
                        nc.vector.tensor_scalar(
                            mn[:], et[:], 1.0, -1.0, OP.min, OP.add)
                        mx = workp.tile([c_in, QS], F16, tag="mx")
                        nc.vector.tensor_scalar(
                            mx[:], xq[q][:], 0.0, None, OP.max)
                        xe = workp.tile([c_in, QS], F16, tag="xe")
                        nc.vector.tensor_tensor(
                            out=xe[:], in0=mn[:], in1=mx[:], op=OP.add)
                        src = xe
                    else:
                        src = xq[q]
                    for j in range(QS // CH):
                        ci = q * (QS // CH) + j
                        qo = j * CH
                        rhs = src[:, qo:qo + CH]
                        ph = psH.tile([M, CH], F32, tag="ph")
                        nc.tensor.matmul(ph[:], w_sb[:], rhs,
                                         start=True, stop=True)
                        dve_copy = (ci % 7 < 3) if elu else (ci % 2 == 1)
                        if dve_copy:
                            nc.vector.tensor_copy(hq[q][:, qo:qo + CH],
                                                  ph[:])
                        else:
                            nc.scalar.activation(hq[q][:, qo:qo + CH],
                                                 ph[:], AF.Copy)
                        if not merged:
                            # stack 2 chunks' al outputs on partitions
                            # 0/32 of one PSUM bank -> 1 copy per pair
                            k = ci % 2
                            if k == 0:
                                quad["pa"] = psA.tile([64, CH], F32,
                                                      tag="paq", name="paq")
                            pa = quad["pa"]
                            nc.tensor.matmul(pa[32 * k:32 * k + 32, :],
                                             wal_sb[:], rhs,
                                             start=True, stop=True)
                            if k == 1:
                                cq = ci // 2
                                if cq % 2 == 0:
                                    nc.vector.tensor_copy(
                                        alout[:, cq * CH:(cq + 1) * CH],
                                        pa[:])
                                else:
                                    nc.scalar.activation(
                                        alout[:, cq * CH:(cq + 1) * CH],
                                        pa[:], AF.Copy)
                    nc.sync.dma_start(out=hT[:, q * QS:(q + 1) * QS],
                                      in_=hq[q][:])

                prev = None
                for q in range(QN):
                    et = qfront(q)
                    if prev is not None:
                        qback(*prev)
                    prev = (q, et)
                qback(*prev)
                if not merged:
                    nc.sync.dma_start(out=alT[:], in_=alout[:])

            if bench_loop > 1:
                with tc.For_i(0, bench_loop, 1) as _iv:
                    body(_iv)
            else:
                body()
    _finalize_kernel(nc)
    return nc


def _build_edge_g(D_list, groups, TOT, Cc, H, bias_out=False, elu_out=False,
                  ald_exp=False, bench_loop=1):
    """Edge aggregation over degree-sorted grids.  Per group of windows:
    one h[src] grid DMA, one DVE logit add per window, one ACT leaky-relu,
    one ACT exp into the message tile's trailing EB columns, one DVE
    multiply, then D accumulating identity matmuls per window.  Epilogues
    run one group late so no engine stalls on PSUM completion."""
    EB = 8 if H > 1 else 4   # exp block: 8 heads, or 4 replicas (1 head)
    SLOT = Cc + EB
    G = Cc // EB
    NW = len(D_list)
    GS = max(sd for _, _, _, sd in groups)
    NWmax = max(nw for _, nw, _, _ in groups)

    nc = bass.Bass()
    hsrc = nc.dram_tensor("hsrc", [P, TOT * Cc], F16, kind="ExternalInput")
    # ald_exp: als carries [al_src | al_dst] interleaved per slot (doubles
    # the per-partition dram run length past the 512 B fast-DMA threshold)
    als = nc.dram_tensor("als", [P, TOT * H * (2 if ald_exp else 1)], F16,
                         kind="ExternalInput")
    if not ald_exp:
        ald = nc.dram_tensor("ald", [P, NW * H], F16, kind="ExternalInput")
    ident = nc.dram_tensor("ident", [P, P], F8, kind="ExternalInput")
    if bias_out:
        brep = nc.dram_tensor("brep", [P, Cc], F32, kind="ExternalInput")
    # partition-major output: per-partition contiguous runs (the [NW*P, Cc]
    # layout had 128-256 B dram runs, under the 512 B fast-DMA threshold)
    out = nc.dram_tensor("out", [P, NW * Cc], F16, kind="ExternalOutput")

    with tile.TileContext(nc) as tc:
        with (
            tc.tile_pool(name="const", bufs=1) as constp,
            tc.tile_pool(name="aldp", bufs=2) as aldp,
            tc.tile_pool(name="alg", bufs=3) as algp,
            tc.tile_pool(name="hs", bufs=3) as hsp,
            tc.tile_pool(name="za", bufs=3) as zap,
            tc.tile_pool(name="msg", bufs=3) as msgp,
            tc.tile_pool(name="epi", bufs=3) as epip,
            tc.tile_pool(name="og", bufs=2) as ogp,
            tc.tile_pool(name="psW", bufs=8, space="PSUM") as pswp,
        ):
            BSLOT = 512 // SLOT      # windows per PSUM bank
            ident_sb = constp.tile([P, P], F8)
            nc.scalar.dma_start(out=ident_sb[:], in_=ident[:])
            ebias_sb = constp.tile([P, 1], F32)
            nc.vector.memset(ebias_sb[:], EXP_BIAS)
            if bias_out:
                brep_sb = constp.tile([P, Cc], F32)
                nc.scalar.dma_start(out=brep_sb[:], in_=brep[:])

            pend = []

            def front(grp, ald_sb):
                """DMA + logit add + leaky-relu + exp for one group."""
                i0, nw, off0, sd = grp
                hs = hsp.tile([P, GS * Cc], F16, tag="hs")
                nc.sync.dma_start(out=hs[:, :sd * Cc],
                                  in_=hsrc[:, off0 * Cc:(off0 + sd) * Cc])
                AW = H * (2 if ald_exp else 1)
                alg = algp.tile([P, GS * AW], F16, tag="alg")
                nc.sync.dma_start(out=alg[:, :sd * AW],
                                  in_=als[:, off0 * AW:(off0 + sd) * AW])
                za = zap.tile([P, GS * H], F16, tag="za")
                if ald_exp:
                    # interleaved [al_src | al_dst] slots: one add per group
                    a0 = alg[:]
                    av = bass.AP(a0.tensor, a0.offset, [a0.ap[0], [2, sd]])
                    bv = bass.AP(a0.tensor, a0.offset + 1,
                                 [a0.ap[0], [2, sd]])
                    nc.vector.tensor_tensor(out=za[:, :sd],
                                            in0=av, in1=bv, op=OP.add)
                doff = 0
                for wl in range(nw) if not ald_exp else ():
                    D = int(D_list[i0 + wl])
                    o0 = doff * H
                    if H > 1:
                        av = alg[:, o0:o0 + D * H].rearrange(
                            "p (d h) -> p d h", d=D)
                        zv = za[:, o0:o0 + D * H].rearrange(
                            "p (d h) -> p d h", d=D)
                        ad = ald_sb[:, (i0 + wl) * H:(i0 + wl + 1) * H]
                        ab = bass.AP(ad.tensor, ad.offset,
                                     [ad.ap[0], [0, D], [1, H]])
                    else:
                        av = alg[:, o0:o0 + D]
                        zv = za[:, o0:o0 + D]
                        ad = ald_sb[:, i0 + wl:i0 + wl + 1]
                        ab = bass.AP(ad.tensor, ad.offset,
                                     [ad.ap[0], [0, D]])
                    nc.vector.tensor_tensor(out=zv, in0=av, in1=ab, op=OP.add)
                    doff += D
                nc.scalar.activation(za[:, :sd * H], za[:, :sd * H],
                                     AF.Prelu, alpha=NEG_SLOPE)
                msg = msgp.tile([P, GS * SLOT], F16, tag="msg")
                m3 = msg[:, :sd * SLOT].rearrange("p (d s) -> p d s", s=SLOT)
                eb_out = m3[:, :, Cc:Cc + EB]
                if H > 1:
                    e_in = za[:, :sd * H].rearrange("p (d h) -> p d h", d=sd)
                else:
                    z0 = za[:, :sd]
                    e_in = bass.AP(z0.tensor, z0.offset,
                                   [z0.ap[0], [1, sd], [0, EB]])
                nc.scalar.activation(eb_out, e_in, AF.Exp, bias=ebias_sb[:])
                return hs, msg

            def back(grp, st):
                """DVE message multiply + PE identity accumulation."""
                i0, nw, off0, sd = grp
                hs, msg = st
                m3 = msg[:, :sd * SLOT].rearrange("p (d s) -> p d s", s=SLOT)
                eb_out = m3[:, :, Cc:Cc + EB]
                mo = m3[:, :, 0:Cc].rearrange("p d (g h) -> p d g h", h=EB)
                hi = hs[:, :sd * Cc].rearrange(
                    "p (d g h) -> p d g h", d=sd, h=EB)
                ei = bass.AP(eb_out.tensor, eb_out.offset,
                             [eb_out.ap[0], eb_out.ap[1], [0, G], [1, EB]])
                nc.vector.tensor_tensor(out=mo, in0=hi, in1=ei, op=OP.mult)
                doff = 0
                bank = None
                for wl in range(nw):
                    D = int(D_list[i0 + wl])
                    if wl % BSLOT == 0:
                        bank = pswp.tile([P, 512], F32, tag="psw",
                                         name="pswbank")
                    sl = (wl % BSLOT) * SLOT
                    psw = bank[:, sl:sl + SLOT]
                    for j in range(D):
                        mv = msg[:, (doff + j) * SLOT:(doff + j + 1) * SLOT]
                        nc.tensor.matmul(psw, ident_sb[:], mv,
                                         start=(j == 0), stop=(j == D - 1))
                    pend.append(psw)
                    doff += D

            ogst = {}

            def epilogue(grp, flush):
                """One f16 PSUM copy per window, then a single reciprocal +
                scale per group; output DMAs batch two groups per write so
                HBM sees fewer read/write turnarounds against the streams."""
                i0, nw, off0, sd = grp
                op_t = epip.tile([P, NWmax * SLOT], F16, tag="o1p")
                for wl in range(nw):
                    psw = pend.pop(0)
                    nc.scalar.activation(op_t[:, wl * SLOT:(wl + 1) * SLOT],
                                         psw, AF.Copy)
                opv = op_t[:, :nw * SLOT]
                rec = epip.tile([P, NWmax * EB], F16, tag="rec")
                rv = rec[:, :nw * EB].rearrange("p (w h) -> p w h", w=nw)
                dap = bass.AP(opv.tensor, opv.offset + Cc,
                              [opv.ap[0], [SLOT, nw], [1, EB]])
                with nc.allow_low_precision(
                        reason="softmax denominators are O(1)"):
                    nc.vector.reciprocal(rv, dap)
                if not ogst:
                    ogst["og"] = ogp.tile([P, 2 * NWmax * Cc], F16,
                                          tag="og", name="ogpair")
                    ogst["i0"] = i0
                    ogst["fill"] = 0
                og = ogst["og"]
                ob = ogst["fill"]
                o_in = bass.AP(opv.tensor, opv.offset,
                               [opv.ap[0], [SLOT, nw], [EB, G], [1, EB]])
                r0 = rec[:]
                r_b = bass.AP(r0.tensor, r0.offset,
                              [r0.ap[0], [EB, nw], [0, G], [1, EB]])
                oo = og[:, ob:ob + nw * Cc].rearrange(
                    "p (w g h) -> p w g h", w=nw, h=EB)
                nc.vector.tensor_tensor(out=oo, in0=o_in, in1=r_b,
                                        op=OP.mult)
                if bias_out:     # layer bias: before the inter-layer elu
                    ov2 = og[:, ob:ob + nw * Cc].rearrange(
                        "p (w c) -> p w c", w=nw)
                    b0 = brep_sb[:]
                    b_b = bass.AP(b0.tensor, b0.offset,
                                  [b0.ap[0], [0, nw], [1, Cc]])
                    nc.vector.tensor_tensor(out=ov2, in0=ov2, in1=b_b,
                                            op=OP.add)
                if elu_out:
                    # elu(x) = max(x,0) + (min(exp(x),1) - 1), in place on og
                    ogv = og[:, ob:ob + nw * Cc]
                    et = epip.tile([P, NWmax * Cc], F16, tag="et")
                    etv = et[:, :nw * Cc]
                    nc.scalar.activation(etv, ogv, AF.Exp)
                    nc.vector.tensor_scalar(etv, etv, 1.0, -1.0,
                                            OP.min, OP.add)
                    nc.vector.scalar_tensor_tensor(ogv, ogv, 0.0, etv,
                                                   OP.max, OP.add)
                ogst["fill"] = ob + nw * Cc
                if flush:
                    f = ogst["fill"]
                    o0 = ogst["i0"] * Cc
                    nc.scalar.dma_start(out=out[:, o0:o0 + f],
                                        in_=og[:, :f])
                    ogst.clear()

            def body(_iv=None):
                if not ald_exp:
                    ald_sb = aldp.tile([P, NW * H], F16, tag="ald")
                    nc.scalar.dma_start(out=ald_sb[:], in_=ald[:])
                else:
                    ald_sb = None
                pend.clear()
                ogst.clear()
                ng = len(groups)
                ep = [0]

                def run_epi(gi):
                    epilogue(groups[gi],
                             flush=(ep[0] % 2 == 1) or (gi == ng - 1))
                    ep[0] += 1

                sts = [None] * ng
                for gi, grp in enumerate(groups):
                    sts[gi] = front(grp, ald_sb)
                    if gi >= 1:
                        back(groups[gi - 1], sts[gi - 1])
                        sts[gi - 1] = None
                    if gi >= 2:
                        run_epi(gi - 2)
                back(groups[ng - 1], sts[ng - 1])
                if ng >= 2:
                    run_epi(ng - 2)
                run_epi(ng - 1)

            if bench_loop > 1:
                with tc.For_i(0, bench_loop, 1) as _iv:
                    body(_iv)
            else:
                body()
    _finalize_kernel(nc)
    return nc


# ------------------------------------------------------------------ runner

def _fold_att(W, a):
    heads, hid = a.shape
    return np.einsum("ihc,hc->ih", W.reshape(W.shape[0], heads, hid), a)


class _GatRunner:
    def __init__(self, n_cores=N_CORES):
        self.C = n_cores
        self._graph = None
        self._graph_key = None
        self._kernels = {}
        self.last_maps = {}

    def graph(self, edge_index, n_nodes):
        key = hash(np.asarray(edge_index).tobytes())
        if key != self._graph_key:
            self._graph = _Graph(edge_index, n_nodes, self.C)
            self._graph_key = key
            self._kernels.clear()
        return self._graph

    def kernel(self, name, bench_loop=1, **kw):
        key = (name, bench_loop, tuple(sorted(kw.items())))
        if key not in self._kernels:
            g = self._graph
            if name.startswith("P"):
                self._kernels[key] = _build_node(
                    g.shard_nodes, bench_loop=bench_loop, **kw)
            elif name == "E1":
                self._kernels[key] = _build_edge_g(
                    g.D, g.groups1, g.TOT, 128, 8,
                    bench_loop=bench_loop, **kw)
            else:
                self._kernels[key] = _build_edge_g(
                    g.D, g.groups2, g.TOT, 64, 1, ald_exp=True,
                    bench_loop=bench_loop, **kw)
        return self._kernels[key]

    def _run(self, name, nc, maps):
        self.last_maps[name] = maps
        res = run_bass_kernel_spmd(nc, maps, core_ids=list(range(self.C)))
        return res.results

    def run(self, x, edge_index, W1, a_src1, a_dst1, b1, W2, a_src2, a_dst2,
            b2):
        C = self.C
        N, IN_C = x.shape
        HEADS, HID = a_src1.shape
        HC = HEADS * HID
        OUT_C = W2.shape[1]
        g = self.graph(edge_index, N)
        SH = g.shard_nodes
        # (c,h)-interleaved channel order for the layer-1 hidden features:
        # col c*H+h of h1 holds math channel h*HID+c. Folded into W1's
        # columns (P0) and W2's rows (P2) on the host - pure permutation.
        perm = np.array([(j % HEADS) * HID + j // HEADS
                         for j in range(HC)], dtype=np.int64)

        # ---- P0: per-node h1 / logits --------------------------------
        xT_pad = np.zeros((IN_C, g.n_pad), dtype=np.float16)
        xT_pad[:, :N] = np.asarray(x, np.float32).T
        w1 = np.asarray(W1, np.float32)
        m_al = 2 * HEADS
        wal1 = np.zeros((IN_C, 32), dtype=np.float32)
        wal1[:, :m_al] = np.concatenate(
            [_fold_att(w1, np.asarray(a_src1, np.float32)),
             _fold_att(w1, np.asarray(a_dst1, np.float32))], axis=1)
        mapsP0 = [{"xT": np.ascontiguousarray(xT_pad[:, k * SH:(k + 1) * SH]),
                   "w": np.ascontiguousarray(w1[:, perm]).astype(np.float16),
                   "wal": wal1.astype(np.float16)} for k in range(C)]
        ncP0 = self.kernel("P0", c_in=IN_C, m_h=HC, m_al=m_al,
                           elu=False, bias_in=False)
        resP0 = self._run("P0", ncP0, mapsP0)
        h1 = np.ascontiguousarray(
            np.concatenate([r["hT"] for r in resP0], axis=1).T)[:N]
        # unscramble the partition-stacked al panel: row 32k+r, col cq*CH+x
        # holds al[r] of chunk 4*cq+k
        nq = SH // (2 * CH)
        al1 = np.concatenate(
            [r["alT"].reshape(2, 32, nq, CH)[:, :m_al]
             .transpose(1, 2, 0, 3).reshape(m_al, SH)
             for r in resP0], axis=1)                    # [16, Np]
        als1 = np.ascontiguousarray(al1[:HEADS, :N].T)
        ald1 = np.ascontiguousarray(al1[HEADS:, :N].T)

        # ---- E1: layer-1 edge aggregation + bias + ELU ---------------
        id8 = g.ident8()
        b1nz = bool(np.any(np.asarray(b1)))
        mapsE1 = []
        for k in range(C):
            m = {"hsrc": g.stream_h(h1, k),
                 "als": g.stream_als(als1, k),
                 "ald": g.stream_ald(ald1, k),
                 "ident": id8}
            if b1nz:
                m["brep"] = np.tile(
                    np.asarray(b1, np.float32)[perm], (P, 1))
            mapsE1.append(m)
        ncE1 = self.kernel("E1", bias_out=b1nz)
        resE1 = self._run("E1", ncE1, mapsE1)
        out1 = np.concatenate(
            [r["out"].reshape(P, g.wpc, HC).transpose(1, 0, 2)
             .reshape(g.wpc * P, HC) for r in resE1], axis=0)
        # rows of out1 are (core, slot, row) -> natural node rowmap
        rowmap = g.rows_nodes.reshape(-1)            # [C*wpc*P]

        # ---- P2: ELU + per-node h2 / logits --------------------------
        o1T = np.ascontiguousarray(out1.T)           # [HC, C*SH] f16
        w2 = np.asarray(W2, np.float32)
        wal2 = np.concatenate(
            [_fold_att(w2, np.asarray(a_src2, np.float32)),
             _fold_att(w2, np.asarray(a_dst2, np.float32))], axis=1)
        w2all = np.concatenate([w2[perm], wal2[perm]], axis=1)  # [HC, 66]
        mapsP2 = [
            {"xT": np.ascontiguousarray(o1T[:, k * SH:(k + 1) * SH]),
             "w": w2all.astype(np.float16)} for k in range(C)]
        # out1 already carries b1 (E1 bias_out); P2 applies the ELU
        ncP2 = self.kernel("P2", c_in=HC, m_h=OUT_C, m_al=2, elu=True,
                           bias_in=False)
        resP2 = self._run("P2", ncP2, mapsP2)
        h2al = np.concatenate([r["hT"] for r in resP2], axis=1)  # [66, Np]
        valid = rowmap >= 0
        vrows = rowmap[valid]
        h2 = np.zeros((N, OUT_C), dtype=np.float16)
        h2[vrows] = h2al[:OUT_C].T[valid]
        als2 = np.zeros((N, 1), dtype=np.float16)
        als2[vrows, 0] = h2al[OUT_C][valid]
        ald2 = np.zeros((N, 1), dtype=np.float16)
        ald2[vrows, 0] = h2al[OUT_C + 1][valid]

        # ---- E2: layer-2 edge aggregation ----------------------------
        b2nz = bool(np.any(np.asarray(b2)))
        mapsE2 = []
        for k in range(C):
            a_s = g.stream_als(als2, k).reshape(P, g.TOT)
            a_d = g.stream_ald_exp(ald2, k).reshape(P, g.TOT)
            m = {"hsrc": g.stream_h(h2, k),
                 "als": np.ascontiguousarray(
                     np.stack([a_s, a_d], axis=2)).reshape(P, g.TOT * 2),
                 "ident": id8}
            if b2nz:
                m["brep"] = np.tile(np.asarray(b2, np.float32), (P, 1))
            mapsE2.append(m)
        ncE2 = self.kernel("E2", bias_out=b2nz)
        resE2 = self._run("E2", ncE2, mapsE2)
        out2 = np.concatenate(
            [r["out"].reshape(P, g.wpc, OUT_C).transpose(1, 0, 2)
             .reshape(g.wpc * P, OUT_C) for r in resE2], axis=0)
        out_full = np.zeros((N, OUT_C), dtype=np.float32)
        out_full[vrows] = out2[valid]
        return out_full


_RUNNER = _GatRunner()


def kernel(x, edge_index, W1, a_src1, a_dst1, b1, W2, a_src2, a_dst2, b2):
    """Full-input / full-output entry point. Returns [N, OUT_C] float32."""
    args = [np.asarray(v) for v in
            (x, edge_index, W1, a_src1, a_dst1, b1, W2, a_src2, a_dst2, b2)]
    return _RUNNER.run(*args).astype(np.float32)


# revision 57
# speedup vs baseline: 1.0698x; 1.0004x over previous
"""Trainium (trn2) Bass kernel for a 2-layer GAT over N=100k nodes / E=1.7M edges.

Strategy (degree-sorted edge grids + identity-stationary PE accumulation)
-------------------------------------------------------------------------
Nodes are sorted by in-degree on the host and packed into windows of 128
similar-degree destination nodes; windows are dealt round-robin across the 8
NeuronCores.  Each window's edges form a dense grid [128 nodes x D slots]
(D = max in-window degree, padded slots carry -inf logits so exp()==0), so
slot j of all 128 nodes is a 128-edge tile whose destination map is the
IDENTITY: the tensor engine accumulates the per-slot message tiles straight
into the window's PSUM bank with a never-changing fp8 identity stationary.
Degree sorting keeps grid padding at ~1.3%, and the one-hot selection stream
of the classic dst-sorted formulation (128 B/edge of pure index overhead)
disappears entirely.

Each GAT layer runs as TWO SPMD kernels with host-side index gathers (pure
permutations / casts - no host FLOPs) between them:

* node kernel (P0/P2): h = x @ W plus folded attention logits computed once
  per node (dense matmuls).  The full per-core input/output panels live in
  SBUF, loaded/stored with a handful of fat DMAs (per-chunk 1 KB/partition
  DMAs were latency-bound at ~140 GB/s); every DMA rides the SP queue since
  a queued DMA holds its issuing engine's sequencer for the whole transfer.
  P0's 16 logit rows stack two chunks per PSUM bank at partitions 0/32
  (tile_position) so one PSUM->SBUF copy drains two matmuls; P2 computes the
  inter-layer ELU as exp (one fat ACT op per quarter-panel, emitted a
  quarter ahead) + two 2x DVE ops, with PSUM copies balanced across ACT/DVE.
* edge kernel (E1/E2): streams h[src] grids (256/128 B per edge slot) and
  al_src logit grids (16/2 B); al_dst is a tiny per-window constant for E1
  and a host-replicated per-slot stream for E2 (one group-wide DVE add
  instead of 21 window-sized ones).  Windows are processed in groups
  (sum of D <= 96/192) software-pipelined three deep: group g's DMA +
  logits + leaky-relu + exp land while g-1 runs its DVE multiply + PE
  accumulation and g-2 runs its epilogue, so no engine ever stalls on
  another's latency.  ACT writes exp(z-4) into the message tile's trailing
  8 columns ((c,h)-interleaved broadcast for layer 1's 8 heads, an 8x
  replica for layer 2's single head so the DVE multiply keeps its
  packed-innermost 2x mode).  Epilogues drain each window's PSUM with a
  single f16 ACT copy, then one reciprocal + one scale per group, into a
  partition-major [128, NW*C] output panel (the row-major layout's 128-256 B
  dram runs fell under the 512 B threshold where DMA cost doubles; the host
  unscrambles for free).

Measured per-core DMA floor is ~343 GB/s on one queue / ~355 on two (HBM
fair share); the edge kernels stream ~62/~32 MB per core per inference and
run within ~15% of that floor.

Environment workarounds: this container's walrus build allows only ONE
semaphore wait per instruction (split onto nop carriers post-scheduling), and
the GPSIMD ucode libraries are absent (so no dma_gather/indirect-DMA fast
paths - hence the host-gather design).
"""
import numpy as np

import concourse.bass as bass
import concourse.mybir as mybir
import concourse.tile as tile
from concourse.bass_utils import run_bass_kernel_spmd

P = 128
F16 = mybir.dt.float16
F32 = mybir.dt.float32
F8 = mybir.dt.float8e4
AF = mybir.ActivationFunctionType
OP = mybir.AluOpType
NEG_SLOPE = 0.2
EXP_BIAS = -4.0     # exp(z + EXP_BIAS): constant shift cancels in softmax
NEG_INF = -60000.0  # pad-slot logit: exp(lrelu(.)+bias) underflows to 0
N_CORES = 8
EPS = 1e-30
CH = 448            # node-kernel matmul chunk (PSUM: 448*4B <= 2KB bank)
GCAP1, NWG1 = 90, 8      # E1 groups: count % 3 == 0 so the For_i seam's
                         # first DMA reuses a buffer freed 3 groups early
GCAP2, NWG2 = 208, 21    # E2 groups: ditto (9 groups, % 3 == 0)

# ------------------------------------------------------------------ patches

_wsplit_counter = [0]


def _split_excess_waits(nc, max_waits=1):
    """This walrus build rejects >1 sem-wait per instruction ("Too many sync
    wait commands"). Move overflow waits onto same-engine nop carriers."""
    n_split = 0
    for f in nc.m.functions:
        for blk in f.blocks:
            changed = False
            out = []
            for inst in blk.instructions:
                si = inst.sync_info
                if si is not None and len(si.on_wait) > max_waits:
                    waits = list(si.on_wait)
                    keep = waits[len(waits) - max_waits:]
                    overflow = waits[: len(waits) - max_waits]
                    for i in range(0, len(overflow), max_waits):
                        _wsplit_counter[0] += 1
                        nop = mybir.InstNoOp(
                            name=f"I-wsplit-{_wsplit_counter[0]}", ins=[], outs=[])
                        nop.engine = inst.engine
                        nop.sync_info = mybir.SyncInfo(
                            on_wait=overflow[i: i + max_waits], on_update=[])
                        out.append(nop)
                    inst.sync_info = mybir.SyncInfo(
                        on_wait=keep, on_update=list(si.on_update))
                    changed = True
                    n_split += 1
                out.append(inst)
            if changed:
                blk.instructions = out
    return n_split


def _finalize_kernel(nc):
    import bass_rust as _bass_rust
    from concourse.library_config import all_libraries, standard
    from concourse.library_overlay import lower_extended_insts

    inst_type_to_lib_mask = {}
    for lib in all_libraries:
        for inst_type in lib.instructions:
            inst_type_to_lib_mask[inst_type] = inst_type_to_lib_mask.get(
                inst_type, 0) | (1 << lib.index)
    _bass_rust.insert_library_loads(
        nc, inst_type_to_lib_mask, len(all_libraries), standard.index)
    lower_extended_insts(nc)
    _split_excess_waits(nc)


# ------------------------------------------------------------------ host prep

class _Graph:
    """Degree-sorted grid preprocessing: sort nodes by in-degree, pack 128
    similar-degree nodes per window, deal windows round-robin across cores
    (slot i of every core shares one padded depth D_i so all cores run one
    identical SPMD program), and scatter each node's edges into its grid row.
    """

    def __init__(self, edge_index, n_nodes, n_cores):
        self.N = n_nodes
        self.C = n_cores
        src = np.asarray(edge_index[0], dtype=np.int64)
        dst = np.asarray(edge_index[1], dtype=np.int64)
        E = src.shape[0]

        deg = np.bincount(dst, minlength=n_nodes)
        order = np.argsort(deg, kind="stable")

        n_win_total = (n_nodes + P - 1) // P
        self.wpc = (n_win_total + n_cores - 1) // n_cores
        n_win = self.wpc * n_cores
        self.n_pad = n_win * P
        self.shard_nodes = self.wpc * P
        n_dummy = self.n_pad - n_nodes

        snode = np.full(self.n_pad, -1, dtype=np.int64)
        snode[n_dummy:] = order                      # ascending degree
        # rows_nodes[k][i, e] = natural node id at (core k, slot i, row e)
        self.rows_nodes = np.ascontiguousarray(
            snode.reshape(self.wpc, n_cores, P).transpose(1, 0, 2))

        wdeg = np.where(snode >= 0, deg[np.clip(snode, 0, None)], 0)
        wmax = wdeg.reshape(self.wpc, n_cores, P).max(axis=2)   # [wpc, cores]
        self.D = np.maximum(wmax.max(axis=1), 1).astype(np.int64)  # [wpc]
        self.off = np.concatenate([[0], np.cumsum(self.D)])
        self.TOT = int(self.D.sum())

        # position of each node in the sorted layout
        posq = np.empty(n_nodes, dtype=np.int64)
        posq[order] = np.arange(n_nodes) + n_dummy

        # scatter edges (dst-sorted, ranked within dst run) into grids
        perm = np.argsort(dst, kind="stable")
        src_s = src[perm]
        dst_s = dst[perm]
        bounds = np.searchsorted(dst_s, np.arange(n_nodes + 1))
        j_e = np.arange(E) - bounds[dst_s]           # rank within dst run
        q_e = posq[dst_s]
        g_e = q_e // P
        row_e = q_e % P
        core_e = g_e % n_cores
        slot_e = g_e // n_cores
        flat_e = self.off[slot_e] + j_e              # grid slot within [TOT]
        self.gidx = np.zeros((n_cores, self.TOT, P), dtype=np.int32)
        self.gidx[core_e, flat_e, row_e] = (src_s + 1).astype(np.int32)

        self.groups1 = self.make_groups(GCAP1, NWG1)
        self.groups2 = self.make_groups(GCAP2, NWG2)
        self.D_key = tuple(int(d) for d in self.D)

    def make_groups(self, gcap, nwg):
        """Window groups: sum(D) <= gcap, <= nwg windows per group."""
        groups = []
        i = 0
        while i < self.wpc:
            i0, sd, nw = i, 0, 0
            while (i < self.wpc and nw < nwg
                   and (nw == 0 or sd + int(self.D[i]) <= gcap)):
                sd += int(self.D[i])
                i += 1
                nw += 1
            groups.append((i0, nw, int(self.off[i0]), sd))
        return groups

    def stream_h(self, table, core):
        """[128, TOT*C] f16 grid gather: table rows by gidx (0 = zero pad)."""
        C = table.shape[1]
        tp = np.zeros((self.N + 1, C), dtype=np.float16)
        tp[1:] = table
        arr = tp[self.gidx[core]]                    # [TOT, P, C]
        return np.ascontiguousarray(arr.transpose(1, 0, 2)).reshape(
            P, self.TOT * C)

    def stream_als(self, table, core):
        """[128, TOT*H] f16: al_src grid; pad slots -> NEG_INF so exp()==0.
        Dummy rows get one j=0 slot with logit 0 so their softmax denominator
        stays finite (their h rows are zero, so the output row is 0)."""
        H = table.shape[1]
        tp = np.full((self.N + 1, H), NEG_INF, dtype=np.float16)
        tp[1:] = table
        arr = tp[self.gidx[core]]                    # [TOT, P, H]
        i_d, e_d = np.nonzero(self.rows_nodes[core] < 0)
        arr[self.off[i_d], e_d, :] = 0.0
        return np.ascontiguousarray(arr.transpose(1, 0, 2)).reshape(
            P, self.TOT * H)

    def stream_ald(self, table, core):
        """[128, wpc*H] f16: al_dst per (window, row). Dummy rows -> 0."""
        H = table.shape[1]
        tp = np.zeros((self.N + 1, H), dtype=np.float16)
        tp[1:] = table
        arr = tp[self.rows_nodes[core] + 1]          # [wpc, P, H]
        return np.ascontiguousarray(arr.transpose(1, 0, 2)).reshape(
            P, self.wpc * H)

    def stream_ald_exp(self, table, core):
        """[128, TOT*H] f16: al_dst replicated across each window's slots
        (slot grids are per-window blocks of D_i slots)."""
        H = table.shape[1]
        tp = np.zeros((self.N + 1, H), dtype=np.float16)
        tp[1:] = table
        arr = tp[self.rows_nodes[core] + 1]          # [wpc, P, H]
        rep = np.repeat(arr, self.D, axis=0)         # [TOT, P, H]
        return np.ascontiguousarray(rep.transpose(1, 0, 2)).reshape(
            P, self.TOT * H)

    def ident8(self):
        import ml_dtypes
        return np.eye(P, dtype=np.float32).astype(ml_dtypes.float8_e4m3)


# ------------------------------------------------------------------ builders

def _build_node(SH, c_in, m_h, m_al, elu, bias_in, bench_loop=1):
    """Per-node transform: hT = (elu?(xT+b)) @ w, alT = same @ wal.
    When m_h+m_al <= 128 the two matmuls merge into one.  The whole per-core
    panel is SBUF-resident: quarters stream in with fat DMAs, chunked matmuls
    write a staged output panel, and a few fat DMAs store it."""
    merged = (m_h + m_al) <= P
    M = m_h + m_al if merged else m_h
    QN = 4
    QS = SH // QN
    NQUAD = SH // (2 * CH)        # 2 al-chunks stack into one PSUM bank
    assert SH % QN == 0 and QS % CH == 0 and SH % (2 * CH) == 0
    nc = bass.Bass()
    xT = nc.dram_tensor("xT", [c_in, SH], F16, kind="ExternalInput")
    w = nc.dram_tensor("w", [c_in, M], F16, kind="ExternalInput")
    if not merged:
        assert m_al <= 32
        wal = nc.dram_tensor("wal", [c_in, 32], F16, kind="ExternalInput")
    if bias_in:
        bvec = nc.dram_tensor("bvec", [c_in, 1], F32, kind="ExternalInput")
    hT = nc.dram_tensor("hT", [M, SH], F16, kind="ExternalOutput")
    if not merged:
        # partition-stacked al panel: row 32k+r, col cq*CH+x holds
        # al[r] of chunk 2*cq+k (host unscrambles)
        alT = nc.dram_tensor("alT", [64, NQUAD * CH], F16,
                             kind="ExternalOutput")

    with tile.TileContext(nc) as tc:
        with (
            tc.tile_pool(name="const", bufs=1) as constp,
            tc.tile_pool(name="xin", bufs=2) as xinp,
            tc.tile_pool(name="hout", bufs=2) as houtp,
            tc.tile_pool(name="work", bufs=4) as workp,
            tc.tile_pool(name="psH", bufs=5, space="PSUM") as psH,
            tc.tile_pool(name="psA", bufs=3, space="PSUM") as psA,
        ):
            w_sb = constp.tile([c_in, M], F16)
            nc.sync.dma_start(out=w_sb[:], in_=w[:])
            if not merged:
                # wal host-padded to 32 cols (zeros) so every partition of
                # the stacked al PSUM region is written (no uninit reads)
                wal_sb = constp.tile([c_in, 32], F16)
                nc.sync.dma_start(out=wal_sb[:], in_=wal[:])
            if bias_in:
                b_sb = constp.tile([c_in, 1], F32)
                nc.sync.dma_start(out=b_sb[:], in_=bvec[:])

            def body(_iv=None):
                # every DMA rides SP: a queued DMA holds its issuing engine's
                # sequencer for the whole transfer, so ACT/DVE must stay clean
                xq = [xinp.tile([c_in, QS], F16, tag=f"x{q}", name=f"xq{q}")
                      for q in range(QN)]
                for q in range(QN):
                    nc.sync.dma_start(out=xq[q][:],
                                      in_=xT[:, q * QS:(q + 1) * QS])
                hq = [houtp.tile([M, QS], F16, tag=f"h{q}", name=f"hq{q}")
                      for q in range(QN)]
                if not merged:
                    alout = houtp.tile([64, NQUAD * CH], F16, tag="alo")
                quad = {}

                def qfront(q):
                    """Quarter-granular ELU stage A: one fat ACT exp."""
                    if not elu:
                        return None
                    rhs = xq[q][:]
                    if bias_in:
                        nc.vector.tensor_scalar(
                            rhs, rhs, b_sb[:, 0:1], None, OP.add)
                    et = workp.tile([c_in, QS], F16, tag="et")
                    nc.scalar.activation(et[:], rhs, AF.Exp)
                    return et

                def qback(q, et):
                    if elu:
                        # elu(x) = (min(exp(x),1) - 1) + max(x,0), all 2x DVE
                        mn = workp.tile([c_in, QS], F16, tag="mn")
                        nc.vector.tensor_scalar(
                            mn[:], et[:], 1.0, -1.0, OP.min, OP.add)
                        mx = workp.tile([c_in, QS], F16, tag="mx")
                        nc.vector.tensor_scalar(
                            mx[:], xq[q][:], 0.0, None, OP.max)
                        xe = workp.tile([c_in, QS], F16, tag="xe")
                        nc.vector.tensor_tensor(
                            out=xe[:], in0=mn[:], in1=mx[:], op=OP.add)
                        src = xe
                    else:
                        src = xq[q]
                    for j in range(QS // CH):
                        ci = q * (QS // CH) + j
                        qo = j * CH
                        rhs = src[:, qo:qo + CH]
                        ph = psH.tile([M, CH], F32, tag="ph")
                        nc.tensor.matmul(ph[:], w_sb[:], rhs,
                                         start=True, stop=True)
                        dve_copy = (ci % 7 < 3) if elu else (ci % 2 == 1)
                        if dve_copy:
                            nc.vector.tensor_copy(hq[q][:, qo:qo + CH],
                                                  ph[:])
                        else:
                            nc.scalar.activation(hq[q][:, qo:qo + CH],
                                                 ph[:], AF.Copy)
                        if not merged:
                            # stack 2 chunks' al outputs on partitions
                            # 0/32 of one PSUM bank -> 1 copy per pair
                            k = ci % 2
                            if k == 0:
                                quad["pa"] = psA.tile([64, CH], F32,
                                                      tag="paq", name="paq")
                            pa = quad["pa"]
                            nc.tensor.matmul(pa[32 * k:32 * k + 32, :],
                                             wal_sb[:], rhs,
                                             start=True, stop=True)
                            if k == 1:
                                cq = ci // 2
                                if cq % 2 == 0:
                                    nc.vector.tensor_copy(
                                        alout[:, cq * CH:(cq + 1) * CH],
                                        pa[:])
                                else:
                                    nc.scalar.activation(
                                        alout[:, cq * CH:(cq + 1) * CH],
                                        pa[:], AF.Copy)
                    nc.sync.dma_start(out=hT[:, q * QS:(q + 1) * QS],
                                      in_=hq[q][:])

                prev = None
                for q in range(QN):
                    et = qfront(q)
                    if prev is not None:
                        qback(*prev)
                    prev = (q, et)
                qback(*prev)
                if not merged:
                    nc.sync.dma_start(out=alT[:], in_=alout[:])

            if bench_loop > 1:
                with tc.For_i(0, bench_loop, 1) as _iv:
                    body(_iv)
            else:
                body()
    _finalize_kernel(nc)
    return nc


def _build_edge_g(D_list, groups, TOT, Cc, H, bias_out=False, elu_out=False,
                  ald_exp=False, bench_loop=1):
    """Edge aggregation over degree-sorted grids.  Per group of windows:
    one h[src] grid DMA, one DVE logit add per window, one ACT leaky-relu,
    one ACT exp into the message tile's trailing EB columns, one DVE
    multiply, then D accumulating identity matmuls per window.  Epilogues
    run one group late so no engine stalls on PSUM completion."""
    EB = 8 if H > 1 else 4   # exp block: 8 heads, or 4 replicas (1 head)
    SLOT = Cc + EB
    G = Cc // EB
    NW = len(D_list)
    GS = max(sd for _, _, _, sd in groups)
    NWmax = max(nw for _, nw, _, _ in groups)

    nc = bass.Bass()
    hsrc = nc.dram_tensor("hsrc", [P, TOT * Cc], F16, kind="ExternalInput")
    # ald_exp: als carries [al_src | al_dst] interleaved per slot (doubles
    # the per-partition dram run length past the 512 B fast-DMA threshold)
    als = nc.dram_tensor("als", [P, TOT * H * (2 if ald_exp else 1)], F16,
                         kind="ExternalInput")
    if not ald_exp:
        ald = nc.dram_tensor("ald", [P, NW * H], F16, kind="ExternalInput")
    ident = nc.dram_tensor("ident", [P, P], F8, kind="ExternalInput")
    if bias_out:
        brep = nc.dram_tensor("brep", [P, Cc], F32, kind="ExternalInput")
    # partition-major output: per-partition contiguous runs (the [NW*P, Cc]
    # layout had 128-256 B dram runs, under the 512 B fast-DMA threshold)
    out = nc.dram_tensor("out", [P, NW * Cc], F16, kind="ExternalOutput")

    with tile.TileContext(nc) as tc:
        with (
            tc.tile_pool(name="const", bufs=1) as constp,
            tc.tile_pool(name="aldp", bufs=2) as aldp,
            tc.tile_pool(name="alg", bufs=3) as algp,
            tc.tile_pool(name="hs", bufs=3) as hsp,
            tc.tile_pool(name="za", bufs=3) as zap,
            tc.tile_pool(name="msg", bufs=3) as msgp,
            tc.tile_pool(name="epi", bufs=3) as epip,
            tc.tile_pool(name="og", bufs=2) as ogp,
            tc.tile_pool(name="psW", bufs=8, space="PSUM") as pswp,
        ):
            BSLOT = 512 // SLOT      # windows per PSUM bank
            ident_sb = constp.tile([P, P], F8)
            nc.scalar.dma_start(out=ident_sb[:], in_=ident[:])
            ebias_sb = constp.tile([P, 1], F32)
            nc.vector.memset(ebias_sb[:], EXP_BIAS)
            if bias_out:
                brep_sb = constp.tile([P, Cc], F32)
                nc.scalar.dma_start(out=brep_sb[:], in_=brep[:])

            pend = []

            def front(grp, ald_sb):
                """DMA + logit add + leaky-relu + exp for one group."""
                i0, nw, off0, sd = grp
                hs = hsp.tile([P, GS * Cc], F16, tag="hs")
                nc.sync.dma_start(out=hs[:, :sd * Cc],
                                  in_=hsrc[:, off0 * Cc:(off0 + sd) * Cc])
                AW = H * (2 if ald_exp else 1)
                alg = algp.tile([P, GS * AW], F16, tag="alg")
                nc.sync.dma_start(out=alg[:, :sd * AW],
                                  in_=als[:, off0 * AW:(off0 + sd) * AW])
                za = zap.tile([P, GS * H], F16, tag="za")
                if ald_exp:
                    # interleaved [al_src | al_dst] slots: one add per group
                    a0 = alg[:]
                    av = bass.AP(a0.tensor, a0.offset, [a0.ap[0], [2, sd]])
                    bv = bass.AP(a0.tensor, a0.offset + 1,
                                 [a0.ap[0], [2, sd]])
                    nc.vector.tensor_tensor(out=za[:, :sd],
                                            in0=av, in1=bv, op=OP.add)
                doff = 0
                for wl in range(nw) if not ald_exp else ():
                    D = int(D_list[i0 + wl])
                    o0 = doff * H
                    if H > 1:
                        av = alg[:, o0:o0 + D * H].rearrange(
                            "p (d h) -> p d h", d=D)
                        zv = za[:, o0:o0 + D * H].rearrange(
                            "p (d h) -> p d h", d=D)
                        ad = ald_sb[:, (i0 + wl) * H:(i0 + wl + 1) * H]
                        ab = bass.AP(ad.tensor, ad.offset,
                                     [ad.ap[0], [0, D], [1, H]])
                    else:
                        av = alg[:, o0:o0 + D]
                        zv = za[:, o0:o0 + D]
                        ad = ald_sb[:, i0 + wl:i0 + wl + 1]
                        ab = bass.AP(ad.tensor, ad.offset,
                                     [ad.ap[0], [0, D]])
                    nc.vector.tensor_tensor(out=zv, in0=av, in1=ab, op=OP.add)
                    doff += D
                nc.scalar.activation(za[:, :sd * H], za[:, :sd * H],
                                     AF.Prelu, alpha=NEG_SLOPE)
                msg = msgp.tile([P, GS * SLOT], F16, tag="msg")
                m3 = msg[:, :sd * SLOT].rearrange("p (d s) -> p d s", s=SLOT)
                eb_out = m3[:, :, Cc:Cc + EB]
                if H > 1:
                    e_in = za[:, :sd * H].rearrange("p (d h) -> p d h", d=sd)
                else:
                    z0 = za[:, :sd]
                    e_in = bass.AP(z0.tensor, z0.offset,
                                   [z0.ap[0], [1, sd], [0, EB]])
                nc.scalar.activation(eb_out, e_in, AF.Exp, bias=ebias_sb[:])
                return hs, msg

            def back(grp, st):
                """DVE message multiply + PE identity accumulation."""
                i0, nw, off0, sd = grp
                hs, msg = st
                m3 = msg[:, :sd * SLOT].rearrange("p (d s) -> p d s", s=SLOT)
                eb_out = m3[:, :, Cc:Cc + EB]
                mo = m3[:, :, 0:Cc].rearrange("p d (g h) -> p d g h", h=EB)
                hi = hs[:, :sd * Cc].rearrange(
                    "p (d g h) -> p d g h", d=sd, h=EB)
                ei = bass.AP(eb_out.tensor, eb_out.offset,
                             [eb_out.ap[0], eb_out.ap[1], [0, G], [1, EB]])
                nc.vector.tensor_tensor(out=mo, in0=hi, in1=ei, op=OP.mult)
                doff = 0
                bank = None
                for wl in range(nw):
                    D = int(D_list[i0 + wl])
                    if wl % BSLOT == 0:
                        bank = pswp.tile([P, 512], F32, tag="psw",
                                         name="pswbank")
                    sl = (wl % BSLOT) * SLOT
                    psw = bank[:, sl:sl + SLOT]
                    for j in range(D):
                        mv = msg[:, (doff + j) * SLOT:(doff + j + 1) * SLOT]
                        nc.tensor.matmul(psw, ident_sb[:], mv,
                                         start=(j == 0), stop=(j == D - 1))
                    pend.append(psw)
                    doff += D

            ogst = {}

            def epilogue(grp, flush):
                """One f16 PSUM copy per window, then a single reciprocal +
                scale per group; output DMAs batch two groups per write so
                HBM sees fewer read/write turnarounds against the streams."""
                i0, nw, off0, sd = grp
                op_t = epip.tile([P, NWmax * SLOT], F16, tag="o1p")
                for wl in range(nw):
                    psw = pend.pop(0)
                    nc.scalar.activation(op_t[:, wl * SLOT:(wl + 1) * SLOT],
                                         psw, AF.Copy)
                opv = op_t[:, :nw * SLOT]
                rec = epip.tile([P, NWmax * EB], F16, tag="rec")
                rv = rec[:, :nw * EB].rearrange("p (w h) -> p w h", w=nw)
                dap = bass.AP(opv.tensor, opv.offset + Cc,
                              [opv.ap[0], [SLOT, nw], [1, EB]])
                with nc.allow_low_precision(
                        reason="softmax denominators are O(1)"):
                    nc.vector.reciprocal(rv, dap)
                if not ogst:
                    ogst["og"] = ogp.tile([P, 2 * NWmax * Cc], F16,
                                          tag="og", name="ogpair")
                    ogst["i0"] = i0
                    ogst["fill"] = 0
                og = ogst["og"]
                ob = ogst["fill"]
                o_in = bass.AP(opv.tensor, opv.offset,
                               [opv.ap[0], [SLOT, nw], [EB, G], [1, EB]])
                r0 = rec[:]
                r_b = bass.AP(r0.tensor, r0.offset,
                              [r0.ap[0], [EB, nw], [0, G], [1, EB]])
                oo = og[:, ob:ob + nw * Cc].rearrange(
                    "p (w g h) -> p w g h", w=nw, h=EB)
                nc.vector.tensor_tensor(out=oo, in0=o_in, in1=r_b,
                                        op=OP.mult)
                if bias_out:     # layer bias: before the inter-layer elu
                    ov2 = og[:, ob:ob + nw * Cc].rearrange(
                        "p (w c) -> p w c", w=nw)
                    b0 = brep_sb[:]
                    b_b = bass.AP(b0.tensor, b0.offset,
                                  [b0.ap[0], [0, nw], [1, Cc]])
                    nc.vector.tensor_tensor(out=ov2, in0=ov2, in1=b_b,
                                            op=OP.add)
                if elu_out:
                    # elu(x) = max(x,0) + (min(exp(x),1) - 1), in place on og
                    ogv = og[:, ob:ob + nw * Cc]
                    et = epip.tile([P, NWmax * Cc], F16, tag="et")
                    etv = et[:, :nw * Cc]
                    nc.scalar.activation(etv, ogv, AF.Exp)
                    nc.vector.tensor_scalar(etv, etv, 1.0, -1.0,
                                            OP.min, OP.add)
                    nc.vector.scalar_tensor_tensor(ogv, ogv, 0.0, etv,
                                                   OP.max, OP.add)
                ogst["fill"] = ob + nw * Cc
                if flush:
                    f = ogst["fill"]
                    o0 = ogst["i0"] * Cc
                    nc.scalar.dma_start(out=out[:, o0:o0 + f],
                                        in_=og[:, :f])
                    ogst.clear()

            def body(_iv=None):
                if not ald_exp:
                    ald_sb = aldp.tile([P, NW * H], F16, tag="ald")
                    nc.scalar.dma_start(out=ald_sb[:], in_=ald[:])
                else:
                    ald_sb = None
                pend.clear()
                ogst.clear()
                ng = len(groups)
                ep = [0]

                def run_epi(gi):
                    epilogue(groups[gi],
                             flush=(ep[0] % 2 == 1) or (gi == ng - 1))
                    ep[0] += 1

                sts = [None] * ng
                for gi, grp in enumerate(groups):
                    sts[gi] = front(grp, ald_sb)
                    if gi >= 1:
                        back(groups[gi - 1], sts[gi - 1])
                        sts[gi - 1] = None
                    if gi >= 2:
                        run_epi(gi - 2)
                back(groups[ng - 1], sts[ng - 1])
                if ng >= 2:
                    run_epi(ng - 2)
                run_epi(ng - 1)

            if bench_loop > 1:
                with tc.For_i(0, bench_loop, 1) as _iv:
                    body(_iv)
            else:
                body()
    _finalize_kernel(nc)
    return nc


# ------------------------------------------------------------------ runner

def _fold_att(W, a):
    heads, hid = a.shape
    return np.einsum("ihc,hc->ih", W.reshape(W.shape[0], heads, hid), a)


class _GatRunner:
    def __init__(self, n_cores=N_CORES):
        self.C = n_cores
        self._graph = None
        self._graph_key = None
        self._kernels = {}
        self.last_maps = {}

    def graph(self, edge_index, n_nodes):
        key = hash(np.asarray(edge_index).tobytes())
        if key != self._graph_key:
            self._graph = _Graph(edge_index, n_nodes, self.C)
            self._graph_key = key
            self._kernels.clear()
        return self._graph

    def kernel(self, name, bench_loop=1, **kw):
        key = (name, bench_loop, tuple(sorted(kw.items())))
        if key not in self._kernels:
            g = self._graph
            if name.startswith("P"):
                self._kernels[key] = _build_node(
                    g.shard_nodes, bench_loop=bench_loop, **kw)
            elif name == "E1":
                self._kernels[key] = _build_edge_g(
                    g.D, g.groups1, g.TOT, 128, 8,
                    bench_loop=bench_loop, **kw)
            else:
                self._kernels[key] = _build_edge_g(
                    g.D, g.groups2, g.TOT, 64, 1, ald_exp=True,
                    bench_loop=bench_loop, **kw)
        return self._kernels[key]

    def _run(self, name, nc, maps):
        self.last_maps[name] = maps
        res = run_bass_kernel_spmd(nc, maps, core_ids=list(range(self.C)))
        return res.results

    def run(self, x, edge_index, W1, a_src1, a_dst1, b1, W2, a_src2, a_dst2,
            b2):
        C = self.C
        N, IN_C = x.shape
        HEADS, HID = a_src1.shape
        HC = HEADS * HID
        OUT_C = W2.shape[1]
        g = self.graph(edge_index, N)
        SH = g.shard_nodes
        # (c,h)-interleaved channel order for the layer-1 hidden features:
        # col c*H+h of h1 holds math channel h*HID+c. Folded into W1's
        # columns (P0) and W2's rows (P2) on the host - pure permutation.
        perm = np.array([(j % HEADS) * HID + j // HEADS
                         for j in range(HC)], dtype=np.int64)

        # ---- P0: per-node h1 / logits --------------------------------
        xT_pad = np.zeros((IN_C, g.n_pad), dtype=np.float16)
        xT_pad[:, :N] = np.asarray(x, np.float32).T
        w1 = np.asarray(W1, np.float32)
        m_al = 2 * HEADS
        wal1 = np.zeros((IN_C, 32), dtype=np.float32)
        wal1[:, :m_al] = np.concatenate(
            [_fold_att(w1, np.asarray(a_src1, np.float32)),
             _fold_att(w1, np.asarray(a_dst1, np.float32))], axis=1)
        mapsP0 = [{"xT": np.ascontiguousarray(xT_pad[:, k * SH:(k + 1) * SH]),
                   "w": np.ascontiguousarray(w1[:, perm]).astype(np.float16),
                   "wal": wal1.astype(np.float16)} for k in range(C)]
        ncP0 = self.kernel("P0", c_in=IN_C, m_h=HC, m_al=m_al,
                           elu=False, bias_in=False)
        resP0 = self._run("P0", ncP0, mapsP0)
        h1 = np.ascontiguousarray(
            np.concatenate([r["hT"] for r in resP0], axis=1).T)[:N]
        # unscramble the partition-stacked al panel: row 32k+r, col cq*CH+x
        # holds al[r] of chunk 4*cq+k
        nq = SH // (2 * CH)
        al1 = np.concatenate(
            [r["alT"].reshape(2, 32, nq, CH)[:, :m_al]
             .transpose(1, 2, 0, 3).reshape(m_al, SH)
             for r in resP0], axis=1)                    # [16, Np]
        als1 = np.ascontiguousarray(al1[:HEADS, :N].T)
        ald1 = np.ascontiguousarray(al1[HEADS:, :N].T)

        # ---- E1: layer-1 edge aggregation + bias + ELU ---------------
        id8 = g.ident8()
        b1nz = bool(np.any(np.asarray(b1)))
        mapsE1 = []
        for k in range(C):
            m = {"hsrc": g.stream_h(h1, k),
                 "als": g.stream_als(als1, k),
                 "ald": g.stream_ald(ald1, k),
                 "ident": id8}
            if b1nz:
                m["brep"] = np.tile(
                    np.asarray(b1, np.float32)[perm], (P, 1))
            mapsE1.append(m)
        ncE1 = self.kernel("E1", bias_out=b1nz)
        resE1 = self._run("E1", ncE1, mapsE1)
        out1 = np.concatenate(
            [r["out"].reshape(P, g.wpc, HC).transpose(1, 0, 2)
             .reshape(g.wpc * P, HC) for r in resE1], axis=0)
        # rows of out1 are (core, slot, row) -> natural node rowmap
        rowmap = g.rows_nodes.reshape(-1)            # [C*wpc*P]

        # ---- P2: ELU + per-node h2 / logits --------------------------
        o1T = np.ascontiguousarray(out1.T)           # [HC, C*SH] f16
        w2 = np.asarray(W2, np.float32)
        wal2 = np.concatenate(
            [_fold_att(w2, np.asarray(a_src2, np.float32)),
             _fold_att(w2, np.asarray(a_dst2, np.float32))], axis=1)
        w2all = np.concatenate([w2[perm], wal2[perm]], axis=1)  # [HC, 66]
        mapsP2 = [
            {"xT": np.ascontiguousarray(o1T[:, k * SH:(k + 1) * SH]),
             "w": w2all.astype(np.float16)} for k in range(C)]
        # out1 already carries b1 (E1 bias_out); P2 applies the ELU
        ncP2 = self.kernel("P2", c_in=HC, m_h=OUT_C, m_al=2, elu=True,
                           bias_in=False)
        resP2 = self._run("P2", ncP2, mapsP2)
        h2al = np.concatenate([r["hT"] for r in resP2], axis=1)  # [66, Np]
        valid = rowmap >= 0
        vrows = rowmap[valid]
        h2 = np.zeros((N, OUT_C), dtype=np.float16)
        h2[vrows] = h2al[:OUT_C].T[valid]
        als2 = np.zeros((N, 1), dtype=np.float16)
        als2[vrows, 0] = h2al[OUT_C][valid]
        ald2 = np.zeros((N, 1), dtype=np.float16)
        ald2[vrows, 0] = h2al[OUT_C + 1][valid]

        # ---- E2: layer-2 edge aggregation ----------------------------
        b2nz = bool(np.any(np.asarray(b2)))
        mapsE2 = []
        for k in range(C):
            a_s = g.stream_als(als2, k).reshape(P, g.TOT)
            a_d = g.stream_ald_exp(ald2, k).reshape(P, g.TOT)
            m = {"hsrc": g.stream_h(h2, k),
                 "als": np.ascontiguousarray(
                     np.stack([a_s, a_d], axis=2)).reshape(P, g.TOT * 2),
                 "ident": id8}
            if b2nz:
                m["brep"] = np.tile(np.asarray(b2, np.float32), (P, 1))
            mapsE2.append(m)
        ncE2 = self.kernel("E2", bias_out=b2nz)
        resE2 = self._run("E2", ncE2, mapsE2)
        out2 = np.concatenate(
            [r["out"].reshape(P, g.wpc, OUT_C).transpose(1, 0, 2)
             .reshape(g.wpc * P, OUT_C) for r in resE2], axis=0)
        out_full = np.zeros((N, OUT_C), dtype=np.float32)
        out_full[vrows] = out2[valid]
        return out_full


_RUNNER = _GatRunner()


def kernel(x, edge_index, W1, a_src1, a_dst1, b1, W2, a_src2, a_dst2, b2):
    """Full-input / full-output entry point. Returns [N, OUT_C] float32."""
    args = [np.asarray(v) for v in
            (x, edge_index, W1, a_src1, a_dst1, b1, W2, a_src2, a_dst2, b2)]
    return _RUNNER.run(*args).astype(np.float32)


# revision 59
# speedup vs baseline: 1.0773x; 1.0070x over previous
"""Trainium (trn2) Bass kernel for a 2-layer GAT over N=100k nodes / E=1.7M edges.

Strategy (degree-sorted edge grids + identity-stationary PE accumulation)
-------------------------------------------------------------------------
Nodes are sorted by in-degree on the host and packed into windows of 128
similar-degree destination nodes; windows are dealt round-robin across the 8
NeuronCores.  Each window's edges form a dense grid [128 nodes x D slots]
(D = max in-window degree, padded slots carry -inf logits so exp()==0), so
slot j of all 128 nodes is a 128-edge tile whose destination map is the
IDENTITY: the tensor engine accumulates the per-slot message tiles straight
into the window's PSUM bank with a never-changing fp8 identity stationary.
Degree sorting keeps grid padding at ~1.3%, and the one-hot selection stream
of the classic dst-sorted formulation (128 B/edge of pure index overhead)
disappears entirely.

Each GAT layer runs as TWO SPMD kernels with host-side index gathers (pure
permutations / casts - no host FLOPs) between them:

* node kernel (P0/P2): h = x @ W plus folded attention logits computed once
  per node (dense matmuls).  The full per-core input/output panels live in
  SBUF, loaded/stored with a handful of fat DMAs (per-chunk 1 KB/partition
  DMAs were latency-bound at ~140 GB/s); every DMA rides the SP queue since
  a queued DMA holds its issuing engine's sequencer for the whole transfer.
  P0's 16 logit rows stack two chunks per PSUM bank at partitions 0/32
  (tile_position) so one PSUM->SBUF copy drains two matmuls; P2 computes the
  inter-layer ELU as exp (one fat ACT op per quarter-panel, emitted a
  quarter ahead) + two 2x DVE ops, with PSUM copies balanced across ACT/DVE.
* edge kernel (E1/E2): streams h[src] grids (256/128 B per edge slot) and
  al_src logit grids (16/2 B); al_dst is a tiny per-window constant for E1
  and a host-replicated per-slot stream for E2 (one group-wide DVE add
  instead of 21 window-sized ones).  Windows are processed in groups
  (sum of D <= 96/192) software-pipelined three deep: group g's DMA +
  logits + leaky-relu + exp land while g-1 runs its DVE multiply + PE
  accumulation and g-2 runs its epilogue, so no engine ever stalls on
  another's latency.  ACT writes exp(z-4) into the message tile's trailing
  8 columns ((c,h)-interleaved broadcast for layer 1's 8 heads, an 8x
  replica for layer 2's single head so the DVE multiply keeps its
  packed-innermost 2x mode).  Epilogues drain each window's PSUM with a
  single f16 ACT copy, then one reciprocal + one scale per group, into a
  partition-major [128, NW*C] output panel (the row-major layout's 128-256 B
  dram runs fell under the 512 B threshold where DMA cost doubles; the host
  unscrambles for free).

Measured per-core DMA floor is ~343 GB/s on one queue / ~355 on two (HBM
fair share); the edge kernels stream ~62/~32 MB per core per inference and
run within ~15% of that floor.

Environment workarounds: this container's walrus build allows only ONE
semaphore wait per instruction (split onto nop carriers post-scheduling), and
the GPSIMD ucode libraries are absent (so no dma_gather/indirect-DMA fast
paths - hence the host-gather design).
"""
import numpy as np

import concourse.bass as bass
import concourse.mybir as mybir
import concourse.tile as tile
from concourse.bass_utils import run_bass_kernel_spmd

P = 128
F16 = mybir.dt.float16
F32 = mybir.dt.float32
F8 = mybir.dt.float8e4
AF = mybir.ActivationFunctionType
OP = mybir.AluOpType
NEG_SLOPE = 0.2
EXP_BIAS = -4.0     # exp(z + EXP_BIAS): constant shift cancels in softmax
NEG_INF = -60000.0  # pad-slot logit: exp(lrelu(.)+bias) underflows to 0
N_CORES = 8
EPS = 1e-30
CH = 448            # node-kernel matmul chunk (PSUM: 448*4B <= 2KB bank)
GCAP1, NWG1 = 90, 8      # E1 groups: count % 3 == 0 so the For_i seam's
                         # first DMA reuses a buffer freed 3 groups early
GCAP2, NWG2 = 208, 21    # E2 groups: ditto (9 groups, % 3 == 0)

# ------------------------------------------------------------------ patches

_wsplit_counter = [0]


def _split_excess_waits(nc, max_waits=1):
    """This walrus build rejects >1 sem-wait per instruction ("Too many sync
    wait commands"). Move overflow waits onto same-engine nop carriers."""
    n_split = 0
    for f in nc.m.functions:
        for blk in f.blocks:
            changed = False
            out = []
            for inst in blk.instructions:
                si = inst.sync_info
                if si is not None and len(si.on_wait) > max_waits:
                    waits = list(si.on_wait)
                    keep = waits[len(waits) - max_waits:]
                    overflow = waits[: len(waits) - max_waits]
                    for i in range(0, len(overflow), max_waits):
                        _wsplit_counter[0] += 1
                        nop = mybir.InstNoOp(
                            name=f"I-wsplit-{_wsplit_counter[0]}", ins=[], outs=[])
                        nop.engine = inst.engine
                        nop.sync_info = mybir.SyncInfo(
                            on_wait=overflow[i: i + max_waits], on_update=[])
                        out.append(nop)
                    inst.sync_info = mybir.SyncInfo(
                        on_wait=keep, on_update=list(si.on_update))
                    changed = True
                    n_split += 1
                out.append(inst)
            if changed:
                blk.instructions = out
    return n_split


def _finalize_kernel(nc):
    import bass_rust as _bass_rust
    from concourse.library_config import all_libraries, standard
    from concourse.library_overlay import lower_extended_insts

    inst_type_to_lib_mask = {}
    for lib in all_libraries:
        for inst_type in lib.instructions:
            inst_type_to_lib_mask[inst_type] = inst_type_to_lib_mask.get(
                inst_type, 0) | (1 << lib.index)
    _bass_rust.insert_library_loads(
        nc, inst_type_to_lib_mask, len(all_libraries), standard.index)
    lower_extended_insts(nc)
    _split_excess_waits(nc)


# ------------------------------------------------------------------ host prep

class _Graph:
    """Degree-sorted grid preprocessing: sort nodes by in-degree, pack 128
    similar-degree nodes per window, deal windows round-robin across cores
    (slot i of every core shares one padded depth D_i so all cores run one
    identical SPMD program), and scatter each node's edges into its grid row.
    """

    def __init__(self, edge_index, n_nodes, n_cores):
        self.N = n_nodes
        self.C = n_cores
        src = np.asarray(edge_index[0], dtype=np.int64)
        dst = np.asarray(edge_index[1], dtype=np.int64)
        E = src.shape[0]

        deg = np.bincount(dst, minlength=n_nodes)
        order = np.argsort(deg, kind="stable")

        n_win_total = (n_nodes + P - 1) // P
        self.wpc = (n_win_total + n_cores - 1) // n_cores
        n_win = self.wpc * n_cores
        self.n_pad = n_win * P
        self.shard_nodes = self.wpc * P
        n_dummy = self.n_pad - n_nodes

        snode = np.full(self.n_pad, -1, dtype=np.int64)
        snode[n_dummy:] = order                      # ascending degree
        # rows_nodes[k][i, e] = natural node id at (core k, slot i, row e)
        self.rows_nodes = np.ascontiguousarray(
            snode.reshape(self.wpc, n_cores, P).transpose(1, 0, 2))

        wdeg = np.where(snode >= 0, deg[np.clip(snode, 0, None)], 0)
        wmax = wdeg.reshape(self.wpc, n_cores, P).max(axis=2)   # [wpc, cores]
        self.D = np.maximum(wmax.max(axis=1), 1).astype(np.int64)  # [wpc]
        self.off = np.concatenate([[0], np.cumsum(self.D)])
        self.TOT = int(self.D.sum())

        # position of each node in the sorted layout
        posq = np.empty(n_nodes, dtype=np.int64)
        posq[order] = np.arange(n_nodes) + n_dummy

        # scatter edges (dst-sorted, ranked within dst run) into grids
        perm = np.argsort(dst, kind="stable")
        src_s = src[perm]
        dst_s = dst[perm]
        bounds = np.searchsorted(dst_s, np.arange(n_nodes + 1))
        j_e = np.arange(E) - bounds[dst_s]           # rank within dst run
        q_e = posq[dst_s]
        g_e = q_e // P
        row_e = q_e % P
        core_e = g_e % n_cores
        slot_e = g_e // n_cores
        flat_e = self.off[slot_e] + j_e              # grid slot within [TOT]
        self.gidx = np.zeros((n_cores, self.TOT, P), dtype=np.int32)
        self.gidx[core_e, flat_e, row_e] = (src_s + 1).astype(np.int32)

        self.groups1 = self.make_groups(GCAP1, NWG1)
        self.groups2 = self.make_groups(GCAP2, NWG2)
        self.D_key = tuple(int(d) for d in self.D)

    def make_groups(self, gcap, nwg):
        """Window groups: sum(D) <= gcap, <= nwg windows per group."""
        groups = []
        i = 0
        while i < self.wpc:
            i0, sd, nw = i, 0, 0
            while (i < self.wpc and nw < nwg
                   and (nw == 0 or sd + int(self.D[i]) <= gcap)):
                sd += int(self.D[i])
                i += 1
                nw += 1
            groups.append((i0, nw, int(self.off[i0]), sd))
        return groups

    def stream_h(self, table, core):
        """[128, TOT*C] f16 grid gather: table rows by gidx (0 = zero pad)."""
        C = table.shape[1]
        tp = np.zeros((self.N + 1, C), dtype=np.float16)
        tp[1:] = table
        arr = tp[self.gidx[core]]                    # [TOT, P, C]
        return np.ascontiguousarray(arr.transpose(1, 0, 2)).reshape(
            P, self.TOT * C)

    def stream_als(self, table, core):
        """[128, TOT*H] f16: al_src grid; pad slots -> NEG_INF so exp()==0.
        Dummy rows get one j=0 slot with logit 0 so their softmax denominator
        stays finite (their h rows are zero, so the output row is 0)."""
        H = table.shape[1]
        tp = np.full((self.N + 1, H), NEG_INF, dtype=np.float16)
        tp[1:] = table
        arr = tp[self.gidx[core]]                    # [TOT, P, H]
        i_d, e_d = np.nonzero(self.rows_nodes[core] < 0)
        arr[self.off[i_d], e_d, :] = 0.0
        return np.ascontiguousarray(arr.transpose(1, 0, 2)).reshape(
            P, self.TOT * H)

    def stream_ald(self, table, core):
        """[128, wpc*H] f16: al_dst per (window, row). Dummy rows -> 0."""
        H = table.shape[1]
        tp = np.zeros((self.N + 1, H), dtype=np.float16)
        tp[1:] = table
        arr = tp[self.rows_nodes[core] + 1]          # [wpc, P, H]
        return np.ascontiguousarray(arr.transpose(1, 0, 2)).reshape(
            P, self.wpc * H)

    def stream_ald_exp(self, table, core):
        """[128, TOT*H] f16: al_dst replicated across each window's slots
        (slot grids are per-window blocks of D_i slots)."""
        H = table.shape[1]
        tp = np.zeros((self.N + 1, H), dtype=np.float16)
        tp[1:] = table
        arr = tp[self.rows_nodes[core] + 1]          # [wpc, P, H]
        rep = np.repeat(arr, self.D, axis=0)         # [TOT, P, H]
        return np.ascontiguousarray(rep.transpose(1, 0, 2)).reshape(
            P, self.TOT * H)

    def ident8(self):
        import ml_dtypes
        return np.eye(P, dtype=np.float32).astype(ml_dtypes.float8_e4m3)


# ------------------------------------------------------------------ builders

def _build_node(SH, c_in, m_h, m_al, elu, bias_in, bench_loop=1):
    """Per-node transform: hT = (elu?(xT+b)) @ w, alT = same @ wal.
    When m_h+m_al <= 128 the two matmuls merge into one.  The whole per-core
    panel is SBUF-resident: quarters stream in with fat DMAs, chunked matmuls
    write a staged output panel, and a few fat DMAs store it."""
    merged = (m_h + m_al) <= P
    M = m_h + m_al if merged else m_h
    QN = 4
    QS = SH // QN
    NQUAD = SH // (2 * CH)        # 2 al-chunks stack into one PSUM bank
    assert SH % QN == 0 and QS % CH == 0 and SH % (2 * CH) == 0
    nc = bass.Bass()
    xT = nc.dram_tensor("xT", [c_in, SH], F16, kind="ExternalInput")
    w = nc.dram_tensor("w", [c_in, M], F16, kind="ExternalInput")
    if not merged:
        assert m_al <= 32
        wal = nc.dram_tensor("wal", [c_in, 32], F16, kind="ExternalInput")
    if bias_in:
        bvec = nc.dram_tensor("bvec", [c_in, 1], F32, kind="ExternalInput")
    hT = nc.dram_tensor("hT", [M, SH], F16, kind="ExternalOutput")
    if not merged:
        # partition-stacked al panel: row 32k+r, col cq*CH+x holds
        # al[r] of chunk 2*cq+k (host unscrambles)
        alT = nc.dram_tensor("alT", [64, NQUAD * CH], F16,
                             kind="ExternalOutput")

    with tile.TileContext(nc) as tc:
        with (
            tc.tile_pool(name="const", bufs=1) as constp,
            tc.tile_pool(name="xin", bufs=2) as xinp,
            tc.tile_pool(name="hout", bufs=2) as houtp,
            tc.tile_pool(name="work", bufs=4) as workp,
            tc.tile_pool(name="psH", bufs=5, space="PSUM") as psH,
            tc.tile_pool(name="psA", bufs=3, space="PSUM") as psA,
        ):
            w_sb = constp.tile([c_in, M], F16)
            nc.sync.dma_start(out=w_sb[:], in_=w[:])
            if not merged:
                # wal host-padded to 32 cols (zeros) so every partition of
                # the stacked al PSUM region is written (no uninit reads)
                wal_sb = constp.tile([c_in, 32], F16)
                nc.sync.dma_start(out=wal_sb[:], in_=wal[:])
            if bias_in:
                b_sb = constp.tile([c_in, 1], F32)
                nc.sync.dma_start(out=b_sb[:], in_=bvec[:])

            def body(_iv=None):
                # every DMA rides SP: a queued DMA holds its issuing engine's
                # sequencer for the whole transfer, so ACT/DVE must stay clean
                xq = [xinp.tile([c_in, QS], F16, tag=f"x{q}", name=f"xq{q}")
                      for q in range(QN)]
                for q in range(QN):
                    nc.sync.dma_start(out=xq[q][:],
                                      in_=xT[:, q * QS:(q + 1) * QS])
                hq = [houtp.tile([M, QS], F16, tag=f"h{q}", name=f"hq{q}")
                      for q in range(QN)]
                if not merged:
                    alout = houtp.tile([64, NQUAD * CH], F16, tag="alo")
                quad = {}

                def qfront(q):
                    """Quarter-granular ELU stage A: one fat ACT exp."""
                    if not elu:
                        return None
                    rhs = xq[q][:]
                    if bias_in:
                        nc.vector.tensor_scalar(
                            rhs, rhs, b_sb[:, 0:1], None, OP.add)
                    et = workp.tile([c_in, QS], F16, tag="et")
                    nc.scalar.activation(et[:], rhs, AF.Exp)
                    return et

                def qback(q, et):
                    if elu:
                        # elu(x) = (min(exp(x),1) - 1) + max(x,0), all 2x DVE
                        mn = workp.tile([c_in, QS], F16, tag="mn")
                        nc.vector.tensor_scalar(
                            mn[:], et[:], 1.0, -1.0, OP.min, OP.add)
                        mx = workp.tile([c_in, QS], F16, tag="mx")
                        nc.vector.tensor_scalar(
                            mx[:], xq[q][:], 0.0, None, OP.max)
                        xe = workp.tile([c_in, QS], F16, tag="xe")
                        nc.vector.tensor_tensor(
                            out=xe[:], in0=mn[:], in1=mx[:], op=OP.add)
                        src = xe
                    else:
                        src = xq[q]
                    for j in range(QS // CH):
                        ci = q * (QS // CH) + j
                        qo = j * CH
                        rhs = src[:, qo:qo + CH]
                        ph = psH.tile([M, CH], F32, tag="ph")
                        nc.tensor.matmul(ph[:], w_sb[:], rhs,
                                         start=True, stop=True)
                        dve_copy = (ci % 7 < 3) if elu else (ci % 2 == 1)
                        if dve_copy:
                            nc.vector.tensor_copy(hq[q][:, qo:qo + CH],
                                                  ph[:])
                        else:
                            nc.scalar.activation(hq[q][:, qo:qo + CH],
                                                 ph[:], AF.Copy)
                        if not merged:
                            # stack 2 chunks' al outputs on partitions
                            # 0/32 of one PSUM bank -> 1 copy per pair
                            k = ci % 2
                            if k == 0:
                                quad["pa"] = psA.tile([64, CH], F32,
                                                      tag="paq", name="paq")
                            pa = quad["pa"]
                            nc.tensor.matmul(pa[32 * k:32 * k + 32, :],
                                             wal_sb[:], rhs,
                                             start=True, stop=True)
                            if k == 1:
                                cq = ci // 2
                                if cq % 2 == 0:
                                    nc.vector.tensor_copy(
                                        alout[:, cq * CH:(cq + 1) * CH],
                                        pa[:])
                                else:
                                    nc.scalar.activation(
                                        alout[:, cq * CH:(cq + 1) * CH],
                                        pa[:], AF.Copy)
                    nc.sync.dma_start(out=hT[:, q * QS:(q + 1) * QS],
                                      in_=hq[q][:])

                prev = None
                for q in range(QN):
                    et = qfront(q)
                    if prev is not None:
                        qback(*prev)
                    prev = (q, et)
                qback(*prev)
                if not merged:
                    nc.sync.dma_start(out=alT[:], in_=alout[:])

            if bench_loop > 1:
                with tc.For_i(0, bench_loop, 1) as _iv:
                    body(_iv)
            else:
                body()
    _finalize_kernel(nc)
    return nc


def _build_edge_g(D_list, groups, TOT, Cc, H, bias_out=False, elu_out=False,
                  ald_exp=False, bench_loop=1):
    """Edge aggregation over degree-sorted grids.  Per group of windows:
    one h[src] grid DMA, one DVE logit add per window, one ACT leaky-relu,
    one ACT exp into the message tile's trailing EB columns, one DVE
    multiply, then D accumulating identity matmuls per window.  Epilogues
    run one group late so no engine stalls on PSUM completion."""
    EB = 8 if H > 1 else 4   # exp block: 8 heads, or 4 replicas (1 head)
    SLOT = Cc + EB
    G = Cc // EB
    NW = len(D_list)
    GS = max(sd for _, _, _, sd in groups)
    NWmax = max(nw for _, nw, _, _ in groups)

    nc = bass.Bass()
    hsrc = nc.dram_tensor("hsrc", [P, TOT * Cc], F16, kind="ExternalInput")
    # ald_exp: als carries [al_src | al_dst] interleaved per slot (doubles
    # the per-partition dram run length past the 512 B fast-DMA threshold)
    als = nc.dram_tensor("als", [P, TOT * H * (2 if ald_exp else 1)], F16,
                         kind="ExternalInput")
    if not ald_exp:
        ald = nc.dram_tensor("ald", [P, NW * H], F16, kind="ExternalInput")
    ident = nc.dram_tensor("ident", [P, P], F8, kind="ExternalInput")
    if bias_out:
        brep = nc.dram_tensor("brep", [P, Cc], F32, kind="ExternalInput")
    # partition-major output: per-partition contiguous runs (the [NW*P, Cc]
    # layout had 128-256 B dram runs, under the 512 B fast-DMA threshold)
    out = nc.dram_tensor("out", [P, NW * Cc], F16, kind="ExternalOutput")

    with tile.TileContext(nc) as tc:
        with (
            tc.tile_pool(name="const", bufs=1) as constp,
            tc.tile_pool(name="aldp", bufs=2) as aldp,
            tc.tile_pool(name="alg", bufs=3) as algp,
            tc.tile_pool(name="hs", bufs=3) as hsp,
            tc.tile_pool(name="za", bufs=3) as zap,
            tc.tile_pool(name="msg", bufs=3) as msgp,
            tc.tile_pool(name="epi", bufs=3) as epip,
            tc.tile_pool(name="og", bufs=2) as ogp,
            tc.tile_pool(name="psW", bufs=8, space="PSUM") as pswp,
        ):
            BSLOT = 512 // SLOT      # windows per PSUM bank
            ident_sb = constp.tile([P, P], F8)
            nc.scalar.dma_start(out=ident_sb[:], in_=ident[:])
            ebias_sb = constp.tile([P, 1], F32)
            nc.vector.memset(ebias_sb[:], EXP_BIAS)
            if bias_out:
                brep_sb = constp.tile([P, Cc], F32)
                nc.scalar.dma_start(out=brep_sb[:], in_=brep[:])

            pend = []

            def front(grp, ald_sb):
                """DMA + logit add + leaky-relu + exp for one group."""
                i0, nw, off0, sd = grp
                hs = hsp.tile([P, GS * Cc], F16, tag="hs")
                nc.sync.dma_start(out=hs[:, :sd * Cc],
                                  in_=hsrc[:, off0 * Cc:(off0 + sd) * Cc])
                AW = H * (2 if ald_exp else 1)
                alg = algp.tile([P, GS * AW], F16, tag="alg")
                nc.sync.dma_start(out=alg[:, :sd * AW],
                                  in_=als[:, off0 * AW:(off0 + sd) * AW])
                za = zap.tile([P, GS * H], F16, tag="za")
                if ald_exp:
                    # interleaved [al_src | al_dst] slots: one add per group
                    a0 = alg[:]
                    av = bass.AP(a0.tensor, a0.offset, [a0.ap[0], [2, sd]])
                    bv = bass.AP(a0.tensor, a0.offset + 1,
                                 [a0.ap[0], [2, sd]])
                    nc.vector.tensor_tensor(out=za[:, :sd],
                                            in0=av, in1=bv, op=OP.add)
                doff = 0
                for wl in range(nw) if not ald_exp else ():
                    D = int(D_list[i0 + wl])
                    o0 = doff * H
                    if H > 1:
                        av = alg[:, o0:o0 + D * H].rearrange(
                            "p (d h) -> p d h", d=D)
                        zv = za[:, o0:o0 + D * H].rearrange(
                            "p (d h) -> p d h", d=D)
                        ad = ald_sb[:, (i0 + wl) * H:(i0 + wl + 1) * H]
                        ab = bass.AP(ad.tensor, ad.offset,
                                     [ad.ap[0], [0, D], [1, H]])
                    else:
                        av = alg[:, o0:o0 + D]
                        zv = za[:, o0:o0 + D]
                        ad = ald_sb[:, i0 + wl:i0 + wl + 1]
                        ab = bass.AP(ad.tensor, ad.offset,
                                     [ad.ap[0], [0, D]])
                    nc.vector.tensor_tensor(out=zv, in0=av, in1=ab, op=OP.add)
                    doff += D
                nc.scalar.activation(za[:, :sd * H], za[:, :sd * H],
                                     AF.Prelu, alpha=NEG_SLOPE)
                msg = msgp.tile([P, GS * SLOT], F16, tag="msg")
                m3 = msg[:, :sd * SLOT].rearrange("p (d s) -> p d s", s=SLOT)
                eb_out = m3[:, :, Cc:Cc + EB]
                if H > 1:
                    e_in = za[:, :sd * H].rearrange("p (d h) -> p d h", d=sd)
                else:
                    z0 = za[:, :sd]
                    e_in = bass.AP(z0.tensor, z0.offset,
                                   [z0.ap[0], [1, sd], [0, EB]])
                nc.scalar.activation(eb_out, e_in, AF.Exp, bias=ebias_sb[:])
                return hs, msg

            def back(grp, st):
                """DVE message multiply + PE identity accumulation."""
                i0, nw, off0, sd = grp
                hs, msg = st
                m3 = msg[:, :sd * SLOT].rearrange("p (d s) -> p d s", s=SLOT)
                eb_out = m3[:, :, Cc:Cc + EB]
                mo = m3[:, :, 0:Cc].rearrange("p d (g h) -> p d g h", h=EB)
                hi = hs[:, :sd * Cc].rearrange(
                    "p (d g h) -> p d g h", d=sd, h=EB)
                ei = bass.AP(eb_out.tensor, eb_out.offset,
                             [eb_out.ap[0], eb_out.ap[1], [0, G], [1, EB]])
                nc.vector.tensor_tensor(out=mo, in0=hi, in1=ei, op=OP.mult)
                doff = 0
                bank = None
                for wl in range(nw):
                    D = int(D_list[i0 + wl])
                    if wl % BSLOT == 0:
                        bank = pswp.tile([P, 512], F32, tag="psw",
                                         name="pswbank")
                    sl = (wl % BSLOT) * SLOT
                    psw = bank[:, sl:sl + SLOT]
                    for j in range(D):
                        mv = msg[:, (doff + j) * SLOT:(doff + j + 1) * SLOT]
                        nc.tensor.matmul(psw, ident_sb[:], mv,
                                         start=(j == 0), stop=(j == D - 1))
                    pend.append(psw)
                    doff += D

            ogst = {}

            def epilogue(grp, flush):
                """One f16 PSUM copy per window, then a single reciprocal +
                scale per group; output DMAs batch two groups per write so
                HBM sees fewer read/write turnarounds against the streams."""
                i0, nw, off0, sd = grp
                op_t = epip.tile([P, NWmax * SLOT], F16, tag="o1p")
                for wl in range(nw):
                    psw = pend.pop(0)
                    nc.scalar.activation(op_t[:, wl * SLOT:(wl + 1) * SLOT],
                                         psw, AF.Copy)
                opv = op_t[:, :nw * SLOT]
                rec = epip.tile([P, NWmax * EB], F16, tag="rec")
                rv = rec[:, :nw * EB].rearrange("p (w h) -> p w h", w=nw)
                dap = bass.AP(opv.tensor, opv.offset + Cc,
                              [opv.ap[0], [SLOT, nw], [1, EB]])
                with nc.allow_low_precision(
                        reason="softmax denominators are O(1)"):
                    nc.vector.reciprocal(rv, dap)
                if not ogst:
                    ogst["og"] = ogp.tile([P, 2 * NWmax * Cc], F16,
                                          tag="og", name="ogpair")
                    ogst["i0"] = i0
                    ogst["fill"] = 0
                og = ogst["og"]
                ob = ogst["fill"]
                o_in = bass.AP(opv.tensor, opv.offset,
                               [opv.ap[0], [SLOT, nw], [EB, G], [1, EB]])
                r0 = rec[:]
                r_b = bass.AP(r0.tensor, r0.offset,
                              [r0.ap[0], [EB, nw], [0, G], [1, EB]])
                oo = og[:, ob:ob + nw * Cc].rearrange(
                    "p (w g h) -> p w g h", w=nw, h=EB)
                nc.vector.tensor_tensor(out=oo, in0=o_in, in1=r_b,
                                        op=OP.mult)
                if bias_out:     # layer bias: before the inter-layer elu
                    ov2 = og[:, ob:ob + nw * Cc].rearrange(
                        "p (w c) -> p w c", w=nw)
                    b0 = brep_sb[:]
                    b_b = bass.AP(b0.tensor, b0.offset,
                                  [b0.ap[0], [0, nw], [1, Cc]])
                    nc.vector.tensor_tensor(out=ov2, in0=ov2, in1=b_b,
                                            op=OP.add)
                if elu_out:
                    # elu(x) = max(x,0) + (min(exp(x),1) - 1), in place on og
                    ogv = og[:, ob:ob + nw * Cc]
                    et = epip.tile([P, NWmax * Cc], F16, tag="et")
                    etv = et[:, :nw * Cc]
                    nc.scalar.activation(etv, ogv, AF.Exp)
                    nc.vector.tensor_scalar(etv, etv, 1.0, -1.0,
                                            OP.min, OP.add)
                    nc.vector.scalar_tensor_tensor(ogv, ogv, 0.0, etv,
                                                   OP.max, OP.add)
                ogst["fill"] = ob + nw * Cc
                if flush:
                    f = ogst["fill"]
                    o0 = ogst["i0"] * Cc
                    nc.scalar.dma_start(out=out[:, o0:o0 + f],
                                        in_=og[:, :f])
                    ogst.clear()

            def body(_iv=None):
                if not ald_exp:
                    ald_sb = aldp.tile([P, NW * H], F16, tag="ald")
                    nc.scalar.dma_start(out=ald_sb[:], in_=ald[:])
                else:
                    ald_sb = None
                pend.clear()
                ogst.clear()
                ng = len(groups)
                ep = [0]

                def run_epi(gi):
                    epilogue(groups[gi],
                             flush=(ep[0] % 2 == 1) or (gi == ng - 1))
                    ep[0] += 1

                sts = [None] * ng
                for gi, grp in enumerate(groups):
                    sts[gi] = front(grp, ald_sb)
                    if gi >= 1:
                        back(groups[gi - 1], sts[gi - 1])
                        sts[gi - 1] = None
                    if gi >= 2:
                        run_epi(gi - 2)
                back(groups[ng - 1], sts[ng - 1])
                if ng >= 2:
                    run_epi(ng - 2)
                run_epi(ng - 1)

            if bench_loop > 1:
                with tc.For_i(0, bench_loop, 1) as _iv:
                    body(_iv)
            else:
                body()
    _finalize_kernel(nc)
    return nc


# ------------------------------------------------------------------ runner

def _fold_att(W, a):
    heads, hid = a.shape
    return np.einsum("ihc,hc->ih", W.reshape(W.shape[0], heads, hid), a)


class _GatRunner:
    def __init__(self, n_cores=N_CORES):
        self.C = n_cores
        self._graph = None
        self._graph_key = None
        self._kernels = {}
        self.last_maps = {}

    def graph(self, edge_index, n_nodes):
        key = hash(np.asarray(edge_index).tobytes())
        if key != self._graph_key:
            self._graph = _Graph(edge_index, n_nodes, self.C)
            self._graph_key = key
            self._kernels.clear()
        return self._graph

    def kernel(self, name, bench_loop=1, **kw):
        key = (name, bench_loop, tuple(sorted(kw.items())))
        if key not in self._kernels:
            g = self._graph
            if name.startswith("P"):
                self._kernels[key] = _build_node(
                    g.shard_nodes, bench_loop=bench_loop, **kw)
            elif name == "E1":
                self._kernels[key] = _build_edge_g(
                    g.D, g.groups1, g.TOT, 128, 8,
                    bench_loop=bench_loop, **kw)
            else:
                self._kernels[key] = _build_edge_g(
                    g.D, g.groups2, g.TOT, 64, 1, ald_exp=True,
                    bench_loop=bench_loop, **kw)
        return self._kernels[key]

    def _run(self, name, nc, maps):
        self.last_maps[name] = maps
        res = run_bass_kernel_spmd(nc, maps, core_ids=list(range(self.C)))
        return res.results

    def run(self, x, edge_index, W1, a_src1, a_dst1, b1, W2, a_src2, a_dst2,
            b2):
        C = self.C
        N, IN_C = x.shape
        HEADS, HID = a_src1.shape
        HC = HEADS * HID
        OUT_C = W2.shape[1]
        g = self.graph(edge_index, N)
        SH = g.shard_nodes
        # (c,h)-interleaved channel order for the layer-1 hidden features:
        # col c*H+h of h1 holds math channel h*HID+c. Folded into W1's
        # columns (P0) and W2's rows (P2) on the host - pure permutation.
        perm = np.array([(j % HEADS) * HID + j // HEADS
                         for j in range(HC)], dtype=np.int64)

        # ---- P0: per-node h1 / logits --------------------------------
        xT_pad = np.zeros((IN_C, g.n_pad), dtype=np.float16)
        xT_pad[:, :N] = np.asarray(x, np.float32).T
        w1 = np.asarray(W1, np.float32)
        m_al = 2 * HEADS
        wal1 = np.zeros((IN_C, 32), dtype=np.float32)
        wal1[:, :m_al] = np.concatenate(
            [_fold_att(w1, np.asarray(a_src1, np.float32)),
             _fold_att(w1, np.asarray(a_dst1, np.float32))], axis=1)
        mapsP0 = [{"xT": np.ascontiguousarray(xT_pad[:, k * SH:(k + 1) * SH]),
                   "w": np.ascontiguousarray(w1[:, perm]).astype(np.float16),
                   "wal": wal1.astype(np.float16)} for k in range(C)]
        ncP0 = self.kernel("P0", c_in=IN_C, m_h=HC, m_al=m_al,
                           elu=False, bias_in=False)
        resP0 = self._run("P0", ncP0, mapsP0)
        h1 = np.ascontiguousarray(
            np.concatenate([r["hT"] for r in resP0], axis=1).T)[:N]
        # unscramble the partition-stacked al panel: row 32k+r, col cq*CH+x
        # holds al[r] of chunk 4*cq+k
        nq = SH // (2 * CH)
        al1 = np.concatenate(
            [r["alT"].reshape(2, 32, nq, CH)[:, :m_al]
             .transpose(1, 2, 0, 3).reshape(m_al, SH)
             for r in resP0], axis=1)                    # [16, Np]
        als1 = np.ascontiguousarray(al1[:HEADS, :N].T)
        ald1 = np.ascontiguousarray(al1[HEADS:, :N].T)

        # ---- E1: layer-1 edge aggregation + bias + ELU ---------------
        id8 = g.ident8()
        b1nz = bool(np.any(np.asarray(b1)))
        mapsE1 = []
        for k in range(C):
            m = {"hsrc": g.stream_h(h1, k),
                 "als": g.stream_als(als1, k),
                 "ald": g.stream_ald(ald1, k),
                 "ident": id8}
            if b1nz:
                m["brep"] = np.tile(
                    np.asarray(b1, np.float32)[perm], (P, 1))
            mapsE1.append(m)
        ncE1 = self.kernel("E1", bias_out=b1nz)
        resE1 = self._run("E1", ncE1, mapsE1)
        out1 = np.concatenate(
            [r["out"].reshape(P, g.wpc, HC).transpose(1, 0, 2)
             .reshape(g.wpc * P, HC) for r in resE1], axis=0)
        # rows of out1 are (core, slot, row) -> natural node rowmap
        rowmap = g.rows_nodes.reshape(-1)            # [C*wpc*P]

        # ---- P2: ELU + per-node h2 / logits --------------------------
        o1T = np.ascontiguousarray(out1.T)           # [HC, C*SH] f16
        w2 = np.asarray(W2, np.float32)
        wal2 = np.concatenate(
            [_fold_att(w2, np.asarray(a_src2, np.float32)),
             _fold_att(w2, np.asarray(a_dst2, np.float32))], axis=1)
        w2all = np.concatenate([w2[perm], wal2[perm]], axis=1)  # [HC, 66]
        mapsP2 = [
            {"xT": np.ascontiguousarray(o1T[:, k * SH:(k + 1) * SH]),
             "w": w2all.astype(np.float16)} for k in range(C)]
        # out1 already carries b1 (E1 bias_out); P2 applies the ELU
        ncP2 = self.kernel("P2", c_in=HC, m_h=OUT_C, m_al=2, elu=True,
                           bias_in=False)
        resP2 = self._run("P2", ncP2, mapsP2)
        h2al = np.concatenate([r["hT"] for r in resP2], axis=1)  # [66, Np]
        valid = rowmap >= 0
        vrows = rowmap[valid]
        h2 = np.zeros((N, OUT_C), dtype=np.float16)
        h2[vrows] = h2al[:OUT_C].T[valid]
        als2 = np.zeros((N, 1), dtype=np.float16)
        als2[vrows, 0] = h2al[OUT_C][valid]
        ald2 = np.zeros((N, 1), dtype=np.float16)
        ald2[vrows, 0] = h2al[OUT_C + 1][valid]

        # ---- E2: layer-2 edge aggregation ----------------------------
        b2nz = bool(np.any(np.asarray(b2)))
        mapsE2 = []
        for k in range(C):
            a_s = g.stream_als(als2, k).reshape(P, g.TOT)
            a_d = g.stream_ald_exp(ald2, k).reshape(P, g.TOT)
            m = {"hsrc": g.stream_h(h2, k),
                 "als": np.ascontiguousarray(
                     np.stack([a_s, a_d], axis=2)).reshape(P, g.TOT * 2),
                 "ident": id8}
            if b2nz:
                m["brep"] = np.tile(np.asarray(b2, np.float32), (P, 1))
            mapsE2.append(m)
        ncE2 = self.kernel("E2", bias_out=b2nz)
        resE2 = self._run("E2", ncE2, mapsE2)
        out2 = np.concatenate(
            [r["out"].reshape(P, g.wpc, OUT_C).transpose(1, 0, 2)
             .reshape(g.wpc * P, OUT_C) for r in resE2], axis=0)
        out_full = np.zeros((N, OUT_C), dtype=np.float32)
        out_full[vrows] = out2[valid]
        return out_full


_RUNNER = _GatRunner()


def kernel(x, edge_index, W1, a_src1, a_dst1, b1, W2, a_src2, a_dst2, b2):
    """Full-input / full-output entry point. Returns [N, OUT_C] float32."""
    args = [np.asarray(v) for v in
            (x, edge_index, W1, a_src1, a_dst1, b1, W2, a_src2, a_dst2, b2)]
    return _RUNNER.run(*args).astype(np.float32)


# revision 60
# speedup vs baseline: 1.0814x; 1.0038x over previous
"""Trainium (trn2) Bass kernel for a 2-layer GAT over N=100k nodes / E=1.7M edges.

Strategy (degree-sorted edge grids + identity-stationary PE accumulation)
-------------------------------------------------------------------------
Nodes are sorted by in-degree on the host and packed into windows of 128
similar-degree destination nodes; windows are dealt round-robin across the 8
NeuronCores.  Each window's edges form a dense grid [128 nodes x D slots]
(D = max in-window degree, padded slots carry -inf logits so exp()==0), so
slot j of all 128 nodes is a 128-edge tile whose destination map is the
IDENTITY: the tensor engine accumulates the per-slot message tiles straight
into the window's PSUM bank with a never-changing fp8 identity stationary.
Degree sorting keeps grid padding at ~1.3%, and the one-hot selection stream
of the classic dst-sorted formulation (128 B/edge of pure index overhead)
disappears entirely.

Each GAT layer runs as TWO SPMD kernels with host-side index gathers (pure
permutations / casts - no host FLOPs) between them:

* node kernel (P0/P2): h = x @ W plus folded attention logits computed once
  per node (dense matmuls).  The full per-core input/output panels live in
  SBUF, loaded/stored with a handful of fat DMAs (per-chunk 1 KB/partition
  DMAs were latency-bound at ~140 GB/s); every DMA rides the SP queue since
  a queued DMA holds its issuing engine's sequencer for the whole transfer.
  P0's 16 logit rows stack two chunks per PSUM bank at partitions 0/32
  (tile_position) so one PSUM->SBUF copy drains two matmuls; P2 computes the
  inter-layer ELU as exp (one fat ACT op per quarter-panel, emitted a
  quarter ahead) + two 2x DVE ops, with PSUM copies balanced across ACT/DVE.
* edge kernel (E1/E2): streams h[src] grids (256/128 B per edge slot) and
  al_src logit grids (16/2 B); al_dst is a tiny per-window constant for E1
  and a host-replicated per-slot stream for E2 (one group-wide DVE add
  instead of 21 window-sized ones).  Windows are processed in groups
  (sum of D <= 96/192) software-pipelined three deep: group g's DMA +
  logits + leaky-relu + exp land while g-1 runs its DVE multiply + PE
  accumulation and g-2 runs its epilogue, so no engine ever stalls on
  another's latency.  ACT writes exp(z-4) into the message tile's trailing
  8 columns ((c,h)-interleaved broadcast for layer 1's 8 heads, an 8x
  replica for layer 2's single head so the DVE multiply keeps its
  packed-innermost 2x mode).  Epilogues drain each window's PSUM with a
  single f16 ACT copy, then one reciprocal + one scale per group, into a
  partition-major [128, NW*C] output panel (the row-major layout's 128-256 B
  dram runs fell under the 512 B threshold where DMA cost doubles; the host
  unscrambles for free).

Measured per-core DMA floor is ~343 GB/s on one queue / ~355 on two (HBM
fair share); the edge kernels stream ~62/~32 MB per core per inference and
run within ~15% of that floor.

Environment workarounds: this container's walrus build allows only ONE
semaphore wait per instruction (split onto nop carriers post-scheduling), and
the GPSIMD ucode libraries are absent (so no dma_gather/indirect-DMA fast
paths - hence the host-gather design).
"""
import numpy as np

import concourse.bass as bass
import concourse.mybir as mybir
import concourse.tile as tile
from concourse.bass_utils import run_bass_kernel_spmd

P = 128
F16 = mybir.dt.float16
F32 = mybir.dt.float32
F8 = mybir.dt.float8e4
AF = mybir.ActivationFunctionType
OP = mybir.AluOpType
NEG_SLOPE = 0.2
EXP_BIAS = -4.0     # exp(z + EXP_BIAS): constant shift cancels in softmax
NEG_INF = -60000.0  # pad-slot logit: exp(lrelu(.)+bias) underflows to 0
N_CORES = 8
EPS = 1e-30
CH = 448            # node-kernel matmul chunk (PSUM: 448*4B <= 2KB bank)
GCAP1, NWG1 = 90, 8      # E1 groups: count % 3 == 0 so the For_i seam's
                         # first DMA reuses a buffer freed 3 groups early
GCAP2, NWG2 = 208, 21    # E2 groups: ditto (9 groups, % 3 == 0)

# ------------------------------------------------------------------ patches

_wsplit_counter = [0]


def _split_excess_waits(nc, max_waits=1):
    """This walrus build rejects >1 sem-wait per instruction ("Too many sync
    wait commands"). Move overflow waits onto same-engine nop carriers."""
    n_split = 0
    for f in nc.m.functions:
        for blk in f.blocks:
            changed = False
            out = []
            for inst in blk.instructions:
                si = inst.sync_info
                if si is not None and len(si.on_wait) > max_waits:
                    waits = list(si.on_wait)
                    keep = waits[len(waits) - max_waits:]
                    overflow = waits[: len(waits) - max_waits]
                    for i in range(0, len(overflow), max_waits):
                        _wsplit_counter[0] += 1
                        nop = mybir.InstNoOp(
                            name=f"I-wsplit-{_wsplit_counter[0]}", ins=[], outs=[])
                        nop.engine = inst.engine
                        nop.sync_info = mybir.SyncInfo(
                            on_wait=overflow[i: i + max_waits], on_update=[])
                        out.append(nop)
                    inst.sync_info = mybir.SyncInfo(
                        on_wait=keep, on_update=list(si.on_update))
                    changed = True
                    n_split += 1
                out.append(inst)
            if changed:
                blk.instructions = out
    return n_split


def _finalize_kernel(nc):
    import bass_rust as _bass_rust
    from concourse.library_config import all_libraries, standard
    from concourse.library_overlay import lower_extended_insts

    inst_type_to_lib_mask = {}
    for lib in all_libraries:
        for inst_type in lib.instructions:
            inst_type_to_lib_mask[inst_type] = inst_type_to_lib_mask.get(
                inst_type, 0) | (1 << lib.index)
    _bass_rust.insert_library_loads(
        nc, inst_type_to_lib_mask, len(all_libraries), standard.index)
    lower_extended_insts(nc)
    _split_excess_waits(nc)


# ------------------------------------------------------------------ host prep

class _Graph:
    """Degree-sorted grid preprocessing: sort nodes by in-degree, pack 128
    similar-degree nodes per window, deal windows round-robin across cores
    (slot i of every core shares one padded depth D_i so all cores run one
    identical SPMD program), and scatter each node's edges into its grid row.
    """

    def __init__(self, edge_index, n_nodes, n_cores):
        self.N = n_nodes
        self.C = n_cores
        src = np.asarray(edge_index[0], dtype=np.int64)
        dst = np.asarray(edge_index[1], dtype=np.int64)
        E = src.shape[0]

        deg = np.bincount(dst, minlength=n_nodes)
        order = np.argsort(deg, kind="stable")

        n_win_total = (n_nodes + P - 1) // P
        self.wpc = (n_win_total + n_cores - 1) // n_cores
        n_win = self.wpc * n_cores
        self.n_pad = n_win * P
        self.shard_nodes = self.wpc * P
        n_dummy = self.n_pad - n_nodes

        snode = np.full(self.n_pad, -1, dtype=np.int64)
        snode[n_dummy:] = order                      # ascending degree
        # rows_nodes[k][i, e] = natural node id at (core k, slot i, row e)
        self.rows_nodes = np.ascontiguousarray(
            snode.reshape(self.wpc, n_cores, P).transpose(1, 0, 2))

        wdeg = np.where(snode >= 0, deg[np.clip(snode, 0, None)], 0)
        wmax = wdeg.reshape(self.wpc, n_cores, P).max(axis=2)   # [wpc, cores]
        self.D = np.maximum(wmax.max(axis=1), 1).astype(np.int64)  # [wpc]
        self.off = np.concatenate([[0], np.cumsum(self.D)])
        self.TOT = int(self.D.sum())

        # position of each node in the sorted layout
        posq = np.empty(n_nodes, dtype=np.int64)
        posq[order] = np.arange(n_nodes) + n_dummy

        # scatter edges (dst-sorted, ranked within dst run) into grids
        perm = np.argsort(dst, kind="stable")
        src_s = src[perm]
        dst_s = dst[perm]
        bounds = np.searchsorted(dst_s, np.arange(n_nodes + 1))
        j_e = np.arange(E) - bounds[dst_s]           # rank within dst run
        q_e = posq[dst_s]
        g_e = q_e // P
        row_e = q_e % P
        core_e = g_e % n_cores
        slot_e = g_e // n_cores
        flat_e = self.off[slot_e] + j_e              # grid slot within [TOT]
        self.gidx = np.zeros((n_cores, self.TOT, P), dtype=np.int32)
        self.gidx[core_e, flat_e, row_e] = (src_s + 1).astype(np.int32)

        self.groups1 = self.make_groups(GCAP1, NWG1)
        self.groups2 = self.make_groups(GCAP2, NWG2)
        self.D_key = tuple(int(d) for d in self.D)

    def make_groups(self, gcap, nwg):
        """Window groups: sum(D) <= gcap, <= nwg windows per group."""
        groups = []
        i = 0
        while i < self.wpc:
            i0, sd, nw = i, 0, 0
            while (i < self.wpc and nw < nwg
                   and (nw == 0 or sd + int(self.D[i]) <= gcap)):
                sd += int(self.D[i])
                i += 1
                nw += 1
            groups.append((i0, nw, int(self.off[i0]), sd))
        return groups

    def stream_h(self, table, core):
        """[128, TOT*C] f16 grid gather: table rows by gidx (0 = zero pad)."""
        C = table.shape[1]
        tp = np.zeros((self.N + 1, C), dtype=np.float16)
        tp[1:] = table
        arr = tp[self.gidx[core]]                    # [TOT, P, C]
        return np.ascontiguousarray(arr.transpose(1, 0, 2)).reshape(
            P, self.TOT * C)

    def stream_als(self, table, core):
        """[128, TOT*H] f16: al_src grid; pad slots -> NEG_INF so exp()==0.
        Dummy rows get one j=0 slot with logit 0 so their softmax denominator
        stays finite (their h rows are zero, so the output row is 0)."""
        H = table.shape[1]
        tp = np.full((self.N + 1, H), NEG_INF, dtype=np.float16)
        tp[1:] = table
        arr = tp[self.gidx[core]]                    # [TOT, P, H]
        i_d, e_d = np.nonzero(self.rows_nodes[core] < 0)
        arr[self.off[i_d], e_d, :] = 0.0
        return np.ascontiguousarray(arr.transpose(1, 0, 2)).reshape(
            P, self.TOT * H)

    def stream_ald(self, table, core):
        """[128, wpc*H] f16: al_dst per (window, row). Dummy rows -> 0."""
        H = table.shape[1]
        tp = np.zeros((self.N + 1, H), dtype=np.float16)
        tp[1:] = table
        arr = tp[self.rows_nodes[core] + 1]          # [wpc, P, H]
        return np.ascontiguousarray(arr.transpose(1, 0, 2)).reshape(
            P, self.wpc * H)

    def stream_ald_exp(self, table, core):
        """[128, TOT*H] f16: al_dst replicated across each window's slots
        (slot grids are per-window blocks of D_i slots)."""
        H = table.shape[1]
        tp = np.zeros((self.N + 1, H), dtype=np.float16)
        tp[1:] = table
        arr = tp[self.rows_nodes[core] + 1]          # [wpc, P, H]
        rep = np.repeat(arr, self.D, axis=0)         # [TOT, P, H]
        return np.ascontiguousarray(rep.transpose(1, 0, 2)).reshape(
            P, self.TOT * H)

    def ident8(self):
        import ml_dtypes
        return np.eye(P, dtype=np.float32).astype(ml_dtypes.float8_e4m3)


# ------------------------------------------------------------------ builders

def _build_node(SH, c_in, m_h, m_al, elu, bias_in, bench_loop=1):
    """Per-node transform: hT = (elu?(xT+b)) @ w, alT = same @ wal.
    When m_h+m_al <= 128 the two matmuls merge into one.  The whole per-core
    panel is SBUF-resident: quarters stream in with fat DMAs, chunked matmuls
    write a staged output panel, and a few fat DMAs store it."""
    merged = (m_h + m_al) <= P
    M = m_h + m_al if merged else m_h
    QN = 4 if elu else 7   # finer input slices when no per-slice ELU cost
    QS = SH // QN
    NQUAD = SH // (2 * CH)        # 2 al-chunks stack into one PSUM bank
    assert SH % QN == 0 and QS % CH == 0 and SH % (2 * CH) == 0
    nc = bass.Bass()
    xT = nc.dram_tensor("xT", [c_in, SH], F16, kind="ExternalInput")
    w = nc.dram_tensor("w", [c_in, M], F16, kind="ExternalInput")
    if not merged:
        assert m_al <= 32
        wal = nc.dram_tensor("wal", [c_in, 32], F16, kind="ExternalInput")
    if bias_in:
        bvec = nc.dram_tensor("bvec", [c_in, 1], F32, kind="ExternalInput")
    hT = nc.dram_tensor("hT", [M, SH], F16, kind="ExternalOutput")
    if not merged:
        # partition-stacked al panel: row 32k+r, col cq*CH+x holds
        # al[r] of chunk 2*cq+k (host unscrambles)
        alT = nc.dram_tensor("alT", [64, NQUAD * CH], F16,
                             kind="ExternalOutput")

    with tile.TileContext(nc) as tc:
        with (
            tc.tile_pool(name="const", bufs=1) as constp,
            tc.tile_pool(name="xin", bufs=2) as xinp,
            tc.tile_pool(name="hout", bufs=2) as houtp,
            tc.tile_pool(name="work", bufs=4) as workp,
            tc.tile_pool(name="psH", bufs=5, space="PSUM") as psH,
            tc.tile_pool(name="psA", bufs=3, space="PSUM") as psA,
        ):
            w_sb = constp.tile([c_in, M], F16)
            nc.sync.dma_start(out=w_sb[:], in_=w[:])
            if not merged:
                # wal host-padded to 32 cols (zeros) so every partition of
                # the stacked al PSUM region is written (no uninit reads)
                wal_sb = constp.tile([c_in, 32], F16)
                nc.sync.dma_start(out=wal_sb[:], in_=wal[:])
            if bias_in:
                b_sb = constp.tile([c_in, 1], F32)
                nc.sync.dma_start(out=b_sb[:], in_=bvec[:])

            def body(_iv=None):
                # every DMA rides SP: a queued DMA holds its issuing engine's
                # sequencer for the whole transfer, so ACT/DVE must stay clean
                xq = [xinp.tile([c_in, QS], F16, tag=f"x{q}", name=f"xq{q}")
                      for q in range(QN)]
                for q in range(QN):
                    nc.sync.dma_start(out=xq[q][:],
                                      in_=xT[:, q * QS:(q + 1) * QS])
                hq = [houtp.tile([M, QS], F16, tag=f"h{q}", name=f"hq{q}")
                      for q in range(QN)]
                if not merged:
                    alout = houtp.tile([64, NQUAD * CH], F16, tag="alo")
                quad = {}

                def qfront(q):
                    """Quarter-granular ELU stage A: one fat ACT exp."""
                    if not elu:
                        return None
                    rhs = xq[q][:]
                    if bias_in:
                        nc.vector.tensor_scalar(
                            rhs, rhs, b_sb[:, 0:1], None, OP.add)
                    et = workp.tile([c_in, QS], F16, tag="et")
                    nc.scalar.activation(et[:], rhs, AF.Exp)
                    return et

                def qback(q, et):
                    if elu:
                        # elu(x) = (min(exp(x),1) - 1) + max(x,0), all 2x DVE
                        mn = workp.tile([c_in, QS], F16, tag="mn")
                        nc.vector.tensor_scalar(
                            mn[:], et[:], 1.0, -1.0, OP.min, OP.add)
                        mx = workp.tile([c_in, QS], F16, tag="mx")
                        nc.vector.tensor_scalar(
                            mx[:], xq[q][:], 0.0, None, OP.max)
                        xe = workp.tile([c_in, QS], F16, tag="xe")
                        nc.vector.tensor_tensor(
                            out=xe[:], in0=mn[:], in1=mx[:], op=OP.add)
                        src = xe
                    else:
                        src = xq[q]
                    for j in range(QS // CH):
                        ci = q * (QS // CH) + j
                        qo = j * CH
                        rhs = src[:, qo:qo + CH]
                        ph = psH.tile([M, CH], F32, tag="ph")
                        nc.tensor.matmul(ph[:], w_sb[:], rhs,
                                         start=True, stop=True)
                        dve_copy = (ci % 7 < 3) if elu else (ci % 2 == 1)
                        if dve_copy:
                            nc.vector.tensor_copy(hq[q][:, qo:qo + CH],
                                                  ph[:])
                        else:
                            nc.scalar.activation(hq[q][:, qo:qo + CH],
                                                 ph[:], AF.Copy)
                        if not merged:
                            # stack 2 chunks' al outputs on partitions
                            # 0/32 of one PSUM bank -> 1 copy per pair
                            k = ci % 2
                            if k == 0:
                                quad["pa"] = psA.tile([64, CH], F32,
                                                      tag="paq", name="paq")
                            pa = quad["pa"]
                            nc.tensor.matmul(pa[32 * k:32 * k + 32, :],
                                             wal_sb[:], rhs,
                                             start=True, stop=True)
                            if k == 1:
                                cq = ci // 2
                                if cq % 2 == 0:
                                    nc.vector.tensor_copy(
                                        alout[:, cq * CH:(cq + 1) * CH],
                                        pa[:])
                                else:
                                    nc.scalar.activation(
                                        alout[:, cq * CH:(cq + 1) * CH],
                                        pa[:], AF.Copy)
                    nc.sync.dma_start(out=hT[:, q * QS:(q + 1) * QS],
                                      in_=hq[q][:])

                prev = None
                for q in range(QN):
                    et = qfront(q)
                    if prev is not None:
                        qback(*prev)
                    prev = (q, et)
                qback(*prev)
                if not merged:
                    nc.sync.dma_start(out=alT[:], in_=alout[:])

            if bench_loop > 1:
                with tc.For_i(0, bench_loop, 1) as _iv:
                    body(_iv)
            else:
                body()
    _finalize_kernel(nc)
    return nc


def _build_edge_g(D_list, groups, TOT, Cc, H, bias_out=False, elu_out=False,
                  ald_exp=False, bench_loop=1):
    """Edge aggregation over degree-sorted grids.  Per group of windows:
    one h[src] grid DMA, one DVE logit add per window, one ACT leaky-relu,
    one ACT exp into the message tile's trailing EB columns, one DVE
    multiply, then D accumulating identity matmuls per window.  Epilogues
    run one group late so no engine stalls on PSUM completion."""
    EB = 8 if H > 1 else 4   # exp block: 8 heads, or 4 replicas (1 head)
    SLOT = Cc + EB
    G = Cc // EB
    NW = len(D_list)
    GS = max(sd for _, _, _, sd in groups)
    NWmax = max(nw for _, nw, _, _ in groups)

    nc = bass.Bass()
    hsrc = nc.dram_tensor("hsrc", [P, TOT * Cc], F16, kind="ExternalInput")
    # ald_exp: als carries [al_src | al_dst] interleaved per slot (doubles
    # the per-partition dram run length past the 512 B fast-DMA threshold)
    als = nc.dram_tensor("als", [P, TOT * H * (2 if ald_exp else 1)], F16,
                         kind="ExternalInput")
    if not ald_exp:
        ald = nc.dram_tensor("ald", [P, NW * H], F16, kind="ExternalInput")
    ident = nc.dram_tensor("ident", [P, P], F8, kind="ExternalInput")
    if bias_out:
        brep = nc.dram_tensor("brep", [P, Cc], F32, kind="ExternalInput")
    # partition-major output: per-partition contiguous runs (the [NW*P, Cc]
    # layout had 128-256 B dram runs, under the 512 B fast-DMA threshold)
    out = nc.dram_tensor("out", [P, NW * Cc], F16, kind="ExternalOutput")

    with tile.TileContext(nc) as tc:
        with (
            tc.tile_pool(name="const", bufs=1) as constp,
            tc.tile_pool(name="aldp", bufs=2) as aldp,
            tc.tile_pool(name="alg", bufs=3) as algp,
            tc.tile_pool(name="hs", bufs=3) as hsp,
            tc.tile_pool(name="za", bufs=3) as zap,
            tc.tile_pool(name="msg", bufs=3) as msgp,
            tc.tile_pool(name="epi", bufs=3) as epip,
            tc.tile_pool(name="og", bufs=2) as ogp,
            tc.tile_pool(name="psW", bufs=8, space="PSUM") as pswp,
        ):
            BSLOT = 512 // SLOT      # windows per PSUM bank
            ident_sb = constp.tile([P, P], F8)
            nc.scalar.dma_start(out=ident_sb[:], in_=ident[:])
            ebias_sb = constp.tile([P, 1], F32)
            nc.vector.memset(ebias_sb[:], EXP_BIAS)
            if bias_out:
                brep_sb = constp.tile([P, Cc], F32)
                nc.scalar.dma_start(out=brep_sb[:], in_=brep[:])

            pend = []

            def front(grp, ald_sb):
                """DMA + logit add + leaky-relu + exp for one group."""
                i0, nw, off0, sd = grp
                hs = hsp.tile([P, GS * Cc], F16, tag="hs")
                nc.sync.dma_start(out=hs[:, :sd * Cc],
                                  in_=hsrc[:, off0 * Cc:(off0 + sd) * Cc])
                AW = H * (2 if ald_exp else 1)
                alg = algp.tile([P, GS * AW], F16, tag="alg")
                nc.sync.dma_start(out=alg[:, :sd * AW],
                                  in_=als[:, off0 * AW:(off0 + sd) * AW])
                za = zap.tile([P, GS * H], F16, tag="za")
                if ald_exp:
                    # interleaved [al_src | al_dst] slots: one add per group
                    a0 = alg[:]
                    av = bass.AP(a0.tensor, a0.offset, [a0.ap[0], [2, sd]])
                    bv = bass.AP(a0.tensor, a0.offset + 1,
                                 [a0.ap[0], [2, sd]])
                    nc.vector.tensor_tensor(out=za[:, :sd],
                                            in0=av, in1=bv, op=OP.add)
                doff = 0
                for wl in range(nw) if not ald_exp else ():
                    D = int(D_list[i0 + wl])
                    o0 = doff * H
                    if H > 1:
                        av = alg[:, o0:o0 + D * H].rearrange(
                            "p (d h) -> p d h", d=D)
                        zv = za[:, o0:o0 + D * H].rearrange(
                            "p (d h) -> p d h", d=D)
                        ad = ald_sb[:, (i0 + wl) * H:(i0 + wl + 1) * H]
                        ab = bass.AP(ad.tensor, ad.offset,
                                     [ad.ap[0], [0, D], [1, H]])
                    else:
                        av = alg[:, o0:o0 + D]
                        zv = za[:, o0:o0 + D]
                        ad = ald_sb[:, i0 + wl:i0 + wl + 1]
                        ab = bass.AP(ad.tensor, ad.offset,
                                     [ad.ap[0], [0, D]])
                    nc.vector.tensor_tensor(out=zv, in0=av, in1=ab, op=OP.add)
                    doff += D
                nc.scalar.activation(za[:, :sd * H], za[:, :sd * H],
                                     AF.Prelu, alpha=NEG_SLOPE)
                msg = msgp.tile([P, GS * SLOT], F16, tag="msg")
                m3 = msg[:, :sd * SLOT].rearrange("p (d s) -> p d s", s=SLOT)
                eb_out = m3[:, :, Cc:Cc + EB]
                if H > 1:
                    e_in = za[:, :sd * H].rearrange("p (d h) -> p d h", d=sd)
                else:
                    z0 = za[:, :sd]
                    e_in = bass.AP(z0.tensor, z0.offset,
                                   [z0.ap[0], [1, sd], [0, EB]])
                nc.scalar.activation(eb_out, e_in, AF.Exp, bias=ebias_sb[:])
                return hs, msg

            def back(grp, st):
                """DVE message multiply + PE identity accumulation."""
                i0, nw, off0, sd = grp
                hs, msg = st
                m3 = msg[:, :sd * SLOT].rearrange("p (d s) -> p d s", s=SLOT)
                eb_out = m3[:, :, Cc:Cc + EB]
                mo = m3[:, :, 0:Cc].rearrange("p d (g h) -> p d g h", h=EB)
                hi = hs[:, :sd * Cc].rearrange(
                    "p (d g h) -> p d g h", d=sd, h=EB)
                ei = bass.AP(eb_out.tensor, eb_out.offset,
                             [eb_out.ap[0], eb_out.ap[1], [0, G], [1, EB]])
                nc.vector.tensor_tensor(out=mo, in0=hi, in1=ei, op=OP.mult)
                doff = 0
                bank = None
                for wl in range(nw):
                    D = int(D_list[i0 + wl])
                    if wl % BSLOT == 0:
                        bank = pswp.tile([P, 512], F32, tag="psw",
                                         name="pswbank")
                    sl = (wl % BSLOT) * SLOT
                    psw = bank[:, sl:sl + SLOT]
                    for j in range(D):
                        mv = msg[:, (doff + j) * SLOT:(doff + j + 1) * SLOT]
                        nc.tensor.matmul(psw, ident_sb[:], mv,
                                         start=(j == 0), stop=(j == D - 1))
                    pend.append(psw)
                    doff += D

            ogst = {}

            def epilogue(grp, flush):
                """One f16 PSUM copy per window, then a single reciprocal +
                scale per group; output DMAs batch two groups per write so
                HBM sees fewer read/write turnarounds against the streams."""
                i0, nw, off0, sd = grp
                op_t = epip.tile([P, NWmax * SLOT], F16, tag="o1p")
                for wl in range(nw):
                    psw = pend.pop(0)
                    nc.scalar.activation(op_t[:, wl * SLOT:(wl + 1) * SLOT],
                                         psw, AF.Copy)
                opv = op_t[:, :nw * SLOT]
                rec = epip.tile([P, NWmax * EB], F16, tag="rec")
                rv = rec[:, :nw * EB].rearrange("p (w h) -> p w h", w=nw)
                dap = bass.AP(opv.tensor, opv.offset + Cc,
                              [opv.ap[0], [SLOT, nw], [1, EB]])
                with nc.allow_low_precision(
                        reason="softmax denominators are O(1)"):
                    nc.vector.reciprocal(rv, dap)
                if not ogst:
                    ogst["og"] = ogp.tile([P, 2 * NWmax * Cc], F16,
                                          tag="og", name="ogpair")
                    ogst["i0"] = i0
                    ogst["fill"] = 0
                og = ogst["og"]
                ob = ogst["fill"]
                o_in = bass.AP(opv.tensor, opv.offset,
                               [opv.ap[0], [SLOT, nw], [EB, G], [1, EB]])
                r0 = rec[:]
                r_b = bass.AP(r0.tensor, r0.offset,
                              [r0.ap[0], [EB, nw], [0, G], [1, EB]])
                oo = og[:, ob:ob + nw * Cc].rearrange(
                    "p (w g h) -> p w g h", w=nw, h=EB)
                nc.vector.tensor_tensor(out=oo, in0=o_in, in1=r_b,
                                        op=OP.mult)
                if bias_out:     # layer bias: before the inter-layer elu
                    ov2 = og[:, ob:ob + nw * Cc].rearrange(
                        "p (w c) -> p w c", w=nw)
                    b0 = brep_sb[:]
                    b_b = bass.AP(b0.tensor, b0.offset,
                                  [b0.ap[0], [0, nw], [1, Cc]])
                    nc.vector.tensor_tensor(out=ov2, in0=ov2, in1=b_b,
                                            op=OP.add)
                if elu_out:
                    # elu(x) = max(x,0) + (min(exp(x),1) - 1), in place on og
                    ogv = og[:, ob:ob + nw * Cc]
                    et = epip.tile([P, NWmax * Cc], F16, tag="et")
                    etv = et[:, :nw * Cc]
                    nc.scalar.activation(etv, ogv, AF.Exp)
                    nc.vector.tensor_scalar(etv, etv, 1.0, -1.0,
                                            OP.min, OP.add)
                    nc.vector.scalar_tensor_tensor(ogv, ogv, 0.0, etv,
                                                   OP.max, OP.add)
                ogst["fill"] = ob + nw * Cc
                if flush:
                    f = ogst["fill"]
                    o0 = ogst["i0"] * Cc
                    nc.scalar.dma_start(out=out[:, o0:o0 + f],
                                        in_=og[:, :f])
                    ogst.clear()

            def body(_iv=None):
                if not ald_exp:
                    ald_sb = aldp.tile([P, NW * H], F16, tag="ald")
                    nc.scalar.dma_start(out=ald_sb[:], in_=ald[:])
                else:
                    ald_sb = None
                pend.clear()
                ogst.clear()
                ng = len(groups)
                ep = [0]

                def run_epi(gi):
                    epilogue(groups[gi],
                             flush=(ep[0] % 2 == 1) or (gi == ng - 1))
                    ep[0] += 1

                sts = [None] * ng
                for gi, grp in enumerate(groups):
                    sts[gi] = front(grp, ald_sb)
                    if gi >= 1:
                        back(groups[gi - 1], sts[gi - 1])
                        sts[gi - 1] = None
                    if gi >= 2:
                        run_epi(gi - 2)
                back(groups[ng - 1], sts[ng - 1])
                if ng >= 2:
                    run_epi(ng - 2)
                run_epi(ng - 1)

            if bench_loop > 1:
                with tc.For_i(0, bench_loop, 1) as _iv:
                    body(_iv)
            else:
                body()
    _finalize_kernel(nc)
    return nc


# ------------------------------------------------------------------ runner

def _fold_att(W, a):
    heads, hid = a.shape
    return np.einsum("ihc,hc->ih", W.reshape(W.shape[0], heads, hid), a)


class _GatRunner:
    def __init__(self, n_cores=N_CORES):
        self.C = n_cores
        self._graph = None
        self._graph_key = None
        self._kernels = {}
        self.last_maps = {}

    def graph(self, edge_index, n_nodes):
        key = hash(np.asarray(edge_index).tobytes())
        if key != self._graph_key:
            self._graph = _Graph(edge_index, n_nodes, self.C)
            self._graph_key = key
            self._kernels.clear()
        return self._graph

    def kernel(self, name, bench_loop=1, **kw):
        key = (name, bench_loop, tuple(sorted(kw.items())))
        if key not in self._kernels:
            g = self._graph
            if name.startswith("P"):
                self._kernels[key] = _build_node(
                    g.shard_nodes, bench_loop=bench_loop, **kw)
            elif name == "E1":
                self._kernels[key] = _build_edge_g(
                    g.D, g.groups1, g.TOT, 128, 8,
                    bench_loop=bench_loop, **kw)
            else:
                self._kernels[key] = _build_edge_g(
                    g.D, g.groups2, g.TOT, 64, 1, ald_exp=True,
                    bench_loop=bench_loop, **kw)
        return self._kernels[key]

    def _run(self, name, nc, maps):
        self.last_maps[name] = maps
        res = run_bass_kernel_spmd(nc, maps, core_ids=list(range(self.C)))
        return res.results

    def run(self, x, edge_index, W1, a_src1, a_dst1, b1, W2, a_src2, a_dst2,
            b2):
        C = self.C
        N, IN_C = x.shape
        HEADS, HID = a_src1.shape
        HC = HEADS * HID
        OUT_C = W2.shape[1]
        g = self.graph(edge_index, N)
        SH = g.shard_nodes
        # (c,h)-interleaved channel order for the layer-1 hidden features:
        # col c*H+h of h1 holds math channel h*HID+c. Folded into W1's
        # columns (P0) and W2's rows (P2) on the host - pure permutation.
        perm = np.array([(j % HEADS) * HID + j // HEADS
                         for j in range(HC)], dtype=np.int64)

        # ---- P0: per-node h1 / logits --------------------------------
        xT_pad = np.zeros((IN_C, g.n_pad), dtype=np.float16)
        xT_pad[:, :N] = np.asarray(x, np.float32).T
        w1 = np.asarray(W1, np.float32)
        m_al = 2 * HEADS
        wal1 = np.zeros((IN_C, 32), dtype=np.float32)
        wal1[:, :m_al] = np.concatenate(
            [_fold_att(w1, np.asarray(a_src1, np.float32)),
             _fold_att(w1, np.asarray(a_dst1, np.float32))], axis=1)
        mapsP0 = [{"xT": np.ascontiguousarray(xT_pad[:, k * SH:(k + 1) * SH]),
                   "w": np.ascontiguousarray(w1[:, perm]).astype(np.float16),
                   "wal": wal1.astype(np.float16)} for k in range(C)]
        ncP0 = self.kernel("P0", c_in=IN_C, m_h=HC, m_al=m_al,
                           elu=False, bias_in=False)
        resP0 = self._run("P0", ncP0, mapsP0)
        h1 = np.ascontiguousarray(
            np.concatenate([r["hT"] for r in resP0], axis=1).T)[:N]
        # unscramble the partition-stacked al panel: row 32k+r, col cq*CH+x
        # holds al[r] of chunk 4*cq+k
        nq = SH // (2 * CH)
        al1 = np.concatenate(
            [r["alT"].reshape(2, 32, nq, CH)[:, :m_al]
             .transpose(1, 2, 0, 3).reshape(m_al, SH)
             for r in resP0], axis=1)                    # [16, Np]
        als1 = np.ascontiguousarray(al1[:HEADS, :N].T)
        ald1 = np.ascontiguousarray(al1[HEADS:, :N].T)

        # ---- E1: layer-1 edge aggregation + bias + ELU ---------------
        id8 = g.ident8()
        b1nz = bool(np.any(np.asarray(b1)))
        mapsE1 = []
        for k in range(C):
            m = {"hsrc": g.stream_h(h1, k),
                 "als": g.stream_als(als1, k),
                 "ald": g.stream_ald(ald1, k),
                 "ident": id8}
            if b1nz:
                m["brep"] = np.tile(
                    np.asarray(b1, np.float32)[perm], (P, 1))
            mapsE1.append(m)
        ncE1 = self.kernel("E1", bias_out=b1nz)
        resE1 = self._run("E1", ncE1, mapsE1)
        out1 = np.concatenate(
            [r["out"].reshape(P, g.wpc, HC).transpose(1, 0, 2)
             .reshape(g.wpc * P, HC) for r in resE1], axis=0)
        # rows of out1 are (core, slot, row) -> natural node rowmap
        rowmap = g.rows_nodes.reshape(-1)            # [C*wpc*P]

        # ---- P2: ELU + per-node h2 / logits --------------------------
        o1T = np.ascontiguousarray(out1.T)           # [HC, C*SH] f16
        w2 = np.asarray(W2, np.float32)
        wal2 = np.concatenate(
            [_fold_att(w2, np.asarray(a_src2, np.float32)),
             _fold_att(w2, np.asarray(a_dst2, np.float32))], axis=1)
        w2all = np.concatenate([w2[perm], wal2[perm]], axis=1)  # [HC, 66]
        mapsP2 = [
            {"xT": np.ascontiguousarray(o1T[:, k * SH:(k + 1) * SH]),
             "w": w2all.astype(np.float16)} for k in range(C)]
        # out1 already carries b1 (E1 bias_out); P2 applies the ELU
        ncP2 = self.kernel("P2", c_in=HC, m_h=OUT_C, m_al=2, elu=True,
                           bias_in=False)
        resP2 = self._run("P2", ncP2, mapsP2)
        h2al = np.concatenate([r["hT"] for r in resP2], axis=1)  # [66, Np]
        valid = rowmap >= 0
        vrows = rowmap[valid]
        h2 = np.zeros((N, OUT_C), dtype=np.float16)
        h2[vrows] = h2al[:OUT_C].T[valid]
        als2 = np.zeros((N, 1), dtype=np.float16)
        als2[vrows, 0] = h2al[OUT_C][valid]
        ald2 = np.zeros((N, 1), dtype=np.float16)
        ald2[vrows, 0] = h2al[OUT_C + 1][valid]

        # ---- E2: layer-2 edge aggregation ----------------------------
        b2nz = bool(np.any(np.asarray(b2)))
        mapsE2 = []
        for k in range(C):
            a_s = g.stream_als(als2, k).reshape(P, g.TOT)
            a_d = g.stream_ald_exp(ald2, k).reshape(P, g.TOT)
            m = {"hsrc": g.stream_h(h2, k),
                 "als": np.ascontiguousarray(
                     np.stack([a_s, a_d], axis=2)).reshape(P, g.TOT * 2),
                 "ident": id8}
            if b2nz:
                m["brep"] = np.tile(np.asarray(b2, np.float32), (P, 1))
            mapsE2.append(m)
        ncE2 = self.kernel("E2", bias_out=b2nz)
        resE2 = self._run("E2", ncE2, mapsE2)
        out2 = np.concatenate(
            [r["out"].reshape(P, g.wpc, OUT_C).transpose(1, 0, 2)
             .reshape(g.wpc * P, OUT_C) for r in resE2], axis=0)
        out_full = np.zeros((N, OUT_C), dtype=np.float32)
        out_full[vrows] = out2[valid]
        return out_full


_RUNNER = _GatRunner()


def kernel(x, edge_index, W1, a_src1, a_dst1, b1, W2, a_src2, a_dst2, b2):
    """Full-input / full-output entry point. Returns [N, OUT_C] float32."""
    args = [np.asarray(v) for v in
            (x, edge_index, W1, a_src1, a_dst1, b1, W2, a_src2, a_dst2, b2)]
    return _RUNNER.run(*args).astype(np.float32)


# revision 68
# speedup vs baseline: 1.0815x; 1.0001x over previous
"""Trainium (trn2) Bass kernel for a 2-layer GAT over N=100k nodes / E=1.7M edges.

Strategy (degree-sorted edge grids + identity-stationary PE accumulation)
-------------------------------------------------------------------------
Nodes are sorted by in-degree on the host and packed into windows of 128
similar-degree destination nodes; windows are dealt round-robin across the 8
NeuronCores.  Each window's edges form a dense grid [128 nodes x D slots]
(D = max in-window degree, padded slots carry -inf logits so exp()==0), so
slot j of all 128 nodes is a 128-edge tile whose destination map is the
IDENTITY: the tensor engine accumulates the per-slot message tiles straight
into the window's PSUM bank with a never-changing fp8 identity stationary.
Degree sorting keeps grid padding at ~1.3%, and the one-hot selection stream
of the classic dst-sorted formulation (128 B/edge of pure index overhead)
disappears entirely.

Each GAT layer runs as TWO SPMD kernels with host-side index gathers (pure
permutations / casts - no host FLOPs) between them:

* node kernel (P0/P2): h = x @ W plus folded attention logits computed once
  per node (dense matmuls).  The full per-core input/output panels live in
  SBUF, loaded/stored with a handful of fat DMAs (per-chunk 1 KB/partition
  DMAs were latency-bound at ~140 GB/s); every DMA rides the SP queue since
  a queued DMA holds its issuing engine's sequencer for the whole transfer.
  P0's 16 logit rows stack two chunks per PSUM bank at partitions 0/32
  (tile_position) so one PSUM->SBUF copy drains two matmuls; P2 computes the
  inter-layer ELU as exp (one fat ACT op per quarter-panel, emitted a
  quarter ahead) + two 2x DVE ops, with PSUM copies balanced across ACT/DVE.
* edge kernel (E1/E2): streams h[src] grids (256/128 B per edge slot) and
  al_src logit grids (16/2 B); al_dst is a tiny per-window constant for E1
  and a host-replicated per-slot stream for E2 (one group-wide DVE add
  instead of 21 window-sized ones).  Windows are processed in groups
  (sum of D <= 96/192) software-pipelined three deep: group g's DMA +
  logits + leaky-relu + exp land while g-1 runs its DVE multiply + PE
  accumulation and g-2 runs its epilogue, so no engine ever stalls on
  another's latency.  ACT writes exp(z-4) into the message tile's trailing
  8 columns ((c,h)-interleaved broadcast for layer 1's 8 heads, an 8x
  replica for layer 2's single head so the DVE multiply keeps its
  packed-innermost 2x mode).  Epilogues drain each window's PSUM with a
  single f16 ACT copy, then one reciprocal + one scale per group, into a
  partition-major [128, NW*C] output panel (the row-major layout's 128-256 B
  dram runs fell under the 512 B threshold where DMA cost doubles; the host
  unscrambles for free).

Measured per-core DMA floor is ~343 GB/s on one queue / ~355 on two (HBM
fair share); the edge kernels stream ~62/~32 MB per core per inference and
run within ~15% of that floor.

Environment workarounds: this container's walrus build allows only ONE
semaphore wait per instruction (split onto nop carriers post-scheduling), and
the GPSIMD ucode libraries are absent (so no dma_gather/indirect-DMA fast
paths - hence the host-gather design).
"""
import numpy as np

import concourse.bass as bass
import concourse.mybir as mybir
import concourse.tile as tile
from concourse.bass_utils import run_bass_kernel_spmd

P = 128
F16 = mybir.dt.float16
F32 = mybir.dt.float32
F8 = mybir.dt.float8e4
AF = mybir.ActivationFunctionType
OP = mybir.AluOpType
NEG_SLOPE = 0.2
EXP_BIAS = -4.0     # exp(z + EXP_BIAS): constant shift cancels in softmax
NEG_INF = -60000.0  # pad-slot logit: exp(lrelu(.)+bias) underflows to 0
N_CORES = 8
EPS = 1e-30
CH = 448            # node-kernel matmul chunk (PSUM: 448*4B <= 2KB bank)
GCAP1, NWG1 = 90, 8      # E1 groups: count % 3 == 0 so the For_i seam's
                         # first DMA reuses a buffer freed 3 groups early
GCAP2, NWG2 = 208, 21    # E2 groups: ditto (9 groups, % 3 == 0)

# ------------------------------------------------------------------ patches

_wsplit_counter = [0]


def _split_excess_waits(nc, max_waits=1):
    """This walrus build rejects >1 sem-wait per instruction ("Too many sync
    wait commands"). Move overflow waits onto same-engine nop carriers."""
    n_split = 0
    for f in nc.m.functions:
        for blk in f.blocks:
            changed = False
            out = []
            for inst in blk.instructions:
                si = inst.sync_info
                if si is not None and len(si.on_wait) > max_waits:
                    waits = list(si.on_wait)
                    keep = waits[len(waits) - max_waits:]
                    overflow = waits[: len(waits) - max_waits]
                    for i in range(0, len(overflow), max_waits):
                        _wsplit_counter[0] += 1
                        nop = mybir.InstNoOp(
                            name=f"I-wsplit-{_wsplit_counter[0]}", ins=[], outs=[])
                        nop.engine = inst.engine
                        nop.sync_info = mybir.SyncInfo(
                            on_wait=overflow[i: i + max_waits], on_update=[])
                        out.append(nop)
                    inst.sync_info = mybir.SyncInfo(
                        on_wait=keep, on_update=list(si.on_update))
                    changed = True
                    n_split += 1
                out.append(inst)
            if changed:
                blk.instructions = out
    return n_split


def _finalize_kernel(nc):
    import bass_rust as _bass_rust
    from concourse.library_config import all_libraries, standard
    from concourse.library_overlay import lower_extended_insts

    inst_type_to_lib_mask = {}
    for lib in all_libraries:
        for inst_type in lib.instructions:
            inst_type_to_lib_mask[inst_type] = inst_type_to_lib_mask.get(
                inst_type, 0) | (1 << lib.index)
    _bass_rust.insert_library_loads(
        nc, inst_type_to_lib_mask, len(all_libraries), standard.index)
    lower_extended_insts(nc)
    _split_excess_waits(nc)


# ------------------------------------------------------------------ host prep

class _Graph:
    """Degree-sorted grid preprocessing: sort nodes by in-degree, pack 128
    similar-degree nodes per window, deal windows round-robin across cores
    (slot i of every core shares one padded depth D_i so all cores run one
    identical SPMD program), and scatter each node's edges into its grid row.
    """

    def __init__(self, edge_index, n_nodes, n_cores):
        self.N = n_nodes
        self.C = n_cores
        src = np.asarray(edge_index[0], dtype=np.int64)
        dst = np.asarray(edge_index[1], dtype=np.int64)
        E = src.shape[0]

        deg = np.bincount(dst, minlength=n_nodes)
        order = np.argsort(deg, kind="stable")

        n_win_total = (n_nodes + P - 1) // P
        self.wpc = (n_win_total + n_cores - 1) // n_cores
        n_win = self.wpc * n_cores
        self.n_pad = n_win * P
        self.shard_nodes = self.wpc * P
        n_dummy = self.n_pad - n_nodes

        snode = np.full(self.n_pad, -1, dtype=np.int64)
        snode[n_dummy:] = order                      # ascending degree
        # rows_nodes[k][i, e] = natural node id at (core k, slot i, row e)
        self.rows_nodes = np.ascontiguousarray(
            snode.reshape(self.wpc, n_cores, P).transpose(1, 0, 2))

        wdeg = np.where(snode >= 0, deg[np.clip(snode, 0, None)], 0)
        wmax = wdeg.reshape(self.wpc, n_cores, P).max(axis=2)   # [wpc, cores]
        self.D = np.maximum(wmax.max(axis=1), 1).astype(np.int64)  # [wpc]
        self.off = np.concatenate([[0], np.cumsum(self.D)])
        self.TOT = int(self.D.sum())

        # position of each node in the sorted layout
        posq = np.empty(n_nodes, dtype=np.int64)
        posq[order] = np.arange(n_nodes) + n_dummy

        # scatter edges (dst-sorted, ranked within dst run) into grids
        perm = np.argsort(dst, kind="stable")
        src_s = src[perm]
        dst_s = dst[perm]
        bounds = np.searchsorted(dst_s, np.arange(n_nodes + 1))
        j_e = np.arange(E) - bounds[dst_s]           # rank within dst run
        q_e = posq[dst_s]
        g_e = q_e // P
        row_e = q_e % P
        core_e = g_e % n_cores
        slot_e = g_e // n_cores
        flat_e = self.off[slot_e] + j_e              # grid slot within [TOT]
        self.gidx = np.zeros((n_cores, self.TOT, P), dtype=np.int32)
        self.gidx[core_e, flat_e, row_e] = (src_s + 1).astype(np.int32)

        self.groups1 = self.make_groups(GCAP1, NWG1)
        self.groups2 = self.make_groups(GCAP2, NWG2)
        self.D_key = tuple(int(d) for d in self.D)

    def make_groups(self, gcap, nwg):
        """Window groups: sum(D) <= gcap, <= nwg windows per group."""
        groups = []
        i = 0
        while i < self.wpc:
            i0, sd, nw = i, 0, 0
            while (i < self.wpc and nw < nwg
                   and (nw == 0 or sd + int(self.D[i]) <= gcap)):
                sd += int(self.D[i])
                i += 1
                nw += 1
            groups.append((i0, nw, int(self.off[i0]), sd))
        return groups

    def stream_h(self, table, core):
        """[128, TOT*C] f16 grid gather: table rows by gidx (0 = zero pad)."""
        C = table.shape[1]
        tp = np.zeros((self.N + 1, C), dtype=np.float16)
        tp[1:] = table
        arr = tp[self.gidx[core]]                    # [TOT, P, C]
        return np.ascontiguousarray(arr.transpose(1, 0, 2)).reshape(
            P, self.TOT * C)

    def stream_als(self, table, core):
        """[128, TOT*H] f16: al_src grid; pad slots -> NEG_INF so exp()==0.
        Dummy rows get one j=0 slot with logit 0 so their softmax denominator
        stays finite (their h rows are zero, so the output row is 0)."""
        H = table.shape[1]
        tp = np.full((self.N + 1, H), NEG_INF, dtype=np.float16)
        tp[1:] = table
        arr = tp[self.gidx[core]]                    # [TOT, P, H]
        i_d, e_d = np.nonzero(self.rows_nodes[core] < 0)
        arr[self.off[i_d], e_d, :] = 0.0
        return np.ascontiguousarray(arr.transpose(1, 0, 2)).reshape(
            P, self.TOT * H)

    def stream_ald(self, table, core):
        """[128, wpc*H] f16: al_dst per (window, row). Dummy rows -> 0."""
        H = table.shape[1]
        tp = np.zeros((self.N + 1, H), dtype=np.float16)
        tp[1:] = table
        arr = tp[self.rows_nodes[core] + 1]          # [wpc, P, H]
        return np.ascontiguousarray(arr.transpose(1, 0, 2)).reshape(
            P, self.wpc * H)

    def stream_ald_exp(self, table, core):
        """[128, TOT*H] f16: al_dst replicated across each window's slots
        (slot grids are per-window blocks of D_i slots)."""
        H = table.shape[1]
        tp = np.zeros((self.N + 1, H), dtype=np.float16)
        tp[1:] = table
        arr = tp[self.rows_nodes[core] + 1]          # [wpc, P, H]
        rep = np.repeat(arr, self.D, axis=0)         # [TOT, P, H]
        return np.ascontiguousarray(rep.transpose(1, 0, 2)).reshape(
            P, self.TOT * H)

    def ident8(self):
        import ml_dtypes
        return np.eye(P, dtype=np.float32).astype(ml_dtypes.float8_e4m3)


# ------------------------------------------------------------------ builders

def _build_node(SH, c_in, m_h, m_al, elu, bias_in, bench_loop=1):
    """Per-node transform: hT = (elu?(xT+b)) @ w, alT = same @ wal.
    When m_h+m_al <= 128 the two matmuls merge into one.  The whole per-core
    panel is SBUF-resident: quarters stream in with fat DMAs, chunked matmuls
    write a staged output panel, and a few fat DMAs store it."""
    merged = (m_h + m_al) <= P
    M = m_h + m_al if merged else m_h
    QN = 4 if elu else 7   # finer input slices when no per-slice ELU cost
    QS = SH // QN
    NQUAD = SH // (2 * CH)        # 2 al-chunks stack into one PSUM bank
    assert SH % QN == 0 and QS % CH == 0 and SH % (2 * CH) == 0
    nc = bass.Bass()
    xT = nc.dram_tensor("xT", [c_in, SH], F16, kind="ExternalInput")
    w = nc.dram_tensor("w", [c_in, M], F16, kind="ExternalInput")
    if not merged:
        assert m_al <= 32
        wal = nc.dram_tensor("wal", [c_in, 32], F16, kind="ExternalInput")
    if bias_in:
        bvec = nc.dram_tensor("bvec", [c_in, 1], F32, kind="ExternalInput")
    hT = nc.dram_tensor("hT", [M, SH], F16, kind="ExternalOutput")
    if not merged:
        # partition-stacked al panel: row 32k+r, col cq*CH+x holds
        # al[r] of chunk 2*cq+k (host unscrambles)
        alT = nc.dram_tensor("alT", [64, NQUAD * CH], F16,
                             kind="ExternalOutput")

    with tile.TileContext(nc) as tc:
        with (
            tc.tile_pool(name="const", bufs=1) as constp,
            tc.tile_pool(name="xin", bufs=2) as xinp,
            tc.tile_pool(name="hout", bufs=2) as houtp,
            tc.tile_pool(name="work", bufs=4) as workp,
            tc.tile_pool(name="psH", bufs=5, space="PSUM") as psH,
            tc.tile_pool(name="psA", bufs=3, space="PSUM") as psA,
        ):
            w_sb = constp.tile([c_in, M], F16)
            nc.sync.dma_start(out=w_sb[:], in_=w[:])
            if not merged:
                # wal host-padded to 32 cols (zeros) so every partition of
                # the stacked al PSUM region is written (no uninit reads)
                wal_sb = constp.tile([c_in, 32], F16)
                nc.sync.dma_start(out=wal_sb[:], in_=wal[:])
            if bias_in:
                b_sb = constp.tile([c_in, 1], F32)
                nc.sync.dma_start(out=b_sb[:], in_=bvec[:])

            def body(_iv=None):
                # every DMA rides SP: a queued DMA holds its issuing engine's
                # sequencer for the whole transfer, so ACT/DVE must stay clean
                xq = [xinp.tile([c_in, QS], F16, tag=f"x{q}", name=f"xq{q}")
                      for q in range(QN)]
                for q in range(QN):
                    nc.sync.dma_start(out=xq[q][:],
                                      in_=xT[:, q * QS:(q + 1) * QS])
                hq = [houtp.tile([M, QS], F16, tag=f"h{q}", name=f"hq{q}")
                      for q in range(QN)]
                if not merged:
                    alout = houtp.tile([64, NQUAD * CH], F16, tag="alo")
                quad = {}

                def qfront(q):
                    """Quarter-granular ELU stage A: one fat ACT exp."""
                    if not elu:
                        return None
                    rhs = xq[q][:]
                    if bias_in:
                        nc.vector.tensor_scalar(
                            rhs, rhs, b_sb[:, 0:1], None, OP.add)
                    et = workp.tile([c_in, QS], F16, tag="et")
                    nc.scalar.activation(et[:], rhs, AF.Exp)
                    return et

                def qback(q, et):
                    if elu:
                        # elu(x) = (min(exp(x),1) - 1) + max(x,0), all 2x DVE
                        mn = workp.tile([c_in, QS], F16, tag="mn")
                        nc.vector.tensor_scalar(
                            mn[:], et[:], 1.0, -1.0, OP.min, OP.add)
                        mx = workp.tile([c_in, QS], F16, tag="mx")
                        nc.vector.tensor_scalar(
                            mx[:], xq[q][:], 0.0, None, OP.max)
                        xe = workp.tile([c_in, QS], F16, tag="xe")
                        nc.vector.tensor_tensor(
                            out=xe[:], in0=mn[:], in1=mx[:], op=OP.add)
                        src = xe
                    else:
                        src = xq[q]
                    for j in range(QS // CH):
                        ci = q * (QS // CH) + j
                        qo = j * CH
                        rhs = src[:, qo:qo + CH]
                        ph = psH.tile([M, CH], F32, tag="ph")
                        nc.tensor.matmul(ph[:], w_sb[:], rhs,
                                         start=True, stop=True)
                        dve_copy = (ci % 7 < 3) if elu else (ci % 2 == 1)
                        if dve_copy:
                            nc.vector.tensor_copy(hq[q][:, qo:qo + CH],
                                                  ph[:])
                        else:
                            nc.scalar.activation(hq[q][:, qo:qo + CH],
                                                 ph[:], AF.Copy)
                        if not merged:
                            # stack 2 chunks' al outputs on partitions
                            # 0/32 of one PSUM bank -> 1 copy per pair
                            k = ci % 2
                            if k == 0:
                                quad["pa"] = psA.tile([64, CH], F32,
                                                      tag="paq", name="paq")
                            pa = quad["pa"]
                            nc.tensor.matmul(pa[32 * k:32 * k + 32, :],
                                             wal_sb[:], rhs,
                                             start=True, stop=True)
                            if k == 1:
                                cq = ci // 2
                                if cq % 2 == 0:
                                    nc.vector.tensor_copy(
                                        alout[:, cq * CH:(cq + 1) * CH],
                                        pa[:])
                                else:
                                    nc.scalar.activation(
                                        alout[:, cq * CH:(cq + 1) * CH],
                                        pa[:], AF.Copy)
                    nc.sync.dma_start(out=hT[:, q * QS:(q + 1) * QS],
                                      in_=hq[q][:])

                prev = None
                for q in range(QN):
                    et = qfront(q)
                    if prev is not None:
                        qback(*prev)
                    prev = (q, et)
                qback(*prev)
                if not merged:
                    nc.sync.dma_start(out=alT[:], in_=alout[:])

            if bench_loop > 1:
                with tc.For_i(0, bench_loop, 1) as _iv:
                    body(_iv)
            else:
                body()
    _finalize_kernel(nc)
    return nc


def _build_edge_g(D_list, groups, TOT, Cc, H, bias_out=False, elu_out=False,
                  ald_exp=False, bench_loop=1):
    """Edge aggregation over degree-sorted grids.  Per group of windows:
    one h[src] grid DMA, one DVE logit add per window, one ACT leaky-relu,
    one ACT exp into the message tile's trailing EB columns, one DVE
    multiply, then D accumulating identity matmuls per window.  Epilogues
    run one group late so no engine stalls on PSUM completion."""
    EB = 8 if H > 1 else 4   # exp block: 8 heads, or 4 replicas (1 head)
    SLOT = Cc + EB
    G = Cc // EB
    NW = len(D_list)
    GS = max(sd for _, _, _, sd in groups)
    NWmax = max(nw for _, nw, _, _ in groups)

    nc = bass.Bass()
    hsrc = nc.dram_tensor("hsrc", [P, TOT * Cc], F16, kind="ExternalInput")
    # ald_exp: als carries [al_src | al_dst] interleaved per slot (doubles
    # the per-partition dram run length past the 512 B fast-DMA threshold)
    als = nc.dram_tensor("als", [P, TOT * H * (2 if ald_exp else 1)], F16,
                         kind="ExternalInput")
    if not ald_exp:
        ald = nc.dram_tensor("ald", [P, NW * H], F16, kind="ExternalInput")
    ident = nc.dram_tensor("ident", [P, P], F8, kind="ExternalInput")
    if bias_out:
        brep = nc.dram_tensor("brep", [P, Cc], F32, kind="ExternalInput")
    # partition-major output: per-partition contiguous runs (the [NW*P, Cc]
    # layout had 128-256 B dram runs, under the 512 B fast-DMA threshold)
    out = nc.dram_tensor("out", [P, NW * Cc], F16, kind="ExternalOutput")

    with tile.TileContext(nc) as tc:
        with (
            tc.tile_pool(name="const", bufs=1) as constp,
            tc.tile_pool(name="aldp", bufs=2) as aldp,
            tc.tile_pool(name="alg", bufs=3) as algp,
            tc.tile_pool(name="hs", bufs=3) as hsp,
            tc.tile_pool(name="za", bufs=3) as zap,
            tc.tile_pool(name="msg", bufs=3) as msgp,
            tc.tile_pool(name="epi", bufs=3) as epip,
            tc.tile_pool(name="og", bufs=2) as ogp,
            tc.tile_pool(name="psW", bufs=8, space="PSUM") as pswp,
        ):
            BSLOT = 512 // SLOT      # windows per PSUM bank
            ident_sb = constp.tile([P, P], F8)
            nc.scalar.dma_start(out=ident_sb[:], in_=ident[:])
            ebias_sb = constp.tile([P, 1], F32)
            nc.vector.memset(ebias_sb[:], EXP_BIAS)
            if bias_out:
                brep_sb = constp.tile([P, Cc], F32)
                nc.scalar.dma_start(out=brep_sb[:], in_=brep[:])

            pend = []

            def front(grp, ald_sb):
                """DMA + logit add + leaky-relu + exp for one group."""
                i0, nw, off0, sd = grp
                hs = hsp.tile([P, GS * Cc], F16, tag="hs")
                nc.sync.dma_start(out=hs[:, :sd * Cc],
                                  in_=hsrc[:, off0 * Cc:(off0 + sd) * Cc])
                AW = H * (2 if ald_exp else 1)
                alg = algp.tile([P, GS * AW], F16, tag="alg")
                nc.sync.dma_start(out=alg[:, :sd * AW],
                                  in_=als[:, off0 * AW:(off0 + sd) * AW])
                za = zap.tile([P, GS * H], F16, tag="za")
                if ald_exp:
                    # interleaved [al_src | al_dst] slots: one add per group
                    a0 = alg[:]
                    av = bass.AP(a0.tensor, a0.offset, [a0.ap[0], [2, sd]])
                    bv = bass.AP(a0.tensor, a0.offset + 1,
                                 [a0.ap[0], [2, sd]])
                    nc.vector.tensor_tensor(out=za[:, :sd],
                                            in0=av, in1=bv, op=OP.add)
                doff = 0
                for wl in range(nw) if not ald_exp else ():
                    D = int(D_list[i0 + wl])
                    o0 = doff * H
                    if H > 1:
                        av = alg[:, o0:o0 + D * H].rearrange(
                            "p (d h) -> p d h", d=D)
                        zv = za[:, o0:o0 + D * H].rearrange(
                            "p (d h) -> p d h", d=D)
                        ad = ald_sb[:, (i0 + wl) * H:(i0 + wl + 1) * H]
                        ab = bass.AP(ad.tensor, ad.offset,
                                     [ad.ap[0], [0, D], [1, H]])
                    else:
                        av = alg[:, o0:o0 + D]
                        zv = za[:, o0:o0 + D]
                        ad = ald_sb[:, i0 + wl:i0 + wl + 1]
                        ab = bass.AP(ad.tensor, ad.offset,
                                     [ad.ap[0], [0, D]])
                    nc.vector.tensor_tensor(out=zv, in0=av, in1=ab, op=OP.add)
                    doff += D
                nc.scalar.activation(za[:, :sd * H], za[:, :sd * H],
                                     AF.Prelu, alpha=NEG_SLOPE)
                msg = msgp.tile([P, GS * SLOT], F16, tag="msg")
                m3 = msg[:, :sd * SLOT].rearrange("p (d s) -> p d s", s=SLOT)
                eb_out = m3[:, :, Cc:Cc + EB]
                if H > 1:
                    e_in = za[:, :sd * H].rearrange("p (d h) -> p d h", d=sd)
                else:
                    z0 = za[:, :sd]
                    e_in = bass.AP(z0.tensor, z0.offset,
                                   [z0.ap[0], [1, sd], [0, EB]])
                nc.scalar.activation(eb_out, e_in, AF.Exp, bias=ebias_sb[:])
                return hs, msg

            def back(grp, st):
                """DVE message multiply + PE identity accumulation."""
                i0, nw, off0, sd = grp
                hs, msg = st
                m3 = msg[:, :sd * SLOT].rearrange("p (d s) -> p d s", s=SLOT)
                eb_out = m3[:, :, Cc:Cc + EB]
                mo = m3[:, :, 0:Cc].rearrange("p d (g h) -> p d g h", h=EB)
                hi = hs[:, :sd * Cc].rearrange(
                    "p (d g h) -> p d g h", d=sd, h=EB)
                ei = bass.AP(eb_out.tensor, eb_out.offset,
                             [eb_out.ap[0], eb_out.ap[1], [0, G], [1, EB]])
                nc.vector.tensor_tensor(out=mo, in0=hi, in1=ei, op=OP.mult)
                doff = 0
                bank = None
                for wl in range(nw):
                    D = int(D_list[i0 + wl])
                    if wl % BSLOT == 0:
                        bank = pswp.tile([P, 512], F32, tag="psw",
                                         name="pswbank")
                    sl = (wl % BSLOT) * SLOT
                    psw = bank[:, sl:sl + SLOT]
                    for j in range(D):
                        mv = msg[:, (doff + j) * SLOT:(doff + j + 1) * SLOT]
                        nc.tensor.matmul(psw, ident_sb[:], mv,
                                         start=(j == 0), stop=(j == D - 1))
                    pend.append(psw)
                    doff += D

            ogst = {}

            def epilogue(grp, flush):
                """One f16 PSUM copy per window, then a single reciprocal +
                scale per group; output DMAs batch two groups per write so
                HBM sees fewer read/write turnarounds against the streams."""
                i0, nw, off0, sd = grp
                op_t = epip.tile([P, NWmax * SLOT], F16, tag="o1p")
                for wl in range(nw):
                    psw = pend.pop(0)
                    nc.scalar.activation(op_t[:, wl * SLOT:(wl + 1) * SLOT],
                                         psw, AF.Copy)
                opv = op_t[:, :nw * SLOT]
                rec = epip.tile([P, NWmax * EB], F16, tag="rec")
                rv = rec[:, :nw * EB].rearrange("p (w h) -> p w h", w=nw)
                dap = bass.AP(opv.tensor, opv.offset + Cc,
                              [opv.ap[0], [SLOT, nw], [1, EB]])
                with nc.allow_low_precision(
                        reason="softmax denominators are O(1)"):
                    nc.vector.reciprocal(rv, dap)
                if not ogst:
                    ogst["og"] = ogp.tile([P, 2 * NWmax * Cc], F16,
                                          tag="og", name="ogpair")
                    ogst["i0"] = i0
                    ogst["fill"] = 0
                og = ogst["og"]
                ob = ogst["fill"]
                o_in = bass.AP(opv.tensor, opv.offset,
                               [opv.ap[0], [SLOT, nw], [EB, G], [1, EB]])
                r0 = rec[:]
                r_b = bass.AP(r0.tensor, r0.offset,
                              [r0.ap[0], [EB, nw], [0, G], [1, EB]])
                oo = og[:, ob:ob + nw * Cc].rearrange(
                    "p (w g h) -> p w g h", w=nw, h=EB)
                nc.vector.tensor_tensor(out=oo, in0=o_in, in1=r_b,
                                        op=OP.mult)
                if bias_out:     # layer bias: before the inter-layer elu
                    ov2 = og[:, ob:ob + nw * Cc].rearrange(
                        "p (w c) -> p w c", w=nw)
                    b0 = brep_sb[:]
                    b_b = bass.AP(b0.tensor, b0.offset,
                                  [b0.ap[0], [0, nw], [1, Cc]])
                    nc.vector.tensor_tensor(out=ov2, in0=ov2, in1=b_b,
                                            op=OP.add)
                if elu_out:
                    # elu(x) = max(x,0) + (min(exp(x),1) - 1), in place on og
                    ogv = og[:, ob:ob + nw * Cc]
                    et = epip.tile([P, NWmax * Cc], F16, tag="et")
                    etv = et[:, :nw * Cc]
                    nc.scalar.activation(etv, ogv, AF.Exp)
                    nc.vector.tensor_scalar(etv, etv, 1.0, -1.0,
                                            OP.min, OP.add)
                    nc.vector.scalar_tensor_tensor(ogv, ogv, 0.0, etv,
                                                   OP.max, OP.add)
                ogst["fill"] = ob + nw * Cc
                if flush:
                    f = ogst["fill"]
                    o0 = ogst["i0"] * Cc
                    nc.scalar.dma_start(out=out[:, o0:o0 + f],
                                        in_=og[:, :f])
                    ogst.clear()

            def body(_iv=None):
                if not ald_exp:
                    ald_sb = aldp.tile([P, NW * H], F16, tag="ald")
                    nc.scalar.dma_start(out=ald_sb[:], in_=ald[:])
                else:
                    ald_sb = None
                pend.clear()
                ogst.clear()
                ng = len(groups)
                ep = [0]

                def run_epi(gi):
                    epilogue(groups[gi],
                             flush=(ep[0] % 2 == 1) or (gi == ng - 1))
                    ep[0] += 1

                sts = [None] * ng
                for gi, grp in enumerate(groups):
                    sts[gi] = front(grp, ald_sb)
                    if gi >= 1:
                        back(groups[gi - 1], sts[gi - 1])
                        sts[gi - 1] = None
                    if gi >= 2:
                        run_epi(gi - 2)
                back(groups[ng - 1], sts[ng - 1])
                if ng >= 2:
                    run_epi(ng - 2)
                run_epi(ng - 1)

            if bench_loop > 1:
                with tc.For_i(0, bench_loop, 1) as _iv:
                    body(_iv)
            else:
                body()
    _finalize_kernel(nc)
    return nc


# ------------------------------------------------------------------ runner

def _fold_att(W, a):
    heads, hid = a.shape
    return np.einsum("ihc,hc->ih", W.reshape(W.shape[0], heads, hid), a)


class _GatRunner:
    def __init__(self, n_cores=N_CORES):
        self.C = n_cores
        self._graph = None
        self._graph_key = None
        self._kernels = {}
        self.last_maps = {}

    def graph(self, edge_index, n_nodes):
        key = hash(np.asarray(edge_index).tobytes())
        if key != self._graph_key:
            self._graph = _Graph(edge_index, n_nodes, self.C)
            self._graph_key = key
            self._kernels.clear()
        return self._graph

    def kernel(self, name, bench_loop=1, **kw):
        key = (name, bench_loop, tuple(sorted(kw.items())))
        if key not in self._kernels:
            g = self._graph
            if name.startswith("P"):
                self._kernels[key] = _build_node(
                    g.shard_nodes, bench_loop=bench_loop, **kw)
            elif name == "E1":
                self._kernels[key] = _build_edge_g(
                    g.D, g.groups1, g.TOT, 128, 8,
                    bench_loop=bench_loop, **kw)
            else:
                self._kernels[key] = _build_edge_g(
                    g.D, g.groups2, g.TOT, 64, 1, ald_exp=True,
                    bench_loop=bench_loop, **kw)
        return self._kernels[key]

    def _run(self, name, nc, maps):
        self.last_maps[name] = maps
        res = run_bass_kernel_spmd(nc, maps, core_ids=list(range(self.C)))
        return res.results

    def run(self, x, edge_index, W1, a_src1, a_dst1, b1, W2, a_src2, a_dst2,
            b2):
        C = self.C
        N, IN_C = x.shape
        HEADS, HID = a_src1.shape
        HC = HEADS * HID
        OUT_C = W2.shape[1]
        g = self.graph(edge_index, N)
        SH = g.shard_nodes
        # (c,h)-interleaved channel order for the layer-1 hidden features:
        # col c*H+h of h1 holds math channel h*HID+c. Folded into W1's
        # columns (P0) and W2's rows (P2) on the host - pure permutation.
        perm = np.array([(j % HEADS) * HID + j // HEADS
                         for j in range(HC)], dtype=np.int64)

        # ---- P0: per-node h1 / logits --------------------------------
        xT_pad = np.zeros((IN_C, g.n_pad), dtype=np.float16)
        xT_pad[:, :N] = np.asarray(x, np.float32).T
        w1 = np.asarray(W1, np.float32)
        m_al = 2 * HEADS
        wal1 = np.zeros((IN_C, 32), dtype=np.float32)
        wal1[:, :m_al] = np.concatenate(
            [_fold_att(w1, np.asarray(a_src1, np.float32)),
             _fold_att(w1, np.asarray(a_dst1, np.float32))], axis=1)
        mapsP0 = [{"xT": np.ascontiguousarray(xT_pad[:, k * SH:(k + 1) * SH]),
                   "w": np.ascontiguousarray(w1[:, perm]).astype(np.float16),
                   "wal": wal1.astype(np.float16)} for k in range(C)]
        ncP0 = self.kernel("P0", c_in=IN_C, m_h=HC, m_al=m_al,
                           elu=False, bias_in=False)
        resP0 = self._run("P0", ncP0, mapsP0)
        h1 = np.ascontiguousarray(
            np.concatenate([r["hT"] for r in resP0], axis=1).T)[:N]
        # unscramble the partition-stacked al panel: row 32k+r, col cq*CH+x
        # holds al[r] of chunk 4*cq+k
        nq = SH // (2 * CH)
        al1 = np.concatenate(
            [r["alT"].reshape(2, 32, nq, CH)[:, :m_al]
             .transpose(1, 2, 0, 3).reshape(m_al, SH)
             for r in resP0], axis=1)                    # [16, Np]
        als1 = np.ascontiguousarray(al1[:HEADS, :N].T)
        ald1 = np.ascontiguousarray(al1[HEADS:, :N].T)

        # ---- E1: layer-1 edge aggregation + bias + ELU ---------------
        id8 = g.ident8()
        b1nz = bool(np.any(np.asarray(b1)))
        mapsE1 = []
        for k in range(C):
            m = {"hsrc": g.stream_h(h1, k),
                 "als": g.stream_als(als1, k),
                 "ald": g.stream_ald(ald1, k),
                 "ident": id8}
            if b1nz:
                m["brep"] = np.tile(
                    np.asarray(b1, np.float32)[perm], (P, 1))
            mapsE1.append(m)
        ncE1 = self.kernel("E1", bias_out=b1nz)
        resE1 = self._run("E1", ncE1, mapsE1)
        out1 = np.concatenate(
            [r["out"].reshape(P, g.wpc, HC).transpose(1, 0, 2)
             .reshape(g.wpc * P, HC) for r in resE1], axis=0)
        # rows of out1 are (core, slot, row) -> natural node rowmap
        rowmap = g.rows_nodes.reshape(-1)            # [C*wpc*P]

        # ---- P2: ELU + per-node h2 / logits --------------------------
        o1T = np.ascontiguousarray(out1.T)           # [HC, C*SH] f16
        w2 = np.asarray(W2, np.float32)
        wal2 = np.concatenate(
            [_fold_att(w2, np.asarray(a_src2, np.float32)),
             _fold_att(w2, np.asarray(a_dst2, np.float32))], axis=1)
        w2all = np.concatenate([w2[perm], wal2[perm]], axis=1)  # [HC, 66]
        mapsP2 = [
            {"xT": np.ascontiguousarray(o1T[:, k * SH:(k + 1) * SH]),
             "w": w2all.astype(np.float16)} for k in range(C)]
        # out1 already carries b1 (E1 bias_out); P2 applies the ELU
        ncP2 = self.kernel("P2", c_in=HC, m_h=OUT_C, m_al=2, elu=True,
                           bias_in=False)
        resP2 = self._run("P2", ncP2, mapsP2)
        h2al = np.concatenate([r["hT"] for r in resP2], axis=1)  # [66, Np]
        valid = rowmap >= 0
        vrows = rowmap[valid]
        h2 = np.zeros((N, OUT_C), dtype=np.float16)
        h2[vrows] = h2al[:OUT_C].T[valid]
        als2 = np.zeros((N, 1), dtype=np.float16)
        als2[vrows, 0] = h2al[OUT_C][valid]
        ald2 = np.zeros((N, 1), dtype=np.float16)
        ald2[vrows, 0] = h2al[OUT_C + 1][valid]

        # ---- E2: layer-2 edge aggregation ----------------------------
        b2nz = bool(np.any(np.asarray(b2)))
        mapsE2 = []
        for k in range(C):
            a_s = g.stream_als(als2, k).reshape(P, g.TOT)
            a_d = g.stream_ald_exp(ald2, k).reshape(P, g.TOT)
            m = {"hsrc": g.stream_h(h2, k),
                 "als": np.ascontiguousarray(
                     np.stack([a_s, a_d], axis=2)).reshape(P, g.TOT * 2),
                 "ident": id8}
            if b2nz:
                m["brep"] = np.tile(np.asarray(b2, np.float32), (P, 1))
            mapsE2.append(m)
        ncE2 = self.kernel("E2", bias_out=b2nz)
        resE2 = self._run("E2", ncE2, mapsE2)
        out2 = np.concatenate(
            [r["out"].reshape(P, g.wpc, OUT_C).transpose(1, 0, 2)
             .reshape(g.wpc * P, OUT_C) for r in resE2], axis=0)
        out_full = np.zeros((N, OUT_C), dtype=np.float32)
        out_full[vrows] = out2[valid]
        return out_full


_RUNNER = _GatRunner()


def kernel(x, edge_index, W1, a_src1, a_dst1, b1, W2, a_src2, a_dst2, b2):
    """Full-input / full-output entry point. Returns [N, OUT_C] float32."""
    args = [np.asarray(v) for v in
            (x, edge_index, W1, a_src1, a_dst1, b1, W2, a_src2, a_dst2, b2)]
    return _RUNNER.run(*args).astype(np.float32)
